# revision 44
# baseline (speedup 1.0000x reference)
"""Trainium2 Bass kernel for nn_Attention (dense_transformer), v3.

Reference computation (per batch n of 4):
  qkv = W_qkv @ x + b          (384, 4096)   [x flattened to (256, 64*64)]
  raw C-order reinterpret of qkv flat buffer as (4096, 384) -> q|k|v (4096,128)
  scores = q @ k.T / 64        (4096, 4096)
  soft = softmax(scores, axis=-2)             [column softmax]
  out = soft @ v               (4096, 128)
  raw reinterpret of out as (128, 4096)
  y = W_out @ out2 + b_out     (256, 4096)

Sharding: 8 cores = 4 batches x 2 j-halves (t-halves of the permuted j
enumeration; the host-side 192-rotation of qkv channels and 64-rotation of
W_out's e-axis make the SPMD program identical on all cores). Host sums the
per-pair partial y.

v3 dataflow (vs v2): the q/k tensors are quantized to fp8e4 at the stage-1
drain and the score matmuls run in DoubleRow perf mode (2 fp8 rows per PE
pass, 2x throughput): contraction d=128 is split into two 64-partition
groups, with the d-hi half moved onto partitions 0:64 by SBUF->SBUF DMAs
(idle engines). exp runs on ACT with accum_out supplying the column-softmax
Z sums for most chunks; a subset of chunks uses a Schraudolph fast-exp on
DVE (bf16 bits = S*128*log2e/64 + 16249.7 via f32->u16 convert, written
through a bitcast view) plus a DVE row-reduce, to keep both engines busy.
v transposes ride the DMA XBAR instead of the PE. Bias enters PSUM via
ones-row matmuls so all stage-1 drains are plain copies. Out accumulation
rotates the 4 spare PSUM banks in two eras split at jb10: groups 4-7 live
jb0-10 then spill; groups 0-3 chain burst(jb0-10)+live(jb11-15); groups
4-7 finish jb11-15 in the tail on psA's freed banks, adding the spill at
drain. proj2 folds the psi_q permutation into stride-3 rhs gathers.
"""

import numpy as np
import ml_dtypes

import concourse.bass as bass
import concourse.bacc as bacc
import concourse.mybir as mybir
from concourse.bass_utils import run_bass_kernel_spmd
from concourse.tile import TileContext, add_dep_helper
from concourse.masks import make_identity
from concourse.alu_op_type import AluOpType

BF16 = mybir.dt.bfloat16
F32 = mybir.dt.float32
FP8 = mybir.dt.float8e4
U16 = mybir.dt.uint16
AF = mybir.ActivationFunctionType
AX = mybir.AxisListType
DR = mybir.MatmulPerfMode.DoubleRow

N, C, E, O, HW = 4, 256, 128, 384, 4096
JC = HW // 2          # j per core
NJB = JC // 128       # 16 j-blocks
SCALE = 1.0 / 64.0    # 1/sqrt(HW)
SPLIT_JB = 10         # era split for out accumulation
DIRECT_CIDX = 8       # chunks below this use unsplit fp8 scores
WARM_MMS = 16
SCHR_CIDX2 = None
FLUSH_BUDGET = 3

# Schraudolph fast-exp on DVE for a subset of chunks: bf16 bits of e^x are
# ~ x*(128*log2e) + 16249.7 (HW rounds on f32->u16 convert; rel rms ~1.8%).
SCHR_A = 128.0 * 1.4426950408889634 * SCALE
SCHR_B = 16249.7
# Schraudolph chunk set: spread over the h2 sweep and the h1 slots of the
# jb-minor phase, avoiding stats-critical h3 chunks and DVE-heavy clusters.
SCHR_CIDX = {16, 19, 22, 33, 37, 41, 45, 49, 53, 57, 61}

_CACHE = {}


def _psiq_inv(m):
    if m <= 10:
        return 3 * m
    if m <= 21:
        return 3 * (m - 11) + 1
    return 3 * (m - 22) + 2


def _proj2_runs(G):
    """Maximal stride-3 source-chunk runs feeding y columns [4G*128,(4G+4)*128)."""
    srcs = [_psiq_inv(4 * G + k) for k in range(4)]
    runs = []
    for s in srcs:
        if runs and s == runs[-1][-1] + 3:
            runs[-1].append(s)
        else:
            runs.append([s])
    return runs


def build_nc(debug_hook=None):
    nc = bacc.Bacc("TRN2", target_bir_lowering=False, debug=False, num_devices=8)

    x_ext = nc.dram_tensor("x", [C, HW], BF16, kind="ExternalInput").ap()
    wqkvT_ext = nc.dram_tensor("wqkvT", [C, 768], BF16, kind="ExternalInput").ap()
    brow_ext = nc.dram_tensor("brow", [1, 768], BF16, kind="ExternalInput").ap()
    woutT_ext = nc.dram_tensor("woutT", [E, C], BF16, kind="ExternalInput").ap()
    bout_ext = nc.dram_tensor("bout", [C, 1], F32, kind="ExternalInput").ap()
    y_ext = nc.dram_tensor("out", [C, HW], BF16, kind="ExternalOutput").ap()

    # persistent SBUF
    xsb = [nc.alloc_sbuf_tensor(f"x{cb}", [128, HW], BF16).ap() for cb in range(2)]
    QK8s = nc.alloc_sbuf_tensor("QK8s", [128, 2 * 6144], FP8).ap()
    vsb = nc.alloc_sbuf_tensor("vsb", [128, JC], BF16).ap()
    P = nc.alloc_sbuf_tensor("P", [128, NJB * HW], BF16).ap()
    outTa = nc.alloc_sbuf_tensor("outTa", [128, HW], BF16).ap()
    out2a = nc.alloc_sbuf_tensor("out2a", [128, HW], BF16).ap()
    spill = [nc.alloc_sbuf_tensor(f"spill{g}", [128, 512], F32).ap() for g in range(4)]
    zacc = nc.alloc_sbuf_tensor("zacc", [128, 64], F32).ap()
    zsum = nc.alloc_sbuf_tensor("zsum", [128, 16], F32).ap()
    zinv = nc.alloc_sbuf_tensor("zinv", [128, 16], F32).ap()


    # gathered layout: per g-half, q contiguous (4096) then k (2048)
    QK8sg = QK8s[0:64, :].rearrange("p (g c) -> p g c", g=2)

    def v_sl(jb):
        return vsb[:, jb * 128:(jb + 1) * 128]

    with TileContext(nc) as tc:
        with tc.tile_pool(name="consts", bufs=1) as consts:
            # ---- weights/constants ----
            nc.sync.dma_start(out=xsb[0][:, 0:512], in_=x_ext[0:128, 0:512])
            nc.sync.dma_start(out=xsb[1][:, 0:512], in_=x_ext[128:256, 0:512])
            brow = consts.tile([1, 768], BF16, name="brow", tag="brow")
            nc.sync.dma_start(out=brow, in_=brow_ext[:])
            wq_all = consts.tile([128, 2 * 768], BF16, name="wq_all", tag="wq_all")
            for cb in range(2):
                nc.sync.dma_start(out=wq_all[:, cb * 768:(cb + 1) * 768],
                                  in_=wqkvT_ext[cb * 128:(cb + 1) * 128, :])
            ones1 = consts.tile([1, 128], BF16, name="ones1", tag="ones1")
            nc.vector.memset(ones1[:], 1.0)

            def wq_sl(cb, r):
                return wq_all[:, cb * 768 + r * 256: cb * 768 + (r + 1) * 256]

            misc = consts.tile([128, C + 128], BF16, name="misc", tag="misc")
            woutT = misc[:, 0:C]
            ident = misc[:, C:C + 128]
            nc.gpsimd.dma_start(out=woutT, in_=woutT_ext[:])
            make_identity(nc, ident)
            bo2 = consts.tile([128, 2], F32, name="bo2", tag="bo2")
            bo = [bo2[:, cb:cb + 1] for cb in range(2)]
            for cb in range(2):
                nc.gpsimd.dma_start(out=bo[cb], in_=bout_ext[cb * 128:(cb + 1) * 128, :])
            # Exp table preload
            scratch = consts.tile([128, 1], F32, name="scratch", tag="scratch")
            nc.vector.memset(scratch[:], 0.0)
            nc.scalar.activation(scratch[:], scratch[:], AF.Exp)

            # ---- PE warmup (p-state ramp) ----
            wsrc = consts.tile([128, 128], BF16, name="wsrc", tag="wsrc")
            nc.vector.memset(wsrc[:], 1.0)
            with tc.tile_pool(name="psW", bufs=1, space="PSUM") as psW:
                wtile = psW.tile([128, 128], F32, tag="warm")
                for _ in range(WARM_MMS):
                    nc.tensor.matmul(wtile[:], wsrc[:], wsrc[:], start=True, stop=True)

            # ---- x loads: all on the HWDGE sync ring, 1024-col chunks
            #      interleaved cb0/cb1 in stage-1 consumption order ----
            for lo, hi in ((512, 1536), (1536, 2560), (2560, 3584), (3584, 4096)):
                nc.sync.dma_start(out=xsb[0][:, lo:hi], in_=x_ext[0:128, lo:hi])
                nc.sync.dma_start(out=xsb[1][:, lo:hi],
                                  in_=x_ext[128:256, lo:hi])

            # ---- phase A chunk order: h-sweeps over jb0-7 (software-
            #      pipelined into stage 1), then jb8-15 h-minor. ----
            order = []
            for h in range(4):
                for jb in range(8):
                    order.append((jb, h))
            for jb in range(8, 16):
                for h in range(4):
                    order.append((jb, h))

            psA_cm = tc.tile_pool(name="psA", bufs=2, space="PSUM")
            psA = psA_cm.__enter__()
            stg_cm = tc.tile_pool(name="stg", bufs=1)
            stg = stg_cm.__enter__()
            Qtmp = stg.tile([128, 32 * 128], FP8, name="Qtmp", tag="Qtmp")
            Ktmp = stg.tile([128, 32 * 64], FP8, name="Ktmp", tag="Ktmp")
            Vf = stg.tile([128, 32 * 64], BF16, name="Vf", tag="Vf")
            Vv = Vf.rearrange("p (b c) -> p b c", c=64)
            holder = {}
            bankX = [None] * 4   # i'-groups 4-7: live jb0..SPLIT_JB
            bankY = [None] * 4   # i'-groups 0-3: burst+live, stop jb15
            pe_q = []   # (ready_chunk_idx, emit_fn): deferred PE MMs

            def flush(cidx, budget=4):
                n = 0
                while pe_q and pe_q[0][0] <= cidx and n < budget:
                    pe_q.pop(0)[1]()
                    n += 1

            def stats(jb):
                nc.vector.reduce_sum(
                    out=zsum[:, jb:jb + 1], in_=zacc[:, jb * 4:(jb + 1) * 4],
                    axis=AX.X)
                nc.vector.reciprocal(zinv[:, jb:jb + 1], zsum[:, jb:jb + 1])
                nc.vector.tensor_scalar_mul(v_sl(jb), v_sl(jb),
                                            zinv[:, jb:jb + 1])

            def out_mm(bank, g, jb, start, stop):
                nc.tensor.matmul(
                    bank[:], v_sl(jb),
                    P[:, jb * HW + g * 512: jb * HW + (g + 1) * 512],
                    start=start, stop=stop,
                )

            def emit_chunk(cidx):
                jb, h = order[cidx]
                if cidx == 63:
                    # reuse chunk 62's banks so psA's other buffer frees
                    # early for the X tail chains
                    pa = holder["prev_pa"]
                else:
                    pa = psA.tile([128, 1024], F32, name="pa", tag="pa")
                    holder["prev_pa"] = pa
                for n2 in range(2):
                    if cidx < 16:
                        # unsplit fp8 (128-partition contraction, one 64-wide
                        # k block per MM, stacked via tile_position):
                        # independent of the gather DMAs
                        for t in range(2):
                            nc.tensor.matmul(
                                pa[64 * t:64 * t + 64,
                                   n2 * 512:(n2 + 1) * 512],
                                Ktmp[:, (2 * jb + t) * 64:
                                     (2 * jb + t + 1) * 64],
                                Qtmp[:, (8 * h + 4 * n2) * 128:
                                     (8 * h + 4 * n2 + 4) * 128],
                                start=True, stop=True,
                                tile_position=(0, 64 * t),
                            )
                    else:
                        o = (8 * h + 4 * n2) * 128
                        nc.tensor.matmul(
                            pa[:, n2 * 512:(n2 + 1) * 512],
                            QK8sg[:, :, 4096 + jb * 128:4096 + (jb + 1) * 128],
                            QK8sg[:, :, o:o + 512],
                            start=True, stop=True,
                            perf_mode=DR,
                        )
                psl = P[:, jb * HW + h * 1024: jb * HW + (h + 1) * 1024]
                zc = zacc[:, jb * 4 + h: jb * 4 + h + 1]
                # Z per chunk: ACT accum_out (free row-sums) for most chunks;
                # DVE Schraudolph exp + reduce for a subset.
                if cidx in SCHR_CIDX:
                    nc.vector.tensor_scalar(
                        out=psl.bitcast(U16), in0=pa[:],
                        scalar1=SCHR_A, scalar2=SCHR_B,
                        op0=AluOpType.mult, op1=AluOpType.add)
                    nc.vector.reduce_sum(out=zc, in_=psl, axis=AX.X)
                else:
                    nc.scalar.activation(out=psl, in_=pa[:], func=AF.Exp,
                                         scale=SCALE, accum_out=zc)
                flush(cidx, budget=FLUSH_BUDGET)
                if h == 3:
                    stats(jb)
                    psBi = holder["psBi"]
                    if jb == 0:
                        for g in range(4):
                            bankX[g] = psBi.tile([128, 512], F32,
                                                 name=f"bk{g}", tag=f"bk{g}")
                    if jb <= SPLIT_JB:
                        for g in range(4):
                            pe_q.append((cidx + 2,
                                         (lambda g=g, jb=jb: out_mm(
                                             bankX[g], g + 4, jb,
                                             jb == 0, jb == SPLIT_JB))))
                    else:
                        for g in range(4):
                            pe_q.append((cidx + 4,
                                         (lambda g=g, jb=jb: out_mm(
                                             bankY[g], g, jb,
                                             False, jb == 15))))
                    if jb == SPLIT_JB:
                        # spill X banks; queue Y bursts (groups 0-3)
                        # jb0..SPLIT_JB from persistent P.
                        def spill_and_y():
                            for g in range(4):
                                nc.vector.tensor_copy(spill[g][:],
                                                      bankX[g][:])
                            for g in range(4):
                                bankY[g] = psBi.tile([128, 512], F32,
                                                     name=f"bk{g}",
                                                     tag=f"bk{g}")
                        pe_q.append((cidx + 2, spill_and_y))
                        for jbq in range(SPLIT_JB + 1):
                            for g in range(4):
                                pe_q.append((cidx + 2 + jbq // 2,
                                             (lambda g=g, jbq=jbq: out_mm(
                                                 bankY[g], g, jbq,
                                                 jbq == 0, False))))

            # chunks emitted between stage-1 quads (deps: k needs its jb's
            # quads, q needs quads 2h,2h+1; all cidx<16 are gather-free)
            INTER = {1: [0, 1, 2, 3], 2: [4, 5], 3: [6, 7, 8, 9, 10, 11],
                     4: [12, 13], 5: [14, 15]}

            # ---- stage 1: quads of FT blocks -> Qtmp/Ktmp (fp8) + Vf
            #      (bf16), gather DMAs -> QK8s, XBAR -> vsb; early phase-A
            #      chunks interleave (psF on banks 4-7, psA on 0-3). ----
            with tc.tile_pool(name="psF", bufs=2, space="PSUM") as psF:
                for q in range(8):
                    pf = psF.tile([128, 1024], F32, name="pf", tag="pf")
                    pf3 = pf.rearrange("p (s c) -> p s c", s=4)
                    for s in range(4):
                        sc = 4 * q + s
                        r = sc % 3
                        nc.tensor.matmul(pf3[:, s, 0:256], ones1[:],
                                         brow[:, r * 256:(r + 1) * 256],
                                         start=True, stop=False)
                        for cb in range(2):
                            nc.tensor.matmul(
                                pf3[:, s, 0:256],
                                xsb[cb][:, sc * 128:(sc + 1) * 128],
                                wq_sl(cb, r),
                                start=False, stop=(cb == 1),
                            )
                    cp = nc.vector.tensor_copy
                    cp(Qtmp[:, q * 512:(q + 1) * 512], pf3[:, :, 0:128])
                    cp(Ktmp[:, q * 256:(q + 1) * 256], pf3[:, :, 128:192])
                    # v tokens for k-block sc live in v-block sc+1: store the
                    # v drain one block down (with wrap) so vsb aligns with kT.
                    if q == 0:
                        cp(Vv[:, 31:32, :], pf3[:, 0:1, 192:256])
                        cp(Vv[:, 0:3, :], pf3[:, 1:4, 192:256])
                    else:
                        cp(Vv[:, 4 * q - 1:4 * q + 3, :], pf3[:, :, 192:256])
                    # gather DMAs per 16-block round: contiguous q/k copies
                    # of each d-half (g=1 is the partition-shifted 64:128
                    # half) into the per-g layout [q 4096 | k 2048].
                    if q % 4 == 3:
                        bg = q // 4
                        for g in range(2):
                            nc.sync.dma_start(
                                out=QK8s[0:64, g * 6144 + bg * 2048:
                                         g * 6144 + (bg + 1) * 2048],
                                in_=Qtmp[64 * g:64 * g + 64,
                                         bg * 2048:(bg + 1) * 2048])
                            nc.scalar.dma_start(
                                out=QK8s[0:64, g * 6144 + 4096 + bg * 1024:
                                         g * 6144 + 4096 + (bg + 1) * 1024],
                                in_=Ktmp[64 * g:64 * g + 64,
                                         bg * 1024:(bg + 1) * 1024])
                    # XBAR transposes: vsb[:, jb, :] = Vf[:, jb, :].T, in
                    # two halves so jb0-7 stats don't wait on all drains
                    if q == 4:
                        nc.scalar.dma_start_transpose(
                            out=vsb[:, 0:1024].rearrange("p (b c) -> p b c",
                                                         c=128),
                            in_=Vf[:, 0:1024])
                    if q == 7:
                        nc.scalar.dma_start_transpose(
                            out=vsb[:, 1024:2048].rearrange(
                                "p (b c) -> p b c", c=128),
                            in_=Vf[:, 1024:2048])
                    for c in INTER.get(q, ()):
                        emit_chunk(c)

            stg_cm.__exit__(None, None, None)

            # ---- phase A remainder ----
            with tc.tile_pool(name="psBi", bufs=1, space="PSUM") as psBi:
                holder["psBi"] = psBi
                for cidx in range(16, 64):
                    emit_chunk(cidx)

                # X g4/g5 tail chains pre-run on psA's freed second buffer
                # (only need exps through jb14 and the banks chunk 61 used).
                paX = psA.tile([128, 1024], F32, name="pa", tag="pa")
                for g in range(2):
                    for jb in range(SPLIT_JB + 1, 15):
                        out_mm(paX[:, g * 512:(g + 1) * 512].__class__ is None
                               and None or paX[:, g * 512:(g + 1) * 512],
                               g + 4, jb, jb == SPLIT_JB + 1, False)
                # Y jb15 MMs and leftovers
                while pe_q:
                    pe_q.pop(0)[1]()
                # Y (groups 0-3) drain to outTa.
                for g in range(4):
                    if g % 2 == 0:
                        nc.scalar.copy(outTa[:, g * 512:(g + 1) * 512],
                                       bankY[g][:])
                    else:
                        nc.vector.tensor_copy(
                            outTa[:, g * 512:(g + 1) * 512], bankY[g][:])
                # X g4/g5 jb15 stop + drain (spill added) while pools are open
                for g in range(2):
                    out_mm(paX[:, g * 512:(g + 1) * 512], g + 4, 15,
                           False, True)
                for g in range(2):
                    nc.vector.tensor_tensor(
                        out=outTa[:, (g + 4) * 512:(g + 5) * 512],
                        in0=paX[:, g * 512:(g + 1) * 512], in1=spill[g][:],
                        op=AluOpType.add)
            psA_cm.__exit__(None, None, None)
            out2a3 = out2a.rearrange("p (b t) -> p b t", t=128)
            with tc.tile_pool(name="psA2", bufs=1, space="PSUM") as psA2, \
                 tc.tile_pool(name="psC", bufs=2, space="PSUM") as psC, \
                 tc.tile_pool(name="psY", bufs=4, space="PSUM") as psY, \
                 tc.tile_pool(name="late", bufs=2) as late:

                # proj2 Gs complete in pairs; each pair's two y-slices per
                # cb ride one strided DMA to halve the descriptor train.
                PAIRS = [(0, 3), (6, 1), (4, 7), (2, 5)]
                pair_of = {g: (pi, hi) for pi, p in enumerate(PAIRS)
                           for hi, g in enumerate(p)}
                ygt = {}

                def proj2(G):
                    runs = _proj2_runs(G)
                    pi, hi = pair_of[G]
                    for cb in range(2):
                        py = psY.tile([128, 512], F32, name="py", tag="py")
                        off = 0
                        for ri, run in enumerate(runs):
                            w = 128 * len(run)
                            rhs = out2a3[:, run[0]:run[-1] + 1:3, :]
                            nc.tensor.matmul(
                                py[:, off:off + w],
                                woutT[:, cb * 128:(cb + 1) * 128], rhs,
                                start=(ri == 0), stop=(ri == len(runs) - 1),
                                skip_group_check=True,
                            )
                            off += w
                        key = (pi % 2, cb)
                        if hi == 0:
                            ygt[(pi, cb)] = late.tile(
                                [128, 1024], BF16, name=f"yg{key}",
                                tag=f"yg{key}")
                        yg = ygt[(pi, cb)]
                        lo, hc = (min(PAIRS[pi]), PAIRS[pi][hi] != min(PAIRS[pi]))
                        sl = yg[:, 512:1024] if hc else yg[:, 0:512]
                        if cb == 0:
                            nc.scalar.activation(sl, py[:], AF.Identity,
                                                 bias=bo[cb])
                        else:
                            nc.vector.tensor_scalar_add(sl, py[:], bo[cb])
                        if hi == 1:
                            d = (max(PAIRS[pi]) - lo) * 512
                            outap = y_ext[cb * 128:(cb + 1) * 128,
                                          lo * 512:].rearrange(
                                "p (a c) -> p a c", c=512)[:, 0:d // 512 + 1:
                                                           d // 512, :]
                            eng = [nc.sync, nc.scalar, nc.gpsimd][
                                (pi * 2 + cb) % 3]
                            eng.dma_start(out=outap, in_=yg[:])

                def tp(g):
                    tpc = psC.tile([128, 512], BF16, name="tpc", tag="tpc")
                    for s in range(4):
                        nc.tensor.transpose(
                            tpc[:, s * 128:(s + 1) * 128],
                            outTa[:, g * 512 + s * 128:
                                  g * 512 + (s + 1) * 128],
                            ident)
                    if g % 2 == 0:
                        nc.scalar.copy(out2a[:, g * 512:(g + 1) * 512], tpc[:])
                    else:
                        nc.vector.tensor_scalar_add(
                            out2a[:, g * 512:(g + 1) * 512], tpc[:], 0.0)

                # Y groups (0-3) completed at phase-A end.
                for g in range(4):
                    tp(g)
                proj2(0)
                proj2(3)
                # X tails: groups 4-7 chain jb SPLIT_JB+1..15 on psA's freed
                # banks (2 alternating), draining with the spill added; the
                # emission interleaves chains / drains / tps / proj2s so the
                # PE never waits on a just-emitted drain.
                bA2 = [None] * 4

                def xchain(g):
                    bA2[g] = psA2.tile([128, 512], F32, name=f"bA2{g % 2}",
                                       tag=f"bA2{g % 2}")
                    for jb in range(SPLIT_JB + 1, 16):
                        out_mm(bA2[g], g + 4, jb, jb == SPLIT_JB + 1, jb == 15)

                def xdrain(g):
                    nc.vector.tensor_tensor(
                        out=outTa[:, (g + 4) * 512:(g + 5) * 512],
                        in0=bA2[g][:], in1=spill[g][:], op=AluOpType.add)
                    tp(g + 4)

                # Y groups done; transposes for g0-5 then staggered proj2s
                for g in range(4):
                    tp(g)
                proj2(0)
                proj2(3)
                tp(4)
                tp(5)
                proj2(6)      # srcs g{2,2,3,4}
                bA2 = [None] * 4

                def xchain(g):
                    bA2[g] = psA2.tile([128, 512], F32, name=f"bA2{g % 2}",
                                       tag=f"bA2{g % 2}")
                    for jb in range(SPLIT_JB + 1, 16):
                        out_mm(bA2[g], g + 4, jb, jb == SPLIT_JB + 1, jb == 15)

                def xdrain(g):
                    nc.vector.tensor_tensor(
                        out=outTa[:, (g + 4) * 512:(g + 5) * 512],
                        in0=bA2[g][:], in1=spill[g][:], op=AluOpType.add)
                    tp(g + 4)

                xchain(2)
                proj2(1)      # g{3,3,4,5}
                xchain(3)
                xdrain(2)
                proj2(4)      # g{4,4,5,6}
                xdrain(3)
                # remaining: G7 g{5,5,6,7}, G2 g{6,6,7,0}, G5 g{7,7,0,1}
                for G in (7, 2, 5):
                    proj2(G)

        if debug_hook:
            debug_hook(nc, dict(QK8s=QK8s, vsb=vsb,
                                P=P, zsum=zsum, zinv=zinv, outTa=outTa,
                                out2a=out2a))

    nc.compile()
    return nc


def get_nc():
    if "nc" not in _CACHE:
        _CACHE["nc"] = build_nc()
    return _CACHE["nc"]


def make_in_maps(x, W_qkv, b_qkv, W_out, b_out):
    x = np.asarray(x, dtype=np.float32)
    W_qkv = np.asarray(W_qkv, dtype=np.float32)
    b_qkv = np.asarray(b_qkv, dtype=np.float32)
    W_out = np.asarray(W_out, dtype=np.float32)
    b_out = np.asarray(b_out, dtype=np.float32)

    operm = (np.arange(O) + O // 2) % O      # rotate qkv channels by 192
    eperm = (np.arange(E) + E // 2) % E      # rotate e-axis by 64

    halves = []
    for h in range(2):
        if h == 0:
            wq, bqv, wo, bov = W_qkv, b_qkv, W_out, b_out
        else:
            wq = W_qkv[operm]
            bqv = b_qkv[operm]
            wo = W_out[:, eperm]
            bov = np.zeros_like(b_out)
        orders = [
            [3 * t + r for t in range(128)]
            + [3 * t + (r + 2) % 3 for t in range(64)]
            + [3 * t + (r + 1) % 3 for t in range(64)]
            for r in range(3)
        ]
        wqv = np.concatenate([wq.T[:, o] for o in orders], axis=1)     # (C, 768)
        brv = np.concatenate([bqv[o][None, :] for o in orders], axis=1)  # (1, 768)
        halves.append({
            "wqkvT": np.ascontiguousarray(wqv).astype(ml_dtypes.bfloat16),
            "brow": np.ascontiguousarray(brv).astype(ml_dtypes.bfloat16),
            "woutT": np.ascontiguousarray(wo.T).astype(ml_dtypes.bfloat16),
            "bout": np.ascontiguousarray(bov.reshape(C, 1)),
        })

    xb = [np.ascontiguousarray(x[n].reshape(C, HW)).astype(ml_dtypes.bfloat16)
          for n in range(N)]
    in_maps = []
    for core in range(8):
        n, h = core // 2, core % 2
        m = {"x": xb[n]}
        m.update(halves[h])
        in_maps.append(m)
    return in_maps


def run(inputs, trace=False, **kw):
    nc = get_nc()
    in_maps = make_in_maps(**inputs)
    res = run_bass_kernel_spmd(nc, in_maps, core_ids=list(range(8)), trace=trace, **kw)
    ys = [np.asarray(res.results[i]["out"], dtype=np.float32) for i in range(8)]
    y = np.stack([ys[2 * n] + ys[2 * n + 1] for n in range(N)])
    return y.reshape(N, C, 64, 64), res


def kernel(**inputs):
    y, _ = run(inputs, trace=False)
    return y


# revision 45
# speedup vs baseline: 1.0251x; 1.0251x over previous
"""Trainium2 Bass kernel for nn_Attention (dense_transformer), v3.

Reference computation (per batch n of 4):
  qkv = W_qkv @ x + b          (384, 4096)   [x flattened to (256, 64*64)]
  raw C-order reinterpret of qkv flat buffer as (4096, 384) -> q|k|v (4096,128)
  scores = q @ k.T / 64        (4096, 4096)
  soft = softmax(scores, axis=-2)             [column softmax]
  out = soft @ v               (4096, 128)
  raw reinterpret of out as (128, 4096)
  y = W_out @ out2 + b_out     (256, 4096)

Sharding: 8 cores = 4 batches x 2 j-halves (t-halves of the permuted j
enumeration; the host-side 192-rotation of qkv channels and 64-rotation of
W_out's e-axis make the SPMD program identical on all cores). Host sums the
per-pair partial y.

v3 dataflow (vs v2): the q/k tensors are quantized to fp8e4 at the stage-1
drain and the score matmuls run in DoubleRow perf mode (2 fp8 rows per PE
pass, 2x throughput): contraction d=128 is split into two 64-partition
groups, with the d-hi half moved onto partitions 0:64 by SBUF->SBUF DMAs
(idle engines). exp runs on ACT with accum_out supplying the column-softmax
Z sums for most chunks; a subset of chunks uses a Schraudolph fast-exp on
DVE (bf16 bits = S*128*log2e/64 + 16249.7 via f32->u16 convert, written
through a bitcast view) plus a DVE row-reduce, to keep both engines busy.
v transposes ride the DMA XBAR instead of the PE. Bias enters PSUM via
ones-row matmuls so all stage-1 drains are plain copies. Out accumulation
rotates the 4 spare PSUM banks in two eras split at jb10: groups 4-7 live
jb0-10 then spill; groups 0-3 chain burst(jb0-10)+live(jb11-15); groups
4-7 finish jb11-15 in the tail on psA's freed banks, adding the spill at
drain. proj2 folds the psi_q permutation into stride-3 rhs gathers.
"""

import numpy as np
import ml_dtypes

import concourse.bass as bass
import concourse.bacc as bacc
import concourse.mybir as mybir
from concourse.bass_utils import run_bass_kernel_spmd
from concourse.tile import TileContext, add_dep_helper
from concourse.masks import make_identity
from concourse.alu_op_type import AluOpType

BF16 = mybir.dt.bfloat16
F32 = mybir.dt.float32
FP8 = mybir.dt.float8e4
U16 = mybir.dt.uint16
AF = mybir.ActivationFunctionType
AX = mybir.AxisListType
DR = mybir.MatmulPerfMode.DoubleRow

N, C, E, O, HW = 4, 256, 128, 384, 4096
JC = HW // 2          # j per core
NJB = JC // 128       # 16 j-blocks
SCALE = 1.0 / 64.0    # 1/sqrt(HW)
SPLIT_JB = 10         # era split for out accumulation
DIRECT_CIDX = 8       # chunks below this use unsplit fp8 scores
WARM_MMS = 16
SCHR_CIDX2 = None
FLUSH_BUDGET = 3

# Schraudolph fast-exp on DVE for a subset of chunks: bf16 bits of e^x are
# ~ x*(128*log2e) + 16249.7 (HW rounds on f32->u16 convert; rel rms ~1.8%).
SCHR_A = 128.0 * 1.4426950408889634 * SCALE
SCHR_B = 16249.7
# Schraudolph chunk set: spread over the h2 sweep and the h1 slots of the
# jb-minor phase, avoiding stats-critical h3 chunks and DVE-heavy clusters.
SCHR_CIDX = {16, 19, 22, 33, 37, 41, 45, 49, 53, 57, 61}

_CACHE = {}


def _psiq_inv(m):
    if m <= 10:
        return 3 * m
    if m <= 21:
        return 3 * (m - 11) + 1
    return 3 * (m - 22) + 2


def _proj2_runs(G):
    """Maximal stride-3 source-chunk runs feeding y columns [4G*128,(4G+4)*128)."""
    srcs = [_psiq_inv(4 * G + k) for k in range(4)]
    runs = []
    for s in srcs:
        if runs and s == runs[-1][-1] + 3:
            runs[-1].append(s)
        else:
            runs.append([s])
    return runs


def build_nc(debug_hook=None):
    nc = bacc.Bacc("TRN2", target_bir_lowering=False, debug=False, num_devices=8)

    x_ext = nc.dram_tensor("x", [C, HW], BF16, kind="ExternalInput").ap()
    wqkvT_ext = nc.dram_tensor("wqkvT", [C, 768], BF16, kind="ExternalInput").ap()
    brow_ext = nc.dram_tensor("brow", [1, 768], BF16, kind="ExternalInput").ap()
    woutT_ext = nc.dram_tensor("woutT", [E, C], BF16, kind="ExternalInput").ap()
    bout_ext = nc.dram_tensor("bout", [C, 1], F32, kind="ExternalInput").ap()
    y_ext = nc.dram_tensor("out", [C, HW], BF16, kind="ExternalOutput").ap()

    # persistent SBUF
    xsb = [nc.alloc_sbuf_tensor(f"x{cb}", [128, HW], BF16).ap() for cb in range(2)]
    QK8s = nc.alloc_sbuf_tensor("QK8s", [128, 2 * 6144], FP8).ap()
    vsb = nc.alloc_sbuf_tensor("vsb", [128, JC], BF16).ap()
    P = nc.alloc_sbuf_tensor("P", [128, NJB * HW], BF16).ap()
    outTa = nc.alloc_sbuf_tensor("outTa", [128, HW], BF16).ap()
    out2a = nc.alloc_sbuf_tensor("out2a", [128, HW], BF16).ap()
    spill = [nc.alloc_sbuf_tensor(f"spill{g}", [128, 512], F32).ap() for g in range(4)]
    zacc = nc.alloc_sbuf_tensor("zacc", [128, 64], F32).ap()
    zsum = nc.alloc_sbuf_tensor("zsum", [128, 16], F32).ap()
    zinv = nc.alloc_sbuf_tensor("zinv", [128, 16], F32).ap()


    # gathered layout: per g-half, q contiguous (4096) then k (2048)
    QK8sg = QK8s[0:64, :].rearrange("p (g c) -> p g c", g=2)

    def v_sl(jb):
        return vsb[:, jb * 128:(jb + 1) * 128]

    with TileContext(nc) as tc:
        with tc.tile_pool(name="consts", bufs=1) as consts:
            # ---- weights/constants ----
            nc.sync.dma_start(out=xsb[0][:, 0:512], in_=x_ext[0:128, 0:512])
            nc.sync.dma_start(out=xsb[1][:, 0:512], in_=x_ext[128:256, 0:512])
            brow = consts.tile([1, 768], BF16, name="brow", tag="brow")
            nc.sync.dma_start(out=brow, in_=brow_ext[:])
            wq_all = consts.tile([128, 2 * 768], BF16, name="wq_all", tag="wq_all")
            for cb in range(2):
                nc.sync.dma_start(out=wq_all[:, cb * 768:(cb + 1) * 768],
                                  in_=wqkvT_ext[cb * 128:(cb + 1) * 128, :])
            ones1 = consts.tile([1, 128], BF16, name="ones1", tag="ones1")
            nc.vector.memset(ones1[:], 1.0)

            def wq_sl(cb, r):
                return wq_all[:, cb * 768 + r * 256: cb * 768 + (r + 1) * 256]

            misc = consts.tile([128, C + 128], BF16, name="misc", tag="misc")
            woutT = misc[:, 0:C]
            ident = misc[:, C:C + 128]
            nc.gpsimd.dma_start(out=woutT, in_=woutT_ext[:])
            make_identity(nc, ident)
            bo2 = consts.tile([128, 2], F32, name="bo2", tag="bo2")
            bo = [bo2[:, cb:cb + 1] for cb in range(2)]
            for cb in range(2):
                nc.gpsimd.dma_start(out=bo[cb], in_=bout_ext[cb * 128:(cb + 1) * 128, :])
            # Exp table preload
            scratch = consts.tile([128, 1], F32, name="scratch", tag="scratch")
            nc.vector.memset(scratch[:], 0.0)
            nc.scalar.activation(scratch[:], scratch[:], AF.Exp)

            # ---- PE warmup (p-state ramp) ----
            wsrc = consts.tile([128, 128], BF16, name="wsrc", tag="wsrc")
            nc.vector.memset(wsrc[:], 1.0)
            with tc.tile_pool(name="psW", bufs=1, space="PSUM") as psW:
                wtile = psW.tile([128, 128], F32, tag="warm")
                for _ in range(WARM_MMS):
                    nc.tensor.matmul(wtile[:], wsrc[:], wsrc[:], start=True, stop=True)

            # ---- x loads: all on the HWDGE sync ring, 1024-col chunks
            #      interleaved cb0/cb1 in stage-1 consumption order ----
            for lo, hi in ((512, 1536), (1536, 2560), (2560, 3584), (3584, 4096)):
                nc.sync.dma_start(out=xsb[0][:, lo:hi], in_=x_ext[0:128, lo:hi])
                nc.sync.dma_start(out=xsb[1][:, lo:hi],
                                  in_=x_ext[128:256, lo:hi])

            # ---- phase A chunk order: h-sweeps over jb0-7 (software-
            #      pipelined into stage 1), then jb8-15 h-minor. ----
            order = []
            for h in range(4):
                for jb in range(8):
                    order.append((jb, h))
            for jb in range(8, 16):
                for h in range(4):
                    order.append((jb, h))

            psA_cm = tc.tile_pool(name="psA", bufs=2, space="PSUM")
            psA = psA_cm.__enter__()
            stg_cm = tc.tile_pool(name="stg", bufs=1)
            stg = stg_cm.__enter__()
            Qtmp = stg.tile([128, 32 * 128], FP8, name="Qtmp", tag="Qtmp")
            Ktmp = stg.tile([128, 32 * 64], FP8, name="Ktmp", tag="Ktmp")
            Vf = stg.tile([128, 32 * 64], BF16, name="Vf", tag="Vf")
            Vv = Vf.rearrange("p (b c) -> p b c", c=64)
            holder = {}
            bankX = [None] * 4   # i'-groups 4-7: live jb0..SPLIT_JB
            bankY = [None] * 4   # i'-groups 0-3: burst+live, stop jb15
            pe_q = []   # (ready_chunk_idx, emit_fn): deferred PE MMs

            def flush(cidx, budget=4):
                n = 0
                while pe_q and pe_q[0][0] <= cidx and n < budget:
                    pe_q.pop(0)[1]()
                    n += 1

            def stats(jb):
                nc.vector.reduce_sum(
                    out=zsum[:, jb:jb + 1], in_=zacc[:, jb * 4:(jb + 1) * 4],
                    axis=AX.X)
                nc.vector.reciprocal(zinv[:, jb:jb + 1], zsum[:, jb:jb + 1])
                nc.vector.tensor_scalar_mul(v_sl(jb), v_sl(jb),
                                            zinv[:, jb:jb + 1])

            def out_mm(bank, g, jb, start, stop):
                nc.tensor.matmul(
                    bank[:], v_sl(jb),
                    P[:, jb * HW + g * 512: jb * HW + (g + 1) * 512],
                    start=start, stop=stop,
                )

            def emit_chunk(cidx):
                jb, h = order[cidx]
                pa = psA.tile([128, 1024], F32, name="pa", tag="pa")
                for n2 in range(2):
                    if cidx < 16:
                        # unsplit fp8 (128-partition contraction, one 64-wide
                        # k block per MM, stacked via tile_position):
                        # independent of the gather DMAs
                        for t in range(2):
                            nc.tensor.matmul(
                                pa[64 * t:64 * t + 64,
                                   n2 * 512:(n2 + 1) * 512],
                                Ktmp[:, (2 * jb + t) * 64:
                                     (2 * jb + t + 1) * 64],
                                Qtmp[:, (8 * h + 4 * n2) * 128:
                                     (8 * h + 4 * n2 + 4) * 128],
                                start=True, stop=True,
                                tile_position=(0, 64 * t),
                            )
                    else:
                        o = (8 * h + 4 * n2) * 128
                        nc.tensor.matmul(
                            pa[:, n2 * 512:(n2 + 1) * 512],
                            QK8sg[:, :, 4096 + jb * 128:4096 + (jb + 1) * 128],
                            QK8sg[:, :, o:o + 512],
                            start=True, stop=True,
                            perf_mode=DR,
                        )
                psl = P[:, jb * HW + h * 1024: jb * HW + (h + 1) * 1024]
                zc = zacc[:, jb * 4 + h: jb * 4 + h + 1]
                # Z per chunk: ACT accum_out (free row-sums) for most chunks;
                # DVE Schraudolph exp + reduce for a subset.
                if cidx in SCHR_CIDX:
                    nc.vector.tensor_scalar(
                        out=psl.bitcast(U16), in0=pa[:],
                        scalar1=SCHR_A, scalar2=SCHR_B,
                        op0=AluOpType.mult, op1=AluOpType.add)
                    nc.vector.reduce_sum(out=zc, in_=psl, axis=AX.X)
                else:
                    nc.scalar.activation(out=psl, in_=pa[:], func=AF.Exp,
                                         scale=SCALE, accum_out=zc)
                flush(cidx, budget=FLUSH_BUDGET)
                if h == 3:
                    stats(jb)
                    psBi = holder["psBi"]
                    if jb == 0:
                        for g in range(4):
                            bankX[g] = psBi.tile([128, 512], F32,
                                                 name=f"bk{g}", tag=f"bk{g}")
                    if jb <= SPLIT_JB:
                        for g in range(4):
                            pe_q.append((cidx + 2,
                                         (lambda g=g, jb=jb: out_mm(
                                             bankX[g], g + 4, jb,
                                             jb == 0, jb == SPLIT_JB))))
                    else:
                        for g in range(4):
                            pe_q.append((cidx + 4,
                                         (lambda g=g, jb=jb: out_mm(
                                             bankY[g], g, jb,
                                             False, jb == 15))))
                    if jb == SPLIT_JB:
                        # spill X banks; queue Y bursts (groups 0-3)
                        # jb0..SPLIT_JB from persistent P.
                        def spill_and_y():
                            for g in range(4):
                                nc.vector.tensor_copy(spill[g][:],
                                                      bankX[g][:])
                            for g in range(4):
                                bankY[g] = psBi.tile([128, 512], F32,
                                                     name=f"bk{g}",
                                                     tag=f"bk{g}")
                        pe_q.append((cidx + 2, spill_and_y))
                        for jbq in range(SPLIT_JB + 1):
                            for g in range(4):
                                pe_q.append((cidx + 2 + jbq // 2,
                                             (lambda g=g, jbq=jbq: out_mm(
                                                 bankY[g], g, jbq,
                                                 jbq == 0, False))))

            # chunks emitted between stage-1 quads (deps: k needs its jb's
            # quads, q needs quads 2h,2h+1; all cidx<16 are gather-free)
            INTER = {1: [0, 1, 2, 3], 2: [4, 5], 3: [6, 7, 8, 9, 10, 11],
                     4: [12, 13], 5: [14, 15]}

            # ---- stage 1: quads of FT blocks -> Qtmp/Ktmp (fp8) + Vf
            #      (bf16), gather DMAs -> QK8s, XBAR -> vsb; early phase-A
            #      chunks interleave (psF on banks 4-7, psA on 0-3). ----
            with tc.tile_pool(name="psF", bufs=2, space="PSUM") as psF:
                for q in range(8):
                    pf = psF.tile([128, 1024], F32, name="pf", tag="pf")
                    pf3 = pf.rearrange("p (s c) -> p s c", s=4)
                    for s in range(4):
                        sc = 4 * q + s
                        r = sc % 3
                        nc.tensor.matmul(pf3[:, s, 0:256], ones1[:],
                                         brow[:, r * 256:(r + 1) * 256],
                                         start=True, stop=False)
                        for cb in range(2):
                            nc.tensor.matmul(
                                pf3[:, s, 0:256],
                                xsb[cb][:, sc * 128:(sc + 1) * 128],
                                wq_sl(cb, r),
                                start=False, stop=(cb == 1),
                            )
                    cp = nc.vector.tensor_copy
                    cp(Qtmp[:, q * 512:(q + 1) * 512], pf3[:, :, 0:128])
                    cp(Ktmp[:, q * 256:(q + 1) * 256], pf3[:, :, 128:192])
                    # v tokens for k-block sc live in v-block sc+1: store the
                    # v drain one block down (with wrap) so vsb aligns with kT.
                    if q == 0:
                        cp(Vv[:, 31:32, :], pf3[:, 0:1, 192:256])
                        cp(Vv[:, 0:3, :], pf3[:, 1:4, 192:256])
                    else:
                        cp(Vv[:, 4 * q - 1:4 * q + 3, :], pf3[:, :, 192:256])
                    # gather DMAs per 16-block round: contiguous q/k copies
                    # of each d-half (g=1 is the partition-shifted 64:128
                    # half) into the per-g layout [q 4096 | k 2048].
                    if q % 4 == 3:
                        bg = q // 4
                        for g in range(2):
                            nc.sync.dma_start(
                                out=QK8s[0:64, g * 6144 + bg * 2048:
                                         g * 6144 + (bg + 1) * 2048],
                                in_=Qtmp[64 * g:64 * g + 64,
                                         bg * 2048:(bg + 1) * 2048])
                            nc.scalar.dma_start(
                                out=QK8s[0:64, g * 6144 + 4096 + bg * 1024:
                                         g * 6144 + 4096 + (bg + 1) * 1024],
                                in_=Ktmp[64 * g:64 * g + 64,
                                         bg * 1024:(bg + 1) * 1024])
                    # XBAR transposes: vsb[:, jb, :] = Vf[:, jb, :].T, in
                    # two halves so jb0-7 stats don't wait on all drains
                    if q == 4:
                        nc.scalar.dma_start_transpose(
                            out=vsb[:, 0:1024].rearrange("p (b c) -> p b c",
                                                         c=128),
                            in_=Vf[:, 0:1024])
                    if q == 7:
                        nc.scalar.dma_start_transpose(
                            out=vsb[:, 1024:2048].rearrange(
                                "p (b c) -> p b c", c=128),
                            in_=Vf[:, 1024:2048])
                    for c in INTER.get(q, ()):
                        emit_chunk(c)

            stg_cm.__exit__(None, None, None)

            # ---- phase A remainder ----
            with tc.tile_pool(name="psBi", bufs=1, space="PSUM") as psBi:
                holder["psBi"] = psBi
                for cidx in range(16, 64):
                    emit_chunk(cidx)
                while pe_q:
                    pe_q.pop(0)[1]()

                # Y (groups 0-3) drain to outTa.
                for g in range(4):
                    if g % 2 == 0:
                        nc.scalar.copy(outTa[:, g * 512:(g + 1) * 512],
                                       bankY[g][:])
                    else:
                        nc.vector.tensor_copy(
                            outTa[:, g * 512:(g + 1) * 512], bankY[g][:])
            psA_cm.__exit__(None, None, None)
            out2a3 = out2a.rearrange("p (b t) -> p b t", t=128)
            with tc.tile_pool(name="psA2", bufs=1, space="PSUM") as psA2, \
                 tc.tile_pool(name="psC", bufs=2, space="PSUM") as psC, \
                 tc.tile_pool(name="psY", bufs=4, space="PSUM") as psY, \
                 tc.tile_pool(name="late", bufs=2) as late:

                # proj2 Gs complete in pairs; each pair's two y-slices per
                # cb ride one strided DMA to halve the descriptor train.
                PAIRS = [(0, 3), (6, 1), (4, 7), (2, 5)]
                pair_of = {g: (pi, hi) for pi, p in enumerate(PAIRS)
                           for hi, g in enumerate(p)}
                ygt = {}

                def proj2(G):
                    runs = _proj2_runs(G)
                    pi, hi = pair_of[G]
                    for cb in range(2):
                        py = psY.tile([128, 512], F32, name="py", tag="py")
                        off = 0
                        for ri, run in enumerate(runs):
                            w = 128 * len(run)
                            rhs = out2a3[:, run[0]:run[-1] + 1:3, :]
                            nc.tensor.matmul(
                                py[:, off:off + w],
                                woutT[:, cb * 128:(cb + 1) * 128], rhs,
                                start=(ri == 0), stop=(ri == len(runs) - 1),
                                skip_group_check=True,
                            )
                            off += w
                        key = (pi % 2, cb)
                        if hi == 0:
                            ygt[(pi, cb)] = late.tile(
                                [128, 1024], BF16, name=f"yg{key}",
                                tag=f"yg{key}")
                        yg = ygt[(pi, cb)]
                        lo, hc = (min(PAIRS[pi]), PAIRS[pi][hi] != min(PAIRS[pi]))
                        sl = yg[:, 512:1024] if hc else yg[:, 0:512]
                        if cb == 0:
                            nc.scalar.activation(sl, py[:], AF.Identity,
                                                 bias=bo[cb])
                        else:
                            nc.vector.tensor_scalar_add(sl, py[:], bo[cb])
                        if hi == 1:
                            d = (max(PAIRS[pi]) - lo) * 512
                            outap = y_ext[cb * 128:(cb + 1) * 128,
                                          lo * 512:].rearrange(
                                "p (a c) -> p a c", c=512)[:, 0:d // 512 + 1:
                                                           d // 512, :]
                            eng = [nc.sync, nc.scalar, nc.gpsimd][
                                (pi * 2 + cb) % 3]
                            eng.dma_start(out=outap, in_=yg[:])

                def tp(g):
                    tpc = psC.tile([128, 512], BF16, name="tpc", tag="tpc")
                    for s in range(4):
                        nc.tensor.transpose(
                            tpc[:, s * 128:(s + 1) * 128],
                            outTa[:, g * 512 + s * 128:
                                  g * 512 + (s + 1) * 128],
                            ident)
                    if g % 2 == 0:
                        nc.scalar.copy(out2a[:, g * 512:(g + 1) * 512], tpc[:])
                    else:
                        nc.vector.tensor_scalar_add(
                            out2a[:, g * 512:(g + 1) * 512], tpc[:], 0.0)

                # Y groups (0-3) completed at phase-A end.
                for g in range(4):
                    tp(g)
                proj2(0)
                proj2(3)
                # X tails: groups 4-7 chain jb SPLIT_JB+1..15 on psA's freed
                # banks (2 alternating), draining with the spill added; the
                # emission interleaves chains / drains / tps / proj2s so the
                # PE never waits on a just-emitted drain.
                bA2 = [None] * 4

                def xchain(g):
                    bA2[g] = psA2.tile([128, 512], F32, name=f"bA2{g % 2}",
                                       tag=f"bA2{g % 2}")
                    for jb in range(SPLIT_JB + 1, 16):
                        out_mm(bA2[g], g + 4, jb, jb == SPLIT_JB + 1, jb == 15)

                def xdrain(g):
                    nc.vector.tensor_tensor(
                        out=outTa[:, (g + 4) * 512:(g + 5) * 512],
                        in0=bA2[g][:], in1=spill[g][:], op=AluOpType.add)
                    tp(g + 4)

                # Y groups (0-3) completed at phase-A end.
                for g in range(4):
                    tp(g)
                proj2(0)
                proj2(3)
                bA2 = [None] * 4

                def xchain(g):
                    bA2[g] = psA2.tile([128, 512], F32, name=f"bA2{g % 2}",
                                       tag=f"bA2{g % 2}")
                    for jb in range(SPLIT_JB + 1, 16):
                        out_mm(bA2[g], g + 4, jb, jb == SPLIT_JB + 1, jb == 15)

                def xdrain(g):
                    nc.vector.tensor_tensor(
                        out=outTa[:, (g + 4) * 512:(g + 5) * 512],
                        in0=bA2[g][:], in1=spill[g][:], op=AluOpType.add)
                    tp(g + 4)

                xchain(0)
                xchain(1)
                xdrain(0)
                xchain(2)
                xdrain(1)
                xchain(3)
                proj2(6)      # srcs g{2,2,3,4}
                xdrain(2)
                proj2(1)      # g{3,3,4,5}
                proj2(4)      # g{4,4,5,6}
                xdrain(3)
                # remaining: G7 g{5,5,6,7}, G2 g{6,6,7,0}, G5 g{7,7,0,1}
                for G in (7, 2, 5):
                    proj2(G)

        if debug_hook:
            debug_hook(nc, dict(QK8s=QK8s, vsb=vsb,
                                P=P, zsum=zsum, zinv=zinv, outTa=outTa,
                                out2a=out2a))

    nc.compile()
    return nc


def get_nc():
    if "nc" not in _CACHE:
        _CACHE["nc"] = build_nc()
    return _CACHE["nc"]


def make_in_maps(x, W_qkv, b_qkv, W_out, b_out):
    x = np.asarray(x, dtype=np.float32)
    W_qkv = np.asarray(W_qkv, dtype=np.float32)
    b_qkv = np.asarray(b_qkv, dtype=np.float32)
    W_out = np.asarray(W_out, dtype=np.float32)
    b_out = np.asarray(b_out, dtype=np.float32)

    operm = (np.arange(O) + O // 2) % O      # rotate qkv channels by 192
    eperm = (np.arange(E) + E // 2) % E      # rotate e-axis by 64

    halves = []
    for h in range(2):
        if h == 0:
            wq, bqv, wo, bov = W_qkv, b_qkv, W_out, b_out
        else:
            wq = W_qkv[operm]
            bqv = b_qkv[operm]
            wo = W_out[:, eperm]
            bov = np.zeros_like(b_out)
        orders = [
            [3 * t + r for t in range(128)]
            + [3 * t + (r + 2) % 3 for t in range(64)]
            + [3 * t + (r + 1) % 3 for t in range(64)]
            for r in range(3)
        ]
        wqv = np.concatenate([wq.T[:, o] for o in orders], axis=1)     # (C, 768)
        brv = np.concatenate([bqv[o][None, :] for o in orders], axis=1)  # (1, 768)
        halves.append({
            "wqkvT": np.ascontiguousarray(wqv).astype(ml_dtypes.bfloat16),
            "brow": np.ascontiguousarray(brv).astype(ml_dtypes.bfloat16),
            "woutT": np.ascontiguousarray(wo.T).astype(ml_dtypes.bfloat16),
            "bout": np.ascontiguousarray(bov.reshape(C, 1)),
        })

    xb = [np.ascontiguousarray(x[n].reshape(C, HW)).astype(ml_dtypes.bfloat16)
          for n in range(N)]
    in_maps = []
    for core in range(8):
        n, h = core // 2, core % 2
        m = {"x": xb[n]}
        m.update(halves[h])
        in_maps.append(m)
    return in_maps


def run(inputs, trace=False, **kw):
    nc = get_nc()
    in_maps = make_in_maps(**inputs)
    res = run_bass_kernel_spmd(nc, in_maps, core_ids=list(range(8)), trace=trace, **kw)
    ys = [np.asarray(res.results[i]["out"], dtype=np.float32) for i in range(8)]
    y = np.stack([ys[2 * n] + ys[2 * n + 1] for n in range(N)])
    return y.reshape(N, C, 64, 64), res


def kernel(**inputs):
    y, _ = run(inputs, trace=False)
    return y


# revision 47
# speedup vs baseline: 1.0355x; 1.0101x over previous
"""Trainium2 Bass kernel for nn_Attention (dense_transformer), v3.

Reference computation (per batch n of 4):
  qkv = W_qkv @ x + b          (384, 4096)   [x flattened to (256, 64*64)]
  raw C-order reinterpret of qkv flat buffer as (4096, 384) -> q|k|v (4096,128)
  scores = q @ k.T / 64        (4096, 4096)
  soft = softmax(scores, axis=-2)             [column softmax]
  out = soft @ v               (4096, 128)
  raw reinterpret of out as (128, 4096)
  y = W_out @ out2 + b_out     (256, 4096)

Sharding: 8 cores = 4 batches x 2 j-halves (t-halves of the permuted j
enumeration; the host-side 192-rotation of qkv channels and 64-rotation of
W_out's e-axis make the SPMD program identical on all cores). Host sums the
per-pair partial y.

v3 dataflow (vs v2): the q/k tensors are quantized to fp8e4 at the stage-1
drain and the score matmuls run in DoubleRow perf mode (2 fp8 rows per PE
pass, 2x throughput): contraction d=128 is split into two 64-partition
groups, with the d-hi half moved onto partitions 0:64 by SBUF->SBUF DMAs
(idle engines). exp runs on ACT with accum_out supplying the column-softmax
Z sums for most chunks; a subset of chunks uses a Schraudolph fast-exp on
DVE (bf16 bits = S*128*log2e/64 + 16249.7 via f32->u16 convert, written
through a bitcast view) plus a DVE row-reduce, to keep both engines busy.
v transposes ride the DMA XBAR instead of the PE. Bias enters PSUM via
ones-row matmuls so all stage-1 drains are plain copies. Out accumulation
rotates the 4 spare PSUM banks in two eras split at jb10: groups 4-7 live
jb0-10 then spill; groups 0-3 chain burst(jb0-10)+live(jb11-15); groups
4-7 finish jb11-15 in the tail on psA's freed banks, adding the spill at
drain. proj2 folds the psi_q permutation into stride-3 rhs gathers.
"""

import numpy as np
import ml_dtypes

import concourse.bass as bass
import concourse.bacc as bacc
import concourse.mybir as mybir
from concourse.bass_utils import run_bass_kernel_spmd
from concourse.tile import TileContext, add_dep_helper
from concourse.masks import make_identity
from concourse.alu_op_type import AluOpType

BF16 = mybir.dt.bfloat16
F32 = mybir.dt.float32
FP8 = mybir.dt.float8e4
U16 = mybir.dt.uint16
AF = mybir.ActivationFunctionType
AX = mybir.AxisListType
DR = mybir.MatmulPerfMode.DoubleRow

N, C, E, O, HW = 4, 256, 128, 384, 4096
JC = HW // 2          # j per core
NJB = JC // 128       # 16 j-blocks
SCALE = 1.0 / 64.0    # 1/sqrt(HW)
SPLIT_JB = 9          # era split for out accumulation
DIRECT_CIDX = 8       # chunks below this use unsplit fp8 scores
WARM_MMS = 16
SCHR_CIDX2 = None
FLUSH_BUDGET = 2

# Schraudolph fast-exp on DVE for a subset of chunks: bf16 bits of e^x are
# ~ x*(128*log2e) + 16249.7 (HW rounds on f32->u16 convert; rel rms ~1.8%).
SCHR_A = 128.0 * 1.4426950408889634 * SCALE
SCHR_B = 16249.7
# Schraudolph chunk set: spread over the h2 sweep and the h1 slots of the
# jb-minor phase, avoiding stats-critical h3 chunks and DVE-heavy clusters.
SCHR_CIDX = {16, 19, 22, 33, 37, 41, 45, 49, 53, 57, 61, 36}

_CACHE = {}


def _psiq_inv(m):
    if m <= 10:
        return 3 * m
    if m <= 21:
        return 3 * (m - 11) + 1
    return 3 * (m - 22) + 2


def _proj2_runs(G):
    """Maximal stride-3 source-chunk runs feeding y columns [4G*128,(4G+4)*128)."""
    srcs = [_psiq_inv(4 * G + k) for k in range(4)]
    runs = []
    for s in srcs:
        if runs and s == runs[-1][-1] + 3:
            runs[-1].append(s)
        else:
            runs.append([s])
    return runs


def build_nc(debug_hook=None):
    nc = bacc.Bacc("TRN2", target_bir_lowering=False, debug=False, num_devices=8)

    x_ext = nc.dram_tensor("x", [C, HW], BF16, kind="ExternalInput").ap()
    wqkvT_ext = nc.dram_tensor("wqkvT", [C, 768], BF16, kind="ExternalInput").ap()
    brow_ext = nc.dram_tensor("brow", [1, 768], BF16, kind="ExternalInput").ap()
    woutT_ext = nc.dram_tensor("woutT", [E, C], BF16, kind="ExternalInput").ap()
    bout_ext = nc.dram_tensor("bout", [C, 1], F32, kind="ExternalInput").ap()
    y_ext = nc.dram_tensor("out", [C, HW], BF16, kind="ExternalOutput").ap()

    # persistent SBUF
    xsb = [nc.alloc_sbuf_tensor(f"x{cb}", [128, HW], BF16).ap() for cb in range(2)]
    QK8s = nc.alloc_sbuf_tensor("QK8s", [128, 2 * 6144], FP8).ap()
    vsb = nc.alloc_sbuf_tensor("vsb", [128, JC], BF16).ap()
    P = nc.alloc_sbuf_tensor("P", [128, NJB * HW], BF16).ap()
    outTa = nc.alloc_sbuf_tensor("outTa", [128, HW], BF16).ap()
    out2a = nc.alloc_sbuf_tensor("out2a", [128, HW], BF16).ap()
    spill = [nc.alloc_sbuf_tensor(f"spill{g}", [128, 512], F32).ap() for g in range(4)]
    zacc = nc.alloc_sbuf_tensor("zacc", [128, 64], F32).ap()
    zsum = nc.alloc_sbuf_tensor("zsum", [128, 16], F32).ap()
    zinv = nc.alloc_sbuf_tensor("zinv", [128, 16], F32).ap()


    # gathered layout: per g-half, q contiguous (4096) then k (2048)
    QK8sg = QK8s[0:64, :].rearrange("p (g c) -> p g c", g=2)

    def v_sl(jb):
        return vsb[:, jb * 128:(jb + 1) * 128]

    with TileContext(nc) as tc:
        with tc.tile_pool(name="consts", bufs=1) as consts:
            # ---- weights/constants ----
            nc.sync.dma_start(out=xsb[0][:, 0:512], in_=x_ext[0:128, 0:512])
            nc.sync.dma_start(out=xsb[1][:, 0:512], in_=x_ext[128:256, 0:512])
            brow = consts.tile([1, 768], BF16, name="brow", tag="brow")
            nc.sync.dma_start(out=brow, in_=brow_ext[:])
            wq_all = consts.tile([128, 2 * 768], BF16, name="wq_all", tag="wq_all")
            for cb in range(2):
                nc.sync.dma_start(out=wq_all[:, cb * 768:(cb + 1) * 768],
                                  in_=wqkvT_ext[cb * 128:(cb + 1) * 128, :])
            ones1 = consts.tile([1, 128], BF16, name="ones1", tag="ones1")
            nc.vector.memset(ones1[:], 1.0)

            def wq_sl(cb, r):
                return wq_all[:, cb * 768 + r * 256: cb * 768 + (r + 1) * 256]

            misc = consts.tile([128, C + 128], BF16, name="misc", tag="misc")
            woutT = misc[:, 0:C]
            ident = misc[:, C:C + 128]
            nc.gpsimd.dma_start(out=woutT, in_=woutT_ext[:])
            make_identity(nc, ident)
            bo2 = consts.tile([128, 2], F32, name="bo2", tag="bo2")
            bo = [bo2[:, cb:cb + 1] for cb in range(2)]
            for cb in range(2):
                nc.gpsimd.dma_start(out=bo[cb], in_=bout_ext[cb * 128:(cb + 1) * 128, :])
            # Exp table preload
            scratch = consts.tile([128, 1], F32, name="scratch", tag="scratch")
            nc.vector.memset(scratch[:], 0.0)
            nc.scalar.activation(scratch[:], scratch[:], AF.Exp)

            # ---- PE warmup (p-state ramp) ----
            wsrc = consts.tile([128, 128], BF16, name="wsrc", tag="wsrc")
            nc.vector.memset(wsrc[:], 1.0)
            with tc.tile_pool(name="psW", bufs=1, space="PSUM") as psW:
                wtile = psW.tile([128, 128], F32, tag="warm")
                for _ in range(WARM_MMS):
                    nc.tensor.matmul(wtile[:], wsrc[:], wsrc[:], start=True, stop=True)

            # ---- x loads: all on the HWDGE sync ring, 1024-col chunks
            #      interleaved cb0/cb1 in stage-1 consumption order ----
            for lo, hi in ((512, 1536), (1536, 2560), (2560, 3584), (3584, 4096)):
                nc.sync.dma_start(out=xsb[0][:, lo:hi], in_=x_ext[0:128, lo:hi])
                nc.sync.dma_start(out=xsb[1][:, lo:hi],
                                  in_=x_ext[128:256, lo:hi])

            # ---- phase A chunk order: h-sweeps over jb0-7 (software-
            #      pipelined into stage 1), then jb8-15 h-minor. ----
            order = []
            for h in range(4):
                for jb in range(8):
                    order.append((jb, h))
            for jb in range(8, 16):
                for h in range(4):
                    order.append((jb, h))

            psA_cm = tc.tile_pool(name="psA", bufs=2, space="PSUM")
            psA = psA_cm.__enter__()
            stg_cm = tc.tile_pool(name="stg", bufs=1)
            stg = stg_cm.__enter__()
            Qtmp = stg.tile([128, 32 * 128], FP8, name="Qtmp", tag="Qtmp")
            Ktmp = stg.tile([128, 32 * 64], FP8, name="Ktmp", tag="Ktmp")
            Vf = stg.tile([128, 32 * 64], BF16, name="Vf", tag="Vf")
            Vv = Vf.rearrange("p (b c) -> p b c", c=64)
            holder = {}
            bankX = [None] * 4   # i'-groups 4-7: live jb0..SPLIT_JB
            bankY = [None] * 4   # i'-groups 0-3: burst+live, stop jb15
            pe_q = []   # (ready_chunk_idx, emit_fn): deferred PE MMs

            def flush(cidx, budget=4):
                n = 0
                while pe_q and pe_q[0][0] <= cidx and n < budget:
                    pe_q.pop(0)[1]()
                    n += 1

            def stats(jb):
                nc.vector.reduce_sum(
                    out=zsum[:, jb:jb + 1], in_=zacc[:, jb * 4:(jb + 1) * 4],
                    axis=AX.X)
                nc.vector.reciprocal(zinv[:, jb:jb + 1], zsum[:, jb:jb + 1])
                nc.vector.tensor_scalar_mul(v_sl(jb), v_sl(jb),
                                            zinv[:, jb:jb + 1])

            def out_mm(bank, g, jb, start, stop):
                nc.tensor.matmul(
                    bank[:], v_sl(jb),
                    P[:, jb * HW + g * 512: jb * HW + (g + 1) * 512],
                    start=start, stop=stop,
                )

            def emit_chunk(cidx):
                jb, h = order[cidx]
                pa = psA.tile([128, 1024], F32, name="pa", tag="pa")
                for n2 in range(2):
                    if cidx < 16:
                        # unsplit fp8 (128-partition contraction, one 64-wide
                        # k block per MM, stacked via tile_position):
                        # independent of the gather DMAs
                        for t in range(2):
                            nc.tensor.matmul(
                                pa[64 * t:64 * t + 64,
                                   n2 * 512:(n2 + 1) * 512],
                                Ktmp[:, (2 * jb + t) * 64:
                                     (2 * jb + t + 1) * 64],
                                Qtmp[:, (8 * h + 4 * n2) * 128:
                                     (8 * h + 4 * n2 + 4) * 128],
                                start=True, stop=True,
                                tile_position=(0, 64 * t),
                            )
                    else:
                        o = (8 * h + 4 * n2) * 128
                        nc.tensor.matmul(
                            pa[:, n2 * 512:(n2 + 1) * 512],
                            QK8sg[:, :, 4096 + jb * 128:4096 + (jb + 1) * 128],
                            QK8sg[:, :, o:o + 512],
                            start=True, stop=True,
                            perf_mode=DR,
                        )
                psl = P[:, jb * HW + h * 1024: jb * HW + (h + 1) * 1024]
                zc = zacc[:, jb * 4 + h: jb * 4 + h + 1]
                # Z per chunk: ACT accum_out (free row-sums) for most chunks;
                # DVE Schraudolph exp + reduce for a subset.
                if cidx in SCHR_CIDX:
                    nc.vector.tensor_scalar(
                        out=psl.bitcast(U16), in0=pa[:],
                        scalar1=SCHR_A, scalar2=SCHR_B,
                        op0=AluOpType.mult, op1=AluOpType.add)
                    nc.vector.reduce_sum(out=zc, in_=psl, axis=AX.X)
                else:
                    nc.scalar.activation(out=psl, in_=pa[:], func=AF.Exp,
                                         scale=SCALE, accum_out=zc)
                flush(cidx, budget=FLUSH_BUDGET)
                if h == 3:
                    stats(jb)
                    psBi = holder["psBi"]
                    if jb == 0:
                        for g in range(4):
                            bankX[g] = psBi.tile([128, 512], F32,
                                                 name=f"bk{g}", tag=f"bk{g}")
                    if jb <= SPLIT_JB:
                        for g in range(4):
                            pe_q.append((cidx + 2,
                                         (lambda g=g, jb=jb: out_mm(
                                             bankX[g], g + 4, jb,
                                             jb == 0, jb == SPLIT_JB))))
                    else:
                        for g in range(4):
                            pe_q.append((cidx + 4,
                                         (lambda g=g, jb=jb: out_mm(
                                             bankY[g], g, jb,
                                             False, jb == 15))))
                    if jb == SPLIT_JB:
                        # spill X banks; queue Y bursts (groups 0-3)
                        # jb0..SPLIT_JB from persistent P.
                        def spill_and_y():
                            for g in range(4):
                                nc.vector.tensor_copy(spill[g][:],
                                                      bankX[g][:])
                            for g in range(4):
                                bankY[g] = psBi.tile([128, 512], F32,
                                                     name=f"bk{g}",
                                                     tag=f"bk{g}")
                        pe_q.append((cidx + 2, spill_and_y))
                        for jbq in range(SPLIT_JB + 1):
                            for g in range(4):
                                pe_q.append((cidx + 2 + jbq // 2,
                                             (lambda g=g, jbq=jbq: out_mm(
                                                 bankY[g], g, jbq,
                                                 jbq == 0, False))))

            # chunks emitted between stage-1 quads (deps: k needs its jb's
            # quads, q needs quads 2h,2h+1; all cidx<16 are gather-free)
            INTER = {1: [0, 1, 2, 3], 2: [4, 5], 3: [6, 7, 8, 9, 10, 11],
                     4: [12, 13], 5: [14, 15]}

            # ---- stage 1: quads of FT blocks -> Qtmp/Ktmp (fp8) + Vf
            #      (bf16), gather DMAs -> QK8s, XBAR -> vsb; early phase-A
            #      chunks interleave (psF on banks 4-7, psA on 0-3). ----
            with tc.tile_pool(name="psF", bufs=2, space="PSUM") as psF:
                for q in range(8):
                    pf = psF.tile([128, 1024], F32, name="pf", tag="pf")
                    pf3 = pf.rearrange("p (s c) -> p s c", s=4)
                    for s in range(4):
                        sc = 4 * q + s
                        r = sc % 3
                        nc.tensor.matmul(pf3[:, s, 0:256], ones1[:],
                                         brow[:, r * 256:(r + 1) * 256],
                                         start=True, stop=False)
                        for cb in range(2):
                            nc.tensor.matmul(
                                pf3[:, s, 0:256],
                                xsb[cb][:, sc * 128:(sc + 1) * 128],
                                wq_sl(cb, r),
                                start=False, stop=(cb == 1),
                            )
                    cp = nc.vector.tensor_copy
                    cp(Qtmp[:, q * 512:(q + 1) * 512], pf3[:, :, 0:128])
                    cp(Ktmp[:, q * 256:(q + 1) * 256], pf3[:, :, 128:192])
                    # v tokens for k-block sc live in v-block sc+1: store the
                    # v drain one block down (with wrap) so vsb aligns with kT.
                    if q == 0:
                        cp(Vv[:, 31:32, :], pf3[:, 0:1, 192:256])
                        cp(Vv[:, 0:3, :], pf3[:, 1:4, 192:256])
                    else:
                        cp(Vv[:, 4 * q - 1:4 * q + 3, :], pf3[:, :, 192:256])
                    # gather DMAs per 16-block round: contiguous q/k copies
                    # of each d-half (g=1 is the partition-shifted 64:128
                    # half) into the per-g layout [q 4096 | k 2048].
                    if q % 4 == 3:
                        bg = q // 4
                        for g in range(2):
                            nc.sync.dma_start(
                                out=QK8s[0:64, g * 6144 + bg * 2048:
                                         g * 6144 + (bg + 1) * 2048],
                                in_=Qtmp[64 * g:64 * g + 64,
                                         bg * 2048:(bg + 1) * 2048])
                            nc.scalar.dma_start(
                                out=QK8s[0:64, g * 6144 + 4096 + bg * 1024:
                                         g * 6144 + 4096 + (bg + 1) * 1024],
                                in_=Ktmp[64 * g:64 * g + 64,
                                         bg * 1024:(bg + 1) * 1024])
                    # XBAR transposes: vsb[:, jb, :] = Vf[:, jb, :].T, in
                    # two halves so jb0-7 stats don't wait on all drains
                    if q == 4:
                        nc.scalar.dma_start_transpose(
                            out=vsb[:, 0:1024].rearrange("p (b c) -> p b c",
                                                         c=128),
                            in_=Vf[:, 0:1024])
                    if q == 7:
                        nc.scalar.dma_start_transpose(
                            out=vsb[:, 1024:2048].rearrange(
                                "p (b c) -> p b c", c=128),
                            in_=Vf[:, 1024:2048])
                    for c in INTER.get(q, ()):
                        emit_chunk(c)

            stg_cm.__exit__(None, None, None)

            # ---- phase A remainder ----
            with tc.tile_pool(name="psBi", bufs=1, space="PSUM") as psBi:
                holder["psBi"] = psBi
                for cidx in range(16, 64):
                    emit_chunk(cidx)
                while pe_q:
                    pe_q.pop(0)[1]()

                # Y (groups 0-3) drain to outTa.
                for g in range(4):
                    if g % 2 == 0:
                        nc.scalar.copy(outTa[:, g * 512:(g + 1) * 512],
                                       bankY[g][:])
                    else:
                        nc.vector.tensor_copy(
                            outTa[:, g * 512:(g + 1) * 512], bankY[g][:])
            psA_cm.__exit__(None, None, None)
            out2a3 = out2a.rearrange("p (b t) -> p b t", t=128)
            with tc.tile_pool(name="psA2", bufs=1, space="PSUM") as psA2, \
                 tc.tile_pool(name="psC", bufs=2, space="PSUM") as psC, \
                 tc.tile_pool(name="psY", bufs=4, space="PSUM") as psY, \
                 tc.tile_pool(name="late", bufs=2) as late:

                # proj2 Gs complete in pairs; each pair's two y-slices per
                # cb ride one strided DMA to halve the descriptor train.
                PAIRS = [(0, 3), (6, 1), (4, 7), (2, 5)]
                pair_of = {g: (pi, hi) for pi, p in enumerate(PAIRS)
                           for hi, g in enumerate(p)}
                ygt = {}

                def proj2(G):
                    runs = _proj2_runs(G)
                    pi, hi = pair_of[G]
                    for cb in range(2):
                        py = psY.tile([128, 512], F32, name="py", tag="py")
                        off = 0
                        for ri, run in enumerate(runs):
                            w = 128 * len(run)
                            rhs = out2a3[:, run[0]:run[-1] + 1:3, :]
                            nc.tensor.matmul(
                                py[:, off:off + w],
                                woutT[:, cb * 128:(cb + 1) * 128], rhs,
                                start=(ri == 0), stop=(ri == len(runs) - 1),
                                skip_group_check=True,
                            )
                            off += w
                        key = (pi % 2, cb)
                        if hi == 0:
                            ygt[(pi, cb)] = late.tile(
                                [128, 1024], BF16, name=f"yg{key}",
                                tag=f"yg{key}")
                        yg = ygt[(pi, cb)]
                        lo, hc = (min(PAIRS[pi]), PAIRS[pi][hi] != min(PAIRS[pi]))
                        sl = yg[:, 512:1024] if hc else yg[:, 0:512]
                        if cb == 0:
                            nc.scalar.activation(sl, py[:], AF.Identity,
                                                 bias=bo[cb])
                        else:
                            nc.vector.tensor_scalar_add(sl, py[:], bo[cb])
                        if hi == 1:
                            d = (max(PAIRS[pi]) - lo) * 512
                            outap = y_ext[cb * 128:(cb + 1) * 128,
                                          lo * 512:].rearrange(
                                "p (a c) -> p a c", c=512)[:, 0:d // 512 + 1:
                                                           d // 512, :]
                            eng = [nc.sync, nc.scalar, nc.gpsimd][
                                (pi * 2 + cb) % 3]
                            eng.dma_start(out=outap, in_=yg[:])

                def tp(g):
                    tpc = psC.tile([128, 512], BF16, name="tpc", tag="tpc")
                    for s in range(4):
                        nc.tensor.transpose(
                            tpc[:, s * 128:(s + 1) * 128],
                            outTa[:, g * 512 + s * 128:
                                  g * 512 + (s + 1) * 128],
                            ident)
                    if g % 2 == 0:
                        nc.scalar.copy(out2a[:, g * 512:(g + 1) * 512], tpc[:])
                    else:
                        nc.vector.tensor_scalar_add(
                            out2a[:, g * 512:(g + 1) * 512], tpc[:], 0.0)

                # Y groups (0-3) completed at phase-A end.
                for g in range(4):
                    tp(g)
                proj2(0)
                proj2(3)
                # X tails: groups 4-7 chain jb SPLIT_JB+1..15 on psA's freed
                # banks (2 alternating), draining with the spill added; the
                # emission interleaves chains / drains / tps / proj2s so the
                # PE never waits on a just-emitted drain.
                bA2 = [None] * 4

                def xchain(g):
                    bA2[g] = psA2.tile([128, 512], F32, name=f"bA2{g % 2}",
                                       tag=f"bA2{g % 2}")
                    for jb in range(SPLIT_JB + 1, 16):
                        out_mm(bA2[g], g + 4, jb, jb == SPLIT_JB + 1, jb == 15)

                def xdrain(g):
                    nc.vector.tensor_tensor(
                        out=outTa[:, (g + 4) * 512:(g + 5) * 512],
                        in0=bA2[g][:], in1=spill[g][:], op=AluOpType.add)
                    tp(g + 4)

                # Y groups (0-3) completed at phase-A end.
                for g in range(4):
                    tp(g)
                proj2(0)
                proj2(3)
                bA2 = [None] * 4

                def xchain(g):
                    bA2[g] = psA2.tile([128, 512], F32, name=f"bA2{g % 2}",
                                       tag=f"bA2{g % 2}")
                    for jb in range(SPLIT_JB + 1, 16):
                        out_mm(bA2[g], g + 4, jb, jb == SPLIT_JB + 1, jb == 15)

                def xdrain(g):
                    nc.vector.tensor_tensor(
                        out=outTa[:, (g + 4) * 512:(g + 5) * 512],
                        in0=bA2[g][:], in1=spill[g][:], op=AluOpType.add)
                    tp(g + 4)

                xchain(0)
                xchain(1)
                xdrain(0)
                xchain(2)
                xdrain(1)
                xchain(3)
                proj2(6)      # srcs g{2,2,3,4}
                xdrain(2)
                proj2(1)      # g{3,3,4,5}
                proj2(4)      # g{4,4,5,6}
                xdrain(3)
                # remaining: G7 g{5,5,6,7}, G2 g{6,6,7,0}, G5 g{7,7,0,1}
                for G in (7, 2, 5):
                    proj2(G)

        if debug_hook:
            debug_hook(nc, dict(QK8s=QK8s, vsb=vsb,
                                P=P, zsum=zsum, zinv=zinv, outTa=outTa,
                                out2a=out2a))

    nc.compile()
    return nc


def get_nc():
    if "nc" not in _CACHE:
        _CACHE["nc"] = build_nc()
    return _CACHE["nc"]


def make_in_maps(x, W_qkv, b_qkv, W_out, b_out):
    x = np.asarray(x, dtype=np.float32)
    W_qkv = np.asarray(W_qkv, dtype=np.float32)
    b_qkv = np.asarray(b_qkv, dtype=np.float32)
    W_out = np.asarray(W_out, dtype=np.float32)
    b_out = np.asarray(b_out, dtype=np.float32)

    operm = (np.arange(O) + O // 2) % O      # rotate qkv channels by 192
    eperm = (np.arange(E) + E // 2) % E      # rotate e-axis by 64

    halves = []
    for h in range(2):
        if h == 0:
            wq, bqv, wo, bov = W_qkv, b_qkv, W_out, b_out
        else:
            wq = W_qkv[operm]
            bqv = b_qkv[operm]
            wo = W_out[:, eperm]
            bov = np.zeros_like(b_out)
        orders = [
            [3 * t + r for t in range(128)]
            + [3 * t + (r + 2) % 3 for t in range(64)]
            + [3 * t + (r + 1) % 3 for t in range(64)]
            for r in range(3)
        ]
        wqv = np.concatenate([wq.T[:, o] for o in orders], axis=1)     # (C, 768)
        brv = np.concatenate([bqv[o][None, :] for o in orders], axis=1)  # (1, 768)
        halves.append({
            "wqkvT": np.ascontiguousarray(wqv).astype(ml_dtypes.bfloat16),
            "brow": np.ascontiguousarray(brv).astype(ml_dtypes.bfloat16),
            "woutT": np.ascontiguousarray(wo.T).astype(ml_dtypes.bfloat16),
            "bout": np.ascontiguousarray(bov.reshape(C, 1)),
        })

    xb = [np.ascontiguousarray(x[n].reshape(C, HW)).astype(ml_dtypes.bfloat16)
          for n in range(N)]
    in_maps = []
    for core in range(8):
        n, h = core // 2, core % 2
        m = {"x": xb[n]}
        m.update(halves[h])
        in_maps.append(m)
    return in_maps


def run(inputs, trace=False, **kw):
    nc = get_nc()
    in_maps = make_in_maps(**inputs)
    res = run_bass_kernel_spmd(nc, in_maps, core_ids=list(range(8)), trace=trace, **kw)
    ys = [np.asarray(res.results[i]["out"], dtype=np.float32) for i in range(8)]
    y = np.stack([ys[2 * n] + ys[2 * n + 1] for n in range(N)])
    return y.reshape(N, C, 64, 64), res


def kernel(**inputs):
    y, _ = run(inputs, trace=False)
    return y


# revision 54
# speedup vs baseline: 1.0735x; 1.0367x over previous
"""Trainium2 Bass kernel for nn_Attention (dense_transformer), v3.

Reference computation (per batch n of 4):
  qkv = W_qkv @ x + b          (384, 4096)   [x flattened to (256, 64*64)]
  raw C-order reinterpret of qkv flat buffer as (4096, 384) -> q|k|v (4096,128)
  scores = q @ k.T / 64        (4096, 4096)
  soft = softmax(scores, axis=-2)             [column softmax]
  out = soft @ v               (4096, 128)
  raw reinterpret of out as (128, 4096)
  y = W_out @ out2 + b_out     (256, 4096)

Sharding: 8 cores = 4 batches x 2 j-halves (t-halves of the permuted j
enumeration; the host-side 192-rotation of qkv channels and 64-rotation of
W_out's e-axis make the SPMD program identical on all cores). Host sums the
per-pair partial y.

v3 dataflow (vs v2): the q/k tensors are quantized to fp8e4 at the stage-1
drain and the score matmuls run in DoubleRow perf mode (2 fp8 rows per PE
pass, 2x throughput): contraction d=128 is split into two 64-partition
groups, with the d-hi half moved onto partitions 0:64 by SBUF->SBUF DMAs
(idle engines). exp runs on ACT with accum_out supplying the column-softmax
Z sums for most chunks; a subset of chunks uses a Schraudolph fast-exp on
DVE (bf16 bits = S*128*log2e/64 + 16249.7 via f32->u16 convert, written
through a bitcast view) plus a DVE row-reduce, to keep both engines busy.
v transposes ride the DMA XBAR instead of the PE. Bias enters PSUM via
ones-row matmuls so all stage-1 drains are plain copies. Out accumulation
rotates the 4 spare PSUM banks in two eras split at jb10: groups 4-7 live
jb0-10 then spill; groups 0-3 chain burst(jb0-10)+live(jb11-15); groups
4-7 finish jb11-15 in the tail on psA's freed banks, adding the spill at
drain. proj2 folds the psi_q permutation into stride-3 rhs gathers.
"""

import numpy as np
import ml_dtypes

import concourse.bass as bass
import concourse.bacc as bacc
import concourse.mybir as mybir
from concourse.bass_utils import run_bass_kernel_spmd
from concourse.tile import TileContext, add_dep_helper
from concourse.masks import make_identity
from concourse.alu_op_type import AluOpType

BF16 = mybir.dt.bfloat16
F32 = mybir.dt.float32
FP8 = mybir.dt.float8e4
U16 = mybir.dt.uint16
AF = mybir.ActivationFunctionType
AX = mybir.AxisListType
DR = mybir.MatmulPerfMode.DoubleRow

N, C, E, O, HW = 4, 256, 128, 384, 4096
JC = HW // 2          # j per core
NJB = JC // 128       # 16 j-blocks
SCALE = 1.0 / 64.0    # 1/sqrt(HW)
SPLIT_JB = 9          # era split for out accumulation
DIRECT_CIDX = 8       # chunks below this use unsplit fp8 scores
WARM_MMS = 16
SCHR_CIDX2 = None
FLUSH_BUDGET = 2
TAIL_ORDER = [("t", 0), ("t", 2), ("p", 0), ("p", 3),
              ("x", 0), ("x", 1), ("d", 0), ("x", 2), ("d", 1), ("t", 4),
              ("x", 3), ("p", 6), ("d", 2), ("p", 1), ("d", 3), ("t", 6),
              ("p", 4)]

# Schraudolph fast-exp on DVE for a subset of chunks: bf16 bits of e^x are
# ~ x*(128*log2e) + 16249.7 (HW rounds on f32->u16 convert; rel rms ~1.8%).
SCHR_A = 128.0 * 1.4426950408889634 * SCALE
SCHR_B = 16249.7
# Schraudolph chunk set: spread over the h2 sweep and the h1 slots of the
# jb-minor phase, avoiding stats-critical h3 chunks and DVE-heavy clusters.
SCHR_CIDX = {16, 19, 22, 25, 28, 30, 33, 36, 39, 42, 45, 48, 51, 54, 57,
             60, 63}
SCHR_W = 1024

_CACHE = {}


def _psiq_inv(m):
    if m <= 10:
        return 3 * m
    if m <= 21:
        return 3 * (m - 11) + 1
    return 3 * (m - 22) + 2


def _proj2_runs(G):
    """Maximal stride-3 source-chunk runs feeding y columns [4G*128,(4G+4)*128)."""
    srcs = [_psiq_inv(4 * G + k) for k in range(4)]
    runs = []
    for s in srcs:
        if runs and s == runs[-1][-1] + 3:
            runs[-1].append(s)
        else:
            runs.append([s])
    return runs


def build_nc(debug_hook=None):
    nc = bacc.Bacc("TRN2", target_bir_lowering=False, debug=False, num_devices=8)

    x_ext = nc.dram_tensor("x", [C, HW], BF16, kind="ExternalInput").ap()
    wqkvT_ext = nc.dram_tensor("wqkvT", [C, 768], BF16, kind="ExternalInput").ap()
    brow_ext = nc.dram_tensor("brow", [1, 768], BF16, kind="ExternalInput").ap()
    woutT_ext = nc.dram_tensor("woutT", [E, C], BF16, kind="ExternalInput").ap()
    bout_ext = nc.dram_tensor("bout", [C, 1], F32, kind="ExternalInput").ap()
    y_ext = nc.dram_tensor("out", [C, HW], BF16, kind="ExternalOutput").ap()

    # persistent SBUF
    xsb = [nc.alloc_sbuf_tensor(f"x{cb}", [128, HW], BF16).ap() for cb in range(2)]
    QK8s = nc.alloc_sbuf_tensor("QK8s", [128, 2 * 6144], FP8).ap()
    vsb = nc.alloc_sbuf_tensor("vsb", [128, JC], BF16).ap()
    P = nc.alloc_sbuf_tensor("P", [128, NJB * HW], BF16).ap()
    outTa = nc.alloc_sbuf_tensor("outTa", [128, HW], BF16).ap()
    out2a = nc.alloc_sbuf_tensor("out2a", [128, HW], BF16).ap()
    spill = [nc.alloc_sbuf_tensor(f"spill{g}", [128, 512], F32).ap() for g in range(4)]
    zacc = nc.alloc_sbuf_tensor("zacc", [128, 128], F32).ap()
    zsum = nc.alloc_sbuf_tensor("zsum", [128, 16], F32).ap()
    zinv = nc.alloc_sbuf_tensor("zinv", [128, 16], F32).ap()


    # gathered layout: per g-half, q contiguous (4096) then k (2048)
    QK8sg = QK8s[0:64, :].rearrange("p (g c) -> p g c", g=2)

    def v_sl(jb):
        return vsb[:, jb * 128:(jb + 1) * 128]

    with TileContext(nc) as tc:
        with tc.tile_pool(name="consts", bufs=1) as consts:
            # ---- weights/constants ----
            nc.sync.dma_start(out=xsb[0][:, 0:512], in_=x_ext[0:128, 0:512])
            nc.sync.dma_start(out=xsb[1][:, 0:512], in_=x_ext[128:256, 0:512])
            brow = consts.tile([1, 768], BF16, name="brow", tag="brow")
            nc.sync.dma_start(out=brow, in_=brow_ext[:])
            wq_all = consts.tile([128, 2 * 768], BF16, name="wq_all", tag="wq_all")
            for cb in range(2):
                nc.sync.dma_start(out=wq_all[:, cb * 768:(cb + 1) * 768],
                                  in_=wqkvT_ext[cb * 128:(cb + 1) * 128, :])
            ones1 = consts.tile([1, 128], BF16, name="ones1", tag="ones1")
            nc.vector.memset(ones1[:], 1.0)

            def wq_sl(cb, r):
                return wq_all[:, cb * 768 + r * 256: cb * 768 + (r + 1) * 256]

            misc = consts.tile([128, C + 128], BF16, name="misc", tag="misc")
            woutT = misc[:, 0:C]
            ident = misc[:, C:C + 128]
            nc.gpsimd.dma_start(out=woutT, in_=woutT_ext[:])
            make_identity(nc, ident)
            bo2 = consts.tile([128, 2], F32, name="bo2", tag="bo2")
            bo = [bo2[:, cb:cb + 1] for cb in range(2)]
            for cb in range(2):
                nc.gpsimd.dma_start(out=bo[cb], in_=bout_ext[cb * 128:(cb + 1) * 128, :])
            nc.vector.memset(zacc[:], 0.0)
            # Exp table preload
            scratch = consts.tile([128, 1], F32, name="scratch", tag="scratch")
            nc.vector.memset(scratch[:], 0.0)
            nc.scalar.activation(scratch[:], scratch[:], AF.Exp)

            # ---- PE warmup (p-state ramp) ----
            wsrc = consts.tile([128, 128], BF16, name="wsrc", tag="wsrc")
            nc.vector.memset(wsrc[:], 1.0)
            with tc.tile_pool(name="psW", bufs=1, space="PSUM") as psW:
                wtile = psW.tile([128, 128], F32, tag="warm")
                for _ in range(WARM_MMS):
                    nc.tensor.matmul(wtile[:], wsrc[:], wsrc[:], start=True, stop=True)

            # ---- x loads: all on the HWDGE sync ring, 1024-col chunks
            #      interleaved cb0/cb1 in stage-1 consumption order ----
            for lo, hi in ((512, 1536), (1536, 2560), (2560, 3584), (3584, 4096)):
                nc.sync.dma_start(out=xsb[0][:, lo:hi], in_=x_ext[0:128, lo:hi])
                nc.sync.dma_start(out=xsb[1][:, lo:hi],
                                  in_=x_ext[128:256, lo:hi])

            # ---- phase A chunk order: h-sweeps over jb0-7 (software-
            #      pipelined into stage 1), then jb8-15 h-minor. ----
            order = []
            for h in range(4):
                for jb in range(8):
                    order.append((jb, h))
            for jb in range(8, 16):
                for h in range(4):
                    order.append((jb, h))

            psA_cm = tc.tile_pool(name="psA", bufs=2, space="PSUM")
            psA = psA_cm.__enter__()
            stg_cm = tc.tile_pool(name="stg", bufs=1)
            stg = stg_cm.__enter__()
            Qtmp = stg.tile([128, 32 * 128], FP8, name="Qtmp", tag="Qtmp")
            Ktmp = stg.tile([128, 32 * 64], FP8, name="Ktmp", tag="Ktmp")
            Vf = stg.tile([128, 32 * 64], BF16, name="Vf", tag="Vf")
            Vv = Vf.rearrange("p (b c) -> p b c", c=64)
            holder = {}
            bankX = [None] * 4   # i'-groups 4-7: live jb0..SPLIT_JB
            bankY = [None] * 4   # i'-groups 0-3: burst+live, stop jb15
            pe_q = []   # (ready_chunk_idx, emit_fn): deferred PE MMs

            def flush(cidx, budget=4):
                n = 0
                while pe_q and pe_q[0][0] <= cidx and n < budget:
                    pe_q.pop(0)[1]()
                    n += 1

            def stats(jb):
                nc.vector.reduce_sum(
                    out=zsum[:, jb:jb + 1], in_=zacc[:, jb * 8:(jb + 1) * 8],
                    axis=AX.X)
                nc.vector.reciprocal(zinv[:, jb:jb + 1], zsum[:, jb:jb + 1])
                nc.vector.tensor_scalar_mul(v_sl(jb), v_sl(jb),
                                            zinv[:, jb:jb + 1])

            def out_mm(bank, g, jb, start, stop):
                nc.tensor.matmul(
                    bank[:], v_sl(jb),
                    P[:, jb * HW + g * 512: jb * HW + (g + 1) * 512],
                    start=start, stop=stop,
                )

            def emit_chunk(cidx):
                jb, h = order[cidx]
                pa = psA.tile([128, 1024], F32, name="pa", tag="pa")
                for n2 in range(2):
                    if cidx < 16:
                        # unsplit fp8 (128-partition contraction, one 64-wide
                        # k block per MM, stacked via tile_position):
                        # independent of the gather DMAs
                        for t in range(2):
                            nc.tensor.matmul(
                                pa[64 * t:64 * t + 64,
                                   n2 * 512:(n2 + 1) * 512],
                                Ktmp[:, (2 * jb + t) * 64:
                                     (2 * jb + t + 1) * 64],
                                Qtmp[:, (8 * h + 4 * n2) * 128:
                                     (8 * h + 4 * n2 + 4) * 128],
                                start=True, stop=True,
                                tile_position=(0, 64 * t),
                            )
                    else:
                        o = (8 * h + 4 * n2) * 128
                        nc.tensor.matmul(
                            pa[:, n2 * 512:(n2 + 1) * 512],
                            QK8sg[:, :, 4096 + jb * 128:4096 + (jb + 1) * 128],
                            QK8sg[:, :, o:o + 512],
                            start=True, stop=True,
                            perf_mode=DR,
                        )
                psl = P[:, jb * HW + h * 1024: jb * HW + (h + 1) * 1024]
                zc0 = zacc[:, jb * 8 + 2 * h: jb * 8 + 2 * h + 1]
                zc1 = zacc[:, jb * 8 + 2 * h + 1: jb * 8 + 2 * h + 2]
                # Z per chunk via ACT accum_out (free row-sums). Split
                # chunks run half on DVE (Schraudolph fast-exp + reduce)
                # and half on ACT concurrently to balance the engines.
                if cidx in SCHR_CIDX:
                    w = SCHR_W
                    nc.vector.tensor_scalar(
                        out=psl[:, 0:w].bitcast(U16), in0=pa[:, 0:w],
                        scalar1=SCHR_A, scalar2=SCHR_B,
                        op0=AluOpType.mult, op1=AluOpType.add)
                    nc.vector.reduce_sum(out=zc0, in_=psl[:, 0:w],
                                         axis=AX.X)
                    if w < 1024:
                        nc.scalar.activation(out=psl[:, w:1024],
                                             in_=pa[:, w:1024], func=AF.Exp,
                                             scale=SCALE, accum_out=zc1)
                else:
                    nc.scalar.activation(out=psl, in_=pa[:], func=AF.Exp,
                                         scale=SCALE, accum_out=zc0)
                flush(cidx, budget=FLUSH_BUDGET)
                if h == 3:
                    stats(jb)
                    psBi = holder["psBi"]
                    if jb == 0:
                        for g in range(4):
                            bankX[g] = psBi.tile([128, 512], F32,
                                                 name=f"bk{g}", tag=f"bk{g}")
                    if jb <= SPLIT_JB:
                        for g in range(4):
                            pe_q.append((cidx + 2,
                                         (lambda g=g, jb=jb: out_mm(
                                             bankX[g], g + 4, jb,
                                             jb == 0, jb == SPLIT_JB))))
                    else:
                        for g in range(4):
                            pe_q.append((cidx + 4,
                                         (lambda g=g, jb=jb: out_mm(
                                             bankY[g], g, jb,
                                             False, jb == 15))))
                    if jb == SPLIT_JB:
                        # spill X banks; queue Y bursts (groups 0-3)
                        # jb0..SPLIT_JB from persistent P.
                        def spill_and_y():
                            for g in range(4):
                                nc.vector.tensor_copy(spill[g][:],
                                                      bankX[g][:])
                            for g in range(4):
                                bankY[g] = psBi.tile([128, 512], F32,
                                                     name=f"bk{g}",
                                                     tag=f"bk{g}")
                        pe_q.append((cidx + 2, spill_and_y))
                        for jbq in range(SPLIT_JB + 1):
                            for g in range(4):
                                pe_q.append((cidx + 2 + jbq // 2,
                                             (lambda g=g, jbq=jbq: out_mm(
                                                 bankY[g], g, jbq,
                                                 jbq == 0, False))))

            # chunks emitted between stage-1 quads (deps: k needs its jb's
            # quads, q needs quads 2h,2h+1; all cidx<16 are gather-free)
            INTER = {1: [0, 1, 2, 3], 2: [4, 5], 3: [6, 7, 8, 9, 10, 11],
                     4: [12, 13], 5: [14, 15]}

            # ---- stage 1: quads of FT blocks -> Qtmp/Ktmp (fp8) + Vf
            #      (bf16), gather DMAs -> QK8s, XBAR -> vsb; early phase-A
            #      chunks interleave (psF on banks 4-7, psA on 0-3). ----
            with tc.tile_pool(name="psF", bufs=2, space="PSUM") as psF:
                for q in range(8):
                    pf = psF.tile([128, 1024], F32, name="pf", tag="pf")
                    pf3 = pf.rearrange("p (s c) -> p s c", s=4)
                    for s in range(4):
                        sc = 4 * q + s
                        r = sc % 3
                        nc.tensor.matmul(pf3[:, s, 0:256], ones1[:],
                                         brow[:, r * 256:(r + 1) * 256],
                                         start=True, stop=False)
                        for cb in range(2):
                            nc.tensor.matmul(
                                pf3[:, s, 0:256],
                                xsb[cb][:, sc * 128:(sc + 1) * 128],
                                wq_sl(cb, r),
                                start=False, stop=(cb == 1),
                            )
                    cp = nc.vector.tensor_copy
                    cp(Qtmp[:, q * 512:(q + 1) * 512], pf3[:, :, 0:128])
                    cp(Ktmp[:, q * 256:(q + 1) * 256], pf3[:, :, 128:192])
                    # v tokens for k-block sc live in v-block sc+1: store the
                    # v drain one block down (with wrap) so vsb aligns with kT.
                    if q == 0:
                        cp(Vv[:, 31:32, :], pf3[:, 0:1, 192:256])
                        cp(Vv[:, 0:3, :], pf3[:, 1:4, 192:256])
                    else:
                        cp(Vv[:, 4 * q - 1:4 * q + 3, :], pf3[:, :, 192:256])
                    # gather DMAs per 16-block round: contiguous q/k copies
                    # of each d-half (g=1 is the partition-shifted 64:128
                    # half) into the per-g layout [q 4096 | k 2048].
                    if q % 4 == 3:
                        bg = q // 4
                        for g in range(2):
                            nc.sync.dma_start(
                                out=QK8s[0:64, g * 6144 + bg * 2048:
                                         g * 6144 + (bg + 1) * 2048],
                                in_=Qtmp[64 * g:64 * g + 64,
                                         bg * 2048:(bg + 1) * 2048])
                            nc.scalar.dma_start(
                                out=QK8s[0:64, g * 6144 + 4096 + bg * 1024:
                                         g * 6144 + 4096 + (bg + 1) * 1024],
                                in_=Ktmp[64 * g:64 * g + 64,
                                         bg * 1024:(bg + 1) * 1024])
                    # XBAR transposes: vsb[:, jb, :] = Vf[:, jb, :].T, in
                    # two halves so jb0-7 stats don't wait on all drains
                    if q == 4:
                        nc.scalar.dma_start_transpose(
                            out=vsb[:, 0:1024].rearrange("p (b c) -> p b c",
                                                         c=128),
                            in_=Vf[:, 0:1024])
                    if q == 7:
                        nc.scalar.dma_start_transpose(
                            out=vsb[:, 1024:2048].rearrange(
                                "p (b c) -> p b c", c=128),
                            in_=Vf[:, 1024:2048])
                    for c in INTER.get(q, ()):
                        emit_chunk(c)

            stg_cm.__exit__(None, None, None)

            # ---- phase A remainder ----
            with tc.tile_pool(name="psBi", bufs=1, space="PSUM") as psBi:
                holder["psBi"] = psBi
                for cidx in range(16, 64):
                    emit_chunk(cidx)
                while pe_q:
                    pe_q.pop(0)[1]()

                # Y (groups 0-3) drain to outTa.
                for g in range(4):
                    if g % 2 == 0:
                        nc.scalar.copy(outTa[:, g * 512:(g + 1) * 512],
                                       bankY[g][:])
                    else:
                        nc.vector.tensor_copy(
                            outTa[:, g * 512:(g + 1) * 512], bankY[g][:])
            psA_cm.__exit__(None, None, None)
            out2a3 = out2a.rearrange("p (b t) -> p b t", t=128)
            with tc.tile_pool(name="psA2", bufs=1, space="PSUM") as psA2, \
                 tc.tile_pool(name="psC", bufs=2, space="PSUM") as psC, \
                 tc.tile_pool(name="psY", bufs=2, space="PSUM") as psY, \
                 tc.tile_pool(name="late", bufs=2) as late:

                # proj2 Gs complete in pairs; each pair's two y-slices per
                # cb ride one strided DMA to halve the descriptor train.
                PAIRS = [(0, 3), (6, 1), (4, 7), (2, 5)]
                pair_of = {g: (pi, hi) for pi, p in enumerate(PAIRS)
                           for hi, g in enumerate(p)}
                ygt = {}

                pyt = {}

                def proj2(G):
                    runs = _proj2_runs(G)
                    pi, hi = pair_of[G]
                    for cb in range(2):
                        if hi == 0:
                            pyt[(pi, cb)] = psY.tile(
                                [128, 1024], F32, name="py", tag="py")
                        lo = min(PAIRS[pi])
                        hc = PAIRS[pi][hi] != lo
                        py = pyt[(pi, cb)][:, 512:1024] if hc else \
                            pyt[(pi, cb)][:, 0:512]
                        off = 0
                        for ri, run in enumerate(runs):
                            w = 128 * len(run)
                            rhs = out2a3[:, run[0]:run[-1] + 1:3, :]
                            nc.tensor.matmul(
                                py[:, off:off + w],
                                woutT[:, cb * 128:(cb + 1) * 128], rhs,
                                start=(ri == 0), stop=(ri == len(runs) - 1),
                                skip_group_check=True,
                            )
                            off += w
                        if hi == 1:
                            # one pair-wide bias + one strided DMA per cb
                            key = (pi % 2, cb)
                            yg = late.tile([128, 1024], BF16,
                                           name=f"yg{key}", tag=f"yg{key}")
                            if cb == 0:
                                nc.scalar.activation(yg[:], pyt[(pi, cb)][:],
                                                     AF.Identity, bias=bo[cb])
                            else:
                                nc.vector.tensor_scalar_add(
                                    yg[:], pyt[(pi, cb)][:], bo[cb])
                            d = (max(PAIRS[pi]) - lo) * 512
                            outap = y_ext[cb * 128:(cb + 1) * 128,
                                          lo * 512:].rearrange(
                                "p (a c) -> p a c", c=512)[:, 0:d // 512 + 1:
                                                           d // 512, :]
                            eng = [nc.sync, nc.scalar, nc.gpsimd][
                                (pi * 2 + cb) % 3]
                            eng.dma_start(out=outap, in_=yg[:])

                def tp2(g0):
                    # transpose a pair of 512-col groups; one wide drain
                    tpc = psC.tile([128, 1024], BF16, name="tpc", tag="tpc")
                    for s in range(8):
                        nc.tensor.transpose(
                            tpc[:, s * 128:(s + 1) * 128],
                            outTa[:, g0 * 512 + s * 128:
                                  g0 * 512 + (s + 1) * 128],
                            ident)
                    if g0 % 4 == 0:
                        nc.scalar.copy(
                            out2a[:, g0 * 512:(g0 + 2) * 512], tpc[:])
                    else:
                        nc.vector.tensor_scalar_add(
                            out2a[:, g0 * 512:(g0 + 2) * 512], tpc[:], 0.0)

                # Y groups (0-3) completed at phase-A end; X tails chain
                # jb SPLIT_JB+1..15 on psA's freed banks, draining with the
                # spill added; TAIL_ORDER interleaves everything.
                bA2 = [None] * 4

                def xchain(g):
                    bA2[g] = psA2.tile([128, 512], F32, name=f"bA2{g % 2}",
                                       tag=f"bA2{g % 2}")
                    for jb in range(SPLIT_JB + 1, 16):
                        out_mm(bA2[g], g + 4, jb, jb == SPLIT_JB + 1, jb == 15)

                def xdrain(g):
                    nc.vector.tensor_tensor(
                        out=outTa[:, (g + 4) * 512:(g + 5) * 512],
                        in0=bA2[g][:], in1=spill[g][:], op=AluOpType.add)

                for step in TAIL_ORDER:
                    kind, arg = step
                    if kind == "x":
                        xchain(arg)
                    elif kind == "d":
                        xdrain(arg)
                    elif kind == "p":
                        proj2(arg)
                    elif kind == "t":
                        tp2(arg)
                # remaining: G7 g{5,5,6,7}, G2 g{6,6,7,0}, G5 g{7,7,0,1}
                for G in (7, 2, 5):
                    proj2(G)

        if debug_hook:
            debug_hook(nc, dict(QK8s=QK8s, vsb=vsb,
                                P=P, zsum=zsum, zinv=zinv, outTa=outTa,
                                out2a=out2a))

    nc.compile()
    return nc


def get_nc():
    if "nc" not in _CACHE:
        _CACHE["nc"] = build_nc()
    return _CACHE["nc"]


def make_in_maps(x, W_qkv, b_qkv, W_out, b_out):
    x = np.asarray(x, dtype=np.float32)
    W_qkv = np.asarray(W_qkv, dtype=np.float32)
    b_qkv = np.asarray(b_qkv, dtype=np.float32)
    W_out = np.asarray(W_out, dtype=np.float32)
    b_out = np.asarray(b_out, dtype=np.float32)

    operm = (np.arange(O) + O // 2) % O      # rotate qkv channels by 192
    eperm = (np.arange(E) + E // 2) % E      # rotate e-axis by 64

    halves = []
    for h in range(2):
        if h == 0:
            wq, bqv, wo, bov = W_qkv, b_qkv, W_out, b_out
        else:
            wq = W_qkv[operm]
            bqv = b_qkv[operm]
            wo = W_out[:, eperm]
            bov = np.zeros_like(b_out)
        orders = [
            [3 * t + r for t in range(128)]
            + [3 * t + (r + 2) % 3 for t in range(64)]
            + [3 * t + (r + 1) % 3 for t in range(64)]
            for r in range(3)
        ]
        wqv = np.concatenate([wq.T[:, o] for o in orders], axis=1)     # (C, 768)
        brv = np.concatenate([bqv[o][None, :] for o in orders], axis=1)  # (1, 768)
        halves.append({
            "wqkvT": np.ascontiguousarray(wqv).astype(ml_dtypes.bfloat16),
            "brow": np.ascontiguousarray(brv).astype(ml_dtypes.bfloat16),
            "woutT": np.ascontiguousarray(wo.T).astype(ml_dtypes.bfloat16),
            "bout": np.ascontiguousarray(bov.reshape(C, 1)),
        })

    xb = [np.ascontiguousarray(x[n].reshape(C, HW)).astype(ml_dtypes.bfloat16)
          for n in range(N)]
    in_maps = []
    for core in range(8):
        n, h = core // 2, core % 2
        m = {"x": xb[n]}
        m.update(halves[h])
        in_maps.append(m)
    return in_maps


def run(inputs, trace=False, **kw):
    nc = get_nc()
    in_maps = make_in_maps(**inputs)
    res = run_bass_kernel_spmd(nc, in_maps, core_ids=list(range(8)), trace=trace, **kw)
    ys = [np.asarray(res.results[i]["out"], dtype=np.float32) for i in range(8)]
    y = np.stack([ys[2 * n] + ys[2 * n + 1] for n in range(N)])
    return y.reshape(N, C, 64, 64), res


def kernel(**inputs):
    y, _ = run(inputs, trace=False)
    return y


# revision 55
# speedup vs baseline: 1.0752x; 1.0016x over previous
"""Trainium2 Bass kernel for nn_Attention (dense_transformer), v3.

Reference computation (per batch n of 4):
  qkv = W_qkv @ x + b          (384, 4096)   [x flattened to (256, 64*64)]
  raw C-order reinterpret of qkv flat buffer as (4096, 384) -> q|k|v (4096,128)
  scores = q @ k.T / 64        (4096, 4096)
  soft = softmax(scores, axis=-2)             [column softmax]
  out = soft @ v               (4096, 128)
  raw reinterpret of out as (128, 4096)
  y = W_out @ out2 + b_out     (256, 4096)

Sharding: 8 cores = 4 batches x 2 j-halves (t-halves of the permuted j
enumeration; the host-side 192-rotation of qkv channels and 64-rotation of
W_out's e-axis make the SPMD program identical on all cores). Host sums the
per-pair partial y.

v3 dataflow (vs v2): the q/k tensors are quantized to fp8e4 at the stage-1
drain and the score matmuls run in DoubleRow perf mode (2 fp8 rows per PE
pass, 2x throughput): contraction d=128 is split into two 64-partition
groups, with the d-hi half moved onto partitions 0:64 by SBUF->SBUF DMAs
(idle engines). exp runs on ACT with accum_out supplying the column-softmax
Z sums for most chunks; a subset of chunks uses a Schraudolph fast-exp on
DVE (bf16 bits = S*128*log2e/64 + 16249.7 via f32->u16 convert, written
through a bitcast view) plus a DVE row-reduce, to keep both engines busy.
v transposes ride the DMA XBAR instead of the PE. Bias enters PSUM via
ones-row matmuls so all stage-1 drains are plain copies. Out accumulation
rotates the 4 spare PSUM banks in two eras split at jb10: groups 4-7 live
jb0-10 then spill; groups 0-3 chain burst(jb0-10)+live(jb11-15); groups
4-7 finish jb11-15 in the tail on psA's freed banks, adding the spill at
drain. proj2 folds the psi_q permutation into stride-3 rhs gathers.
"""

import numpy as np
import ml_dtypes

import concourse.bass as bass
import concourse.bacc as bacc
import concourse.mybir as mybir
from concourse.bass_utils import run_bass_kernel_spmd
from concourse.tile import TileContext, add_dep_helper
from concourse.masks import make_identity
from concourse.alu_op_type import AluOpType

BF16 = mybir.dt.bfloat16
F32 = mybir.dt.float32
FP8 = mybir.dt.float8e4
U16 = mybir.dt.uint16
AF = mybir.ActivationFunctionType
AX = mybir.AxisListType
DR = mybir.MatmulPerfMode.DoubleRow

N, C, E, O, HW = 4, 256, 128, 384, 4096
JC = HW // 2          # j per core
NJB = JC // 128       # 16 j-blocks
SCALE = 1.0 / 64.0    # 1/sqrt(HW)
SPLIT_JB = 9          # era split for out accumulation
DIRECT_CIDX = 8       # chunks below this use unsplit fp8 scores
WARM_MMS = 16
SCHR_CIDX2 = None
FLUSH_BUDGET = 1
ORDER_D = False
TAIL_ORDER = [("t", 0), ("t", 2), ("p", 0), ("p", 3),
              ("x", 0), ("x", 1), ("d", 0), ("x", 2), ("d", 1), ("t", 4),
              ("x", 3), ("p", 6), ("d", 2), ("p", 1), ("d", 3), ("t", 6),
              ("p", 4)]

# Schraudolph fast-exp on DVE for a subset of chunks: bf16 bits of e^x are
# ~ x*(128*log2e) + 16249.7 (HW rounds on f32->u16 convert; rel rms ~1.8%).
SCHR_A = 128.0 * 1.4426950408889634 * SCALE
SCHR_B = 16249.7
# Schraudolph chunk set: spread over the h2 sweep and the h1 slots of the
# jb-minor phase, avoiding stats-critical h3 chunks and DVE-heavy clusters.
SCHR_CIDX = {16, 19, 22, 25, 28, 30, 33, 36, 39, 42, 45, 48, 51, 54, 57,
             60, 63}
SCHR_W = 1024

_CACHE = {}


def _psiq_inv(m):
    if m <= 10:
        return 3 * m
    if m <= 21:
        return 3 * (m - 11) + 1
    return 3 * (m - 22) + 2


def _proj2_runs(G):
    """Maximal stride-3 source-chunk runs feeding y columns [4G*128,(4G+4)*128)."""
    srcs = [_psiq_inv(4 * G + k) for k in range(4)]
    runs = []
    for s in srcs:
        if runs and s == runs[-1][-1] + 3:
            runs[-1].append(s)
        else:
            runs.append([s])
    return runs


def build_nc(debug_hook=None):
    nc = bacc.Bacc("TRN2", target_bir_lowering=False, debug=False, num_devices=8)

    x_ext = nc.dram_tensor("x", [C, HW], BF16, kind="ExternalInput").ap()
    wqkvT_ext = nc.dram_tensor("wqkvT", [C, 768], BF16, kind="ExternalInput").ap()
    brow_ext = nc.dram_tensor("brow", [1, 768], BF16, kind="ExternalInput").ap()
    woutT_ext = nc.dram_tensor("woutT", [E, C], BF16, kind="ExternalInput").ap()
    bout_ext = nc.dram_tensor("bout", [C, 1], F32, kind="ExternalInput").ap()
    y_ext = nc.dram_tensor("out", [C, HW], BF16, kind="ExternalOutput").ap()

    # persistent SBUF
    xsb = [nc.alloc_sbuf_tensor(f"x{cb}", [128, HW], BF16).ap() for cb in range(2)]
    QK8s = nc.alloc_sbuf_tensor("QK8s", [128, 2 * 6144], FP8).ap()
    vsb = nc.alloc_sbuf_tensor("vsb", [128, JC], BF16).ap()
    P = nc.alloc_sbuf_tensor("P", [128, NJB * HW], BF16).ap()
    outTa = nc.alloc_sbuf_tensor("outTa", [128, HW], BF16).ap()
    out2a = nc.alloc_sbuf_tensor("out2a", [128, HW], BF16).ap()
    spill = [nc.alloc_sbuf_tensor(f"spill{g}", [128, 512], F32).ap() for g in range(4)]
    zacc = nc.alloc_sbuf_tensor("zacc", [128, 128], F32).ap()
    zsum = nc.alloc_sbuf_tensor("zsum", [128, 16], F32).ap()
    zinv = nc.alloc_sbuf_tensor("zinv", [128, 16], F32).ap()


    # gathered layout: per g-half, q contiguous (4096) then k (2048)
    QK8sg = QK8s[0:64, :].rearrange("p (g c) -> p g c", g=2)

    def v_sl(jb):
        return vsb[:, jb * 128:(jb + 1) * 128]

    with TileContext(nc) as tc:
        with tc.tile_pool(name="consts", bufs=1) as consts:
            # ---- weights/constants ----
            nc.sync.dma_start(out=xsb[0][:, 0:512], in_=x_ext[0:128, 0:512])
            nc.sync.dma_start(out=xsb[1][:, 0:512], in_=x_ext[128:256, 0:512])
            brow = consts.tile([1, 768], BF16, name="brow", tag="brow")
            nc.sync.dma_start(out=brow, in_=brow_ext[:])
            wq_all = consts.tile([128, 2 * 768], BF16, name="wq_all", tag="wq_all")
            for cb in range(2):
                nc.sync.dma_start(out=wq_all[:, cb * 768:(cb + 1) * 768],
                                  in_=wqkvT_ext[cb * 128:(cb + 1) * 128, :])
            ones1 = consts.tile([1, 128], BF16, name="ones1", tag="ones1")
            nc.vector.memset(ones1[:], 1.0)

            def wq_sl(cb, r):
                return wq_all[:, cb * 768 + r * 256: cb * 768 + (r + 1) * 256]

            misc = consts.tile([128, C + 128], BF16, name="misc", tag="misc")
            woutT = misc[:, 0:C]
            ident = misc[:, C:C + 128]
            nc.gpsimd.dma_start(out=woutT, in_=woutT_ext[:])
            make_identity(nc, ident)
            bo2 = consts.tile([128, 2], F32, name="bo2", tag="bo2")
            bo = [bo2[:, cb:cb + 1] for cb in range(2)]
            for cb in range(2):
                nc.gpsimd.dma_start(out=bo[cb], in_=bout_ext[cb * 128:(cb + 1) * 128, :])
            nc.vector.memset(zacc[:], 0.0)
            # Exp table preload
            scratch = consts.tile([128, 1], F32, name="scratch", tag="scratch")
            nc.vector.memset(scratch[:], 0.0)
            nc.scalar.activation(scratch[:], scratch[:], AF.Exp)

            # ---- PE warmup (p-state ramp) ----
            wsrc = consts.tile([128, 128], BF16, name="wsrc", tag="wsrc")
            nc.vector.memset(wsrc[:], 1.0)
            with tc.tile_pool(name="psW", bufs=1, space="PSUM") as psW:
                wtile = psW.tile([128, 128], F32, tag="warm")
                for _ in range(WARM_MMS):
                    nc.tensor.matmul(wtile[:], wsrc[:], wsrc[:], start=True, stop=True)

            # ---- x loads: all on the HWDGE sync ring, 1024-col chunks
            #      interleaved cb0/cb1 in stage-1 consumption order ----
            for lo, hi in ((512, 1536), (1536, 2560), (2560, 3584), (3584, 4096)):
                nc.sync.dma_start(out=xsb[0][:, lo:hi], in_=x_ext[0:128, lo:hi])
                nc.sync.dma_start(out=xsb[1][:, lo:hi],
                                  in_=x_ext[128:256, lo:hi])

            # ---- phase A chunk order: h-sweeps over jb0-7 (software-
            #      pipelined into stage 1), then jb8-15 h-minor. ----
            order = []
            for h in range(2):
                for jb in range(8):
                    order.append((jb, h))
            if ORDER_D:
                for jb in range(8):
                    order.append((jb, 2))
                    order.append((jb, 3))
            else:
                for h in (2, 3):
                    for jb in range(8):
                        order.append((jb, h))
            for jb in range(8, 16):
                for h in range(4):
                    order.append((jb, h))

            psA_cm = tc.tile_pool(name="psA", bufs=2, space="PSUM")
            psA = psA_cm.__enter__()
            stg_cm = tc.tile_pool(name="stg", bufs=1)
            stg = stg_cm.__enter__()
            Qtmp = stg.tile([128, 32 * 128], FP8, name="Qtmp", tag="Qtmp")
            Ktmp = stg.tile([128, 32 * 64], FP8, name="Ktmp", tag="Ktmp")
            Vf = stg.tile([128, 32 * 64], BF16, name="Vf", tag="Vf")
            Vv = Vf.rearrange("p (b c) -> p b c", c=64)
            holder = {}
            bankX = [None] * 4   # i'-groups 4-7: live jb0..SPLIT_JB
            bankY = [None] * 4   # i'-groups 0-3: burst+live, stop jb15
            pe_q = []   # (ready_chunk_idx, emit_fn): deferred PE MMs

            def flush(cidx, budget=4):
                n = 0
                while pe_q and pe_q[0][0] <= cidx and n < budget:
                    pe_q.pop(0)[1]()
                    n += 1

            def stats(jb):
                nc.vector.reduce_sum(
                    out=zsum[:, jb:jb + 1], in_=zacc[:, jb * 8:(jb + 1) * 8],
                    axis=AX.X)
                nc.vector.reciprocal(zinv[:, jb:jb + 1], zsum[:, jb:jb + 1])
                nc.vector.tensor_scalar_mul(v_sl(jb), v_sl(jb),
                                            zinv[:, jb:jb + 1])

            def out_mm(bank, g, jb, start, stop):
                nc.tensor.matmul(
                    bank[:], v_sl(jb),
                    P[:, jb * HW + g * 512: jb * HW + (g + 1) * 512],
                    start=start, stop=stop,
                )

            def emit_chunk(cidx):
                jb, h = order[cidx]
                pa = psA.tile([128, 1024], F32, name="pa", tag="pa")
                for n2 in range(2):
                    if cidx < 16:
                        # unsplit fp8 (128-partition contraction, one 64-wide
                        # k block per MM, stacked via tile_position):
                        # independent of the gather DMAs
                        for t in range(2):
                            nc.tensor.matmul(
                                pa[64 * t:64 * t + 64,
                                   n2 * 512:(n2 + 1) * 512],
                                Ktmp[:, (2 * jb + t) * 64:
                                     (2 * jb + t + 1) * 64],
                                Qtmp[:, (8 * h + 4 * n2) * 128:
                                     (8 * h + 4 * n2 + 4) * 128],
                                start=True, stop=True,
                                tile_position=(0, 64 * t),
                            )
                    else:
                        o = (8 * h + 4 * n2) * 128
                        nc.tensor.matmul(
                            pa[:, n2 * 512:(n2 + 1) * 512],
                            QK8sg[:, :, 4096 + jb * 128:4096 + (jb + 1) * 128],
                            QK8sg[:, :, o:o + 512],
                            start=True, stop=True,
                            perf_mode=DR,
                        )
                psl = P[:, jb * HW + h * 1024: jb * HW + (h + 1) * 1024]
                zc0 = zacc[:, jb * 8 + 2 * h: jb * 8 + 2 * h + 1]
                zc1 = zacc[:, jb * 8 + 2 * h + 1: jb * 8 + 2 * h + 2]
                # Z per chunk via ACT accum_out (free row-sums). Split
                # chunks run half on DVE (Schraudolph fast-exp + reduce)
                # and half on ACT concurrently to balance the engines.
                if cidx in SCHR_CIDX:
                    w = SCHR_W
                    nc.vector.tensor_scalar(
                        out=psl[:, 0:w].bitcast(U16), in0=pa[:, 0:w],
                        scalar1=SCHR_A, scalar2=SCHR_B,
                        op0=AluOpType.mult, op1=AluOpType.add)
                    nc.vector.reduce_sum(out=zc0, in_=psl[:, 0:w],
                                         axis=AX.X)
                    if w < 1024:
                        nc.scalar.activation(out=psl[:, w:1024],
                                             in_=pa[:, w:1024], func=AF.Exp,
                                             scale=SCALE, accum_out=zc1)
                else:
                    nc.scalar.activation(out=psl, in_=pa[:], func=AF.Exp,
                                         scale=SCALE, accum_out=zc0)
                flush(cidx, budget=FLUSH_BUDGET)
                if h == 3:
                    stats(jb)
                    psBi = holder["psBi"]
                    if jb == 0:
                        for g in range(4):
                            bankX[g] = psBi.tile([128, 512], F32,
                                                 name=f"bk{g}", tag=f"bk{g}")
                    if jb <= SPLIT_JB:
                        for g in range(4):
                            pe_q.append((cidx + 2,
                                         (lambda g=g, jb=jb: out_mm(
                                             bankX[g], g + 4, jb,
                                             jb == 0, jb == SPLIT_JB))))
                    else:
                        for g in range(4):
                            pe_q.append((cidx + 4,
                                         (lambda g=g, jb=jb: out_mm(
                                             bankY[g], g, jb,
                                             False, jb == 15))))
                    if jb == SPLIT_JB:
                        # spill X banks; queue Y bursts (groups 0-3)
                        # jb0..SPLIT_JB from persistent P.
                        def spill_and_y():
                            for g in range(4):
                                nc.vector.tensor_copy(spill[g][:],
                                                      bankX[g][:])
                            for g in range(4):
                                bankY[g] = psBi.tile([128, 512], F32,
                                                     name=f"bk{g}",
                                                     tag=f"bk{g}")
                        pe_q.append((cidx + 2, spill_and_y))
                        for jbq in range(SPLIT_JB + 1):
                            for g in range(4):
                                pe_q.append((cidx + 2 + jbq // 2,
                                             (lambda g=g, jbq=jbq: out_mm(
                                                 bankY[g], g, jbq,
                                                 jbq == 0, False))))

            # chunks emitted between stage-1 quads (deps: k needs its jb's
            # quads, q needs quads 2h,2h+1; all cidx<16 are gather-free)
            INTER = {1: [0, 1, 2, 3], 2: [4, 5], 3: [6, 7, 8, 9, 10, 11],
                     4: [12, 13], 5: [14, 15]}

            # ---- stage 1: quads of FT blocks -> Qtmp/Ktmp (fp8) + Vf
            #      (bf16), gather DMAs -> QK8s, XBAR -> vsb; early phase-A
            #      chunks interleave (psF on banks 4-7, psA on 0-3). ----
            with tc.tile_pool(name="psF", bufs=2, space="PSUM") as psF:
                for q in range(8):
                    pf = psF.tile([128, 1024], F32, name="pf", tag="pf")
                    pf3 = pf.rearrange("p (s c) -> p s c", s=4)
                    for s in range(4):
                        sc = 4 * q + s
                        r = sc % 3
                        nc.tensor.matmul(pf3[:, s, 0:256], ones1[:],
                                         brow[:, r * 256:(r + 1) * 256],
                                         start=True, stop=False)
                        for cb in range(2):
                            nc.tensor.matmul(
                                pf3[:, s, 0:256],
                                xsb[cb][:, sc * 128:(sc + 1) * 128],
                                wq_sl(cb, r),
                                start=False, stop=(cb == 1),
                            )
                    cp = nc.vector.tensor_copy
                    cp(Qtmp[:, q * 512:(q + 1) * 512], pf3[:, :, 0:128])
                    cp(Ktmp[:, q * 256:(q + 1) * 256], pf3[:, :, 128:192])
                    # v tokens for k-block sc live in v-block sc+1: store the
                    # v drain one block down (with wrap) so vsb aligns with kT.
                    if q == 0:
                        cp(Vv[:, 31:32, :], pf3[:, 0:1, 192:256])
                        cp(Vv[:, 0:3, :], pf3[:, 1:4, 192:256])
                    else:
                        cp(Vv[:, 4 * q - 1:4 * q + 3, :], pf3[:, :, 192:256])
                    # gather DMAs per 16-block round: contiguous q/k copies
                    # of each d-half (g=1 is the partition-shifted 64:128
                    # half) into the per-g layout [q 4096 | k 2048].
                    if q % 4 == 3:
                        bg = q // 4
                        for g in range(2):
                            nc.sync.dma_start(
                                out=QK8s[0:64, g * 6144 + bg * 2048:
                                         g * 6144 + (bg + 1) * 2048],
                                in_=Qtmp[64 * g:64 * g + 64,
                                         bg * 2048:(bg + 1) * 2048])
                            nc.scalar.dma_start(
                                out=QK8s[0:64, g * 6144 + 4096 + bg * 1024:
                                         g * 6144 + 4096 + (bg + 1) * 1024],
                                in_=Ktmp[64 * g:64 * g + 64,
                                         bg * 1024:(bg + 1) * 1024])
                    # XBAR transposes: vsb[:, jb, :] = Vf[:, jb, :].T, in
                    # two halves so jb0-7 stats don't wait on all drains
                    if q == 4:
                        nc.scalar.dma_start_transpose(
                            out=vsb[:, 0:1024].rearrange("p (b c) -> p b c",
                                                         c=128),
                            in_=Vf[:, 0:1024])
                    if q == 7:
                        nc.scalar.dma_start_transpose(
                            out=vsb[:, 1024:2048].rearrange(
                                "p (b c) -> p b c", c=128),
                            in_=Vf[:, 1024:2048])
                    for c in INTER.get(q, ()):
                        emit_chunk(c)

            stg_cm.__exit__(None, None, None)

            # ---- phase A remainder ----
            with tc.tile_pool(name="psBi", bufs=1, space="PSUM") as psBi:
                holder["psBi"] = psBi
                for cidx in range(16, 64):
                    emit_chunk(cidx)
                while pe_q:
                    pe_q.pop(0)[1]()

                # Y (groups 0-3) drain to outTa.
                for g in range(4):
                    if g % 2 == 0:
                        nc.scalar.copy(outTa[:, g * 512:(g + 1) * 512],
                                       bankY[g][:])
                    else:
                        nc.vector.tensor_copy(
                            outTa[:, g * 512:(g + 1) * 512], bankY[g][:])
            psA_cm.__exit__(None, None, None)
            out2a3 = out2a.rearrange("p (b t) -> p b t", t=128)
            with tc.tile_pool(name="psA2", bufs=1, space="PSUM") as psA2, \
                 tc.tile_pool(name="psC", bufs=2, space="PSUM") as psC, \
                 tc.tile_pool(name="psY", bufs=2, space="PSUM") as psY, \
                 tc.tile_pool(name="late", bufs=2) as late:

                # proj2 Gs complete in pairs; each pair's two y-slices per
                # cb ride one strided DMA to halve the descriptor train.
                PAIRS = [(0, 3), (6, 1), (4, 7), (2, 5)]
                pair_of = {g: (pi, hi) for pi, p in enumerate(PAIRS)
                           for hi, g in enumerate(p)}
                ygt = {}

                pyt = {}

                def proj2(G):
                    runs = _proj2_runs(G)
                    pi, hi = pair_of[G]
                    for cb in range(2):
                        if hi == 0:
                            pyt[(pi, cb)] = psY.tile(
                                [128, 1024], F32, name="py", tag="py")
                        lo = min(PAIRS[pi])
                        hc = PAIRS[pi][hi] != lo
                        py = pyt[(pi, cb)][:, 512:1024] if hc else \
                            pyt[(pi, cb)][:, 0:512]
                        off = 0
                        for ri, run in enumerate(runs):
                            w = 128 * len(run)
                            rhs = out2a3[:, run[0]:run[-1] + 1:3, :]
                            nc.tensor.matmul(
                                py[:, off:off + w],
                                woutT[:, cb * 128:(cb + 1) * 128], rhs,
                                start=(ri == 0), stop=(ri == len(runs) - 1),
                                skip_group_check=True,
                            )
                            off += w
                        if hi == 1:
                            # one pair-wide bias + one strided DMA per cb
                            key = (pi % 2, cb)
                            yg = late.tile([128, 1024], BF16,
                                           name=f"yg{key}", tag=f"yg{key}")
                            if cb == 0:
                                nc.scalar.activation(yg[:], pyt[(pi, cb)][:],
                                                     AF.Identity, bias=bo[cb])
                            else:
                                nc.vector.tensor_scalar_add(
                                    yg[:], pyt[(pi, cb)][:], bo[cb])
                            d = (max(PAIRS[pi]) - lo) * 512
                            outap = y_ext[cb * 128:(cb + 1) * 128,
                                          lo * 512:].rearrange(
                                "p (a c) -> p a c", c=512)[:, 0:d // 512 + 1:
                                                           d // 512, :]
                            eng = [nc.sync, nc.scalar, nc.gpsimd][
                                (pi * 2 + cb) % 3]
                            eng.dma_start(out=outap, in_=yg[:])

                def tp2(g0):
                    # transpose a pair of 512-col groups; one wide drain
                    tpc = psC.tile([128, 1024], BF16, name="tpc", tag="tpc")
                    for s in range(8):
                        nc.tensor.transpose(
                            tpc[:, s * 128:(s + 1) * 128],
                            outTa[:, g0 * 512 + s * 128:
                                  g0 * 512 + (s + 1) * 128],
                            ident)
                    if g0 % 4 == 0:
                        nc.scalar.copy(
                            out2a[:, g0 * 512:(g0 + 2) * 512], tpc[:])
                    else:
                        nc.vector.tensor_scalar_add(
                            out2a[:, g0 * 512:(g0 + 2) * 512], tpc[:], 0.0)

                # Y groups (0-3) completed at phase-A end; X tails chain
                # jb SPLIT_JB+1..15 on psA's freed banks, draining with the
                # spill added; TAIL_ORDER interleaves everything.
                bA2 = [None] * 4

                def xchain(g):
                    bA2[g] = psA2.tile([128, 512], F32, name=f"bA2{g % 2}",
                                       tag=f"bA2{g % 2}")
                    for jb in range(SPLIT_JB + 1, 16):
                        out_mm(bA2[g], g + 4, jb, jb == SPLIT_JB + 1, jb == 15)

                def xdrain(g):
                    nc.vector.tensor_tensor(
                        out=outTa[:, (g + 4) * 512:(g + 5) * 512],
                        in0=bA2[g][:], in1=spill[g][:], op=AluOpType.add)

                for step in TAIL_ORDER:
                    kind, arg = step
                    if kind == "x":
                        xchain(arg)
                    elif kind == "d":
                        xdrain(arg)
                    elif kind == "p":
                        proj2(arg)
                    elif kind == "t":
                        tp2(arg)
                # remaining: G7 g{5,5,6,7}, G2 g{6,6,7,0}, G5 g{7,7,0,1}
                for G in (7, 2, 5):
                    proj2(G)

        if debug_hook:
            debug_hook(nc, dict(QK8s=QK8s, vsb=vsb,
                                P=P, zsum=zsum, zinv=zinv, outTa=outTa,
                                out2a=out2a))

    nc.compile()
    return nc


def get_nc():
    if "nc" not in _CACHE:
        _CACHE["nc"] = build_nc()
    return _CACHE["nc"]


def make_in_maps(x, W_qkv, b_qkv, W_out, b_out):
    x = np.asarray(x, dtype=np.float32)
    W_qkv = np.asarray(W_qkv, dtype=np.float32)
    b_qkv = np.asarray(b_qkv, dtype=np.float32)
    W_out = np.asarray(W_out, dtype=np.float32)
    b_out = np.asarray(b_out, dtype=np.float32)

    operm = (np.arange(O) + O // 2) % O      # rotate qkv channels by 192
    eperm = (np.arange(E) + E // 2) % E      # rotate e-axis by 64

    halves = []
    for h in range(2):
        if h == 0:
            wq, bqv, wo, bov = W_qkv, b_qkv, W_out, b_out
        else:
            wq = W_qkv[operm]
            bqv = b_qkv[operm]
            wo = W_out[:, eperm]
            bov = np.zeros_like(b_out)
        orders = [
            [3 * t + r for t in range(128)]
            + [3 * t + (r + 2) % 3 for t in range(64)]
            + [3 * t + (r + 1) % 3 for t in range(64)]
            for r in range(3)
        ]
        wqv = np.concatenate([wq.T[:, o] for o in orders], axis=1)     # (C, 768)
        brv = np.concatenate([bqv[o][None, :] for o in orders], axis=1)  # (1, 768)
        halves.append({
            "wqkvT": np.ascontiguousarray(wqv).astype(ml_dtypes.bfloat16),
            "brow": np.ascontiguousarray(brv).astype(ml_dtypes.bfloat16),
            "woutT": np.ascontiguousarray(wo.T).astype(ml_dtypes.bfloat16),
            "bout": np.ascontiguousarray(bov.reshape(C, 1)),
        })

    xb = [np.ascontiguousarray(x[n].reshape(C, HW)).astype(ml_dtypes.bfloat16)
          for n in range(N)]
    in_maps = []
    for core in range(8):
        n, h = core // 2, core % 2
        m = {"x": xb[n]}
        m.update(halves[h])
        in_maps.append(m)
    return in_maps


def run(inputs, trace=False, **kw):
    nc = get_nc()
    in_maps = make_in_maps(**inputs)
    res = run_bass_kernel_spmd(nc, in_maps, core_ids=list(range(8)), trace=trace, **kw)
    ys = [np.asarray(res.results[i]["out"], dtype=np.float32) for i in range(8)]
    y = np.stack([ys[2 * n] + ys[2 * n + 1] for n in range(N)])
    return y.reshape(N, C, 64, 64), res


def kernel(**inputs):
    y, _ = run(inputs, trace=False)
    return y


# revision 57
# speedup vs baseline: 1.0841x; 1.0083x over previous
"""Trainium2 Bass kernel for nn_Attention (dense_transformer), v3.

Reference computation (per batch n of 4):
  qkv = W_qkv @ x + b          (384, 4096)   [x flattened to (256, 64*64)]
  raw C-order reinterpret of qkv flat buffer as (4096, 384) -> q|k|v (4096,128)
  scores = q @ k.T / 64        (4096, 4096)
  soft = softmax(scores, axis=-2)             [column softmax]
  out = soft @ v               (4096, 128)
  raw reinterpret of out as (128, 4096)
  y = W_out @ out2 + b_out     (256, 4096)

Sharding: 8 cores = 4 batches x 2 j-halves (t-halves of the permuted j
enumeration; the host-side 192-rotation of qkv channels and 64-rotation of
W_out's e-axis make the SPMD program identical on all cores). Host sums the
per-pair partial y.

v3 dataflow (vs v2): the q/k tensors are quantized to fp8e4 at the stage-1
drain and the score matmuls run in DoubleRow perf mode (2 fp8 rows per PE
pass, 2x throughput): contraction d=128 is split into two 64-partition
groups, with the d-hi half moved onto partitions 0:64 by SBUF->SBUF DMAs
(idle engines). exp runs on ACT with accum_out supplying the column-softmax
Z sums for most chunks; a subset of chunks uses a Schraudolph fast-exp on
DVE (bf16 bits = S*128*log2e/64 + 16249.7 via f32->u16 convert, written
through a bitcast view) plus a DVE row-reduce, to keep both engines busy.
v transposes ride the DMA XBAR instead of the PE. Bias enters PSUM via
ones-row matmuls so all stage-1 drains are plain copies. Out accumulation
rotates the 4 spare PSUM banks in two eras split at jb10: groups 4-7 live
jb0-10 then spill; groups 0-3 chain burst(jb0-10)+live(jb11-15); groups
4-7 finish jb11-15 in the tail on psA's freed banks, adding the spill at
drain. proj2 folds the psi_q permutation into stride-3 rhs gathers.
"""

import numpy as np
import ml_dtypes

import concourse.bass as bass
import concourse.bacc as bacc
import concourse.mybir as mybir
from concourse.bass_utils import run_bass_kernel_spmd
from concourse.tile import TileContext, add_dep_helper
from concourse.masks import make_identity
from concourse.alu_op_type import AluOpType

BF16 = mybir.dt.bfloat16
F32 = mybir.dt.float32
FP8 = mybir.dt.float8e4
U16 = mybir.dt.uint16
AF = mybir.ActivationFunctionType
AX = mybir.AxisListType
DR = mybir.MatmulPerfMode.DoubleRow

N, C, E, O, HW = 4, 256, 128, 384, 4096
JC = HW // 2          # j per core
NJB = JC // 128       # 16 j-blocks
SCALE = 1.0 / 64.0    # 1/sqrt(HW)
SPLIT_JB = 9          # era split for out accumulation
DIRECT_CIDX = 8       # chunks below this use unsplit fp8 scores
WARM_MMS = 16
SCHR_CIDX2 = None
FLUSH_BUDGET = 1
ORDER_D = False
DRAIN_ACT_QUADS = 2   # quads below this drain on ACT instead of DVE
SPILL_ACT = False
XRDY = 2
YRDY = 4
TAIL_ORDER = [("t", 0), ("t", 2), ("p", 0), ("p", 3),
              ("x", 0), ("x", 1), ("d", 0), ("x", 2), ("d", 1), ("t", 4),
              ("x", 3), ("p", 6), ("d", 2), ("p", 1), ("d", 3), ("t", 6),
              ("p", 4)]

# Schraudolph fast-exp on DVE for a subset of chunks: bf16 bits of e^x are
# ~ x*(128*log2e) + 16249.7 (HW rounds on f32->u16 convert; rel rms ~1.8%).
SCHR_A = 128.0 * 1.4426950408889634 * SCALE
SCHR_B = 16249.7
# Schraudolph chunk set: spread over the h2 sweep and the h1 slots of the
# jb-minor phase, avoiding stats-critical h3 chunks and DVE-heavy clusters.
SCHR_CIDX = {16, 19, 22, 25, 28, 30, 33, 36, 39, 42, 45, 48, 51, 54, 57,
             60, 63}
SCHR_W = 1024

_CACHE = {}


def _psiq_inv(m):
    if m <= 10:
        return 3 * m
    if m <= 21:
        return 3 * (m - 11) + 1
    return 3 * (m - 22) + 2


def _proj2_runs(G):
    """Maximal stride-3 source-chunk runs feeding y columns [4G*128,(4G+4)*128)."""
    srcs = [_psiq_inv(4 * G + k) for k in range(4)]
    runs = []
    for s in srcs:
        if runs and s == runs[-1][-1] + 3:
            runs[-1].append(s)
        else:
            runs.append([s])
    return runs


def build_nc(debug_hook=None):
    nc = bacc.Bacc("TRN2", target_bir_lowering=False, debug=False, num_devices=8)

    x_ext = nc.dram_tensor("x", [C, HW], BF16, kind="ExternalInput").ap()
    wqkvT_ext = nc.dram_tensor("wqkvT", [C, 768], BF16, kind="ExternalInput").ap()
    brow_ext = nc.dram_tensor("brow", [1, 768], BF16, kind="ExternalInput").ap()
    woutT_ext = nc.dram_tensor("woutT", [E, C], BF16, kind="ExternalInput").ap()
    bout_ext = nc.dram_tensor("bout", [C, 1], F32, kind="ExternalInput").ap()
    y_ext = nc.dram_tensor("out", [C, HW], BF16, kind="ExternalOutput").ap()

    # persistent SBUF
    xsb = [nc.alloc_sbuf_tensor(f"x{cb}", [128, HW], BF16).ap() for cb in range(2)]
    QK8s = nc.alloc_sbuf_tensor("QK8s", [128, 2 * 6144], FP8).ap()
    vsb = nc.alloc_sbuf_tensor("vsb", [128, JC], BF16).ap()
    P = nc.alloc_sbuf_tensor("P", [128, NJB * HW], BF16).ap()
    outTa = nc.alloc_sbuf_tensor("outTa", [128, HW], BF16).ap()
    out2a = nc.alloc_sbuf_tensor("out2a", [128, HW], BF16).ap()
    spill = [nc.alloc_sbuf_tensor(f"spill{g}", [128, 512], F32).ap() for g in range(4)]
    zacc = nc.alloc_sbuf_tensor("zacc", [128, 128], F32).ap()
    zsum = nc.alloc_sbuf_tensor("zsum", [128, 16], F32).ap()
    zinv = nc.alloc_sbuf_tensor("zinv", [128, 16], F32).ap()


    # gathered layout: per g-half, q contiguous (4096) then k (2048)
    QK8sg = QK8s[0:64, :].rearrange("p (g c) -> p g c", g=2)

    def v_sl(jb):
        return vsb[:, jb * 128:(jb + 1) * 128]

    with TileContext(nc) as tc:
        with tc.tile_pool(name="consts", bufs=1) as consts:
            # ---- weights/constants ----
            nc.sync.dma_start(out=xsb[0][:, 0:512], in_=x_ext[0:128, 0:512])
            nc.sync.dma_start(out=xsb[1][:, 0:512], in_=x_ext[128:256, 0:512])
            brow = consts.tile([1, 768], BF16, name="brow", tag="brow")
            nc.sync.dma_start(out=brow, in_=brow_ext[:])
            wq_all = consts.tile([128, 2 * 768], BF16, name="wq_all", tag="wq_all")
            for cb in range(2):
                nc.sync.dma_start(out=wq_all[:, cb * 768:(cb + 1) * 768],
                                  in_=wqkvT_ext[cb * 128:(cb + 1) * 128, :])
            ones1 = consts.tile([1, 128], BF16, name="ones1", tag="ones1")
            nc.vector.memset(ones1[:], 1.0)

            def wq_sl(cb, r):
                return wq_all[:, cb * 768 + r * 256: cb * 768 + (r + 1) * 256]

            misc = consts.tile([128, C + 128], BF16, name="misc", tag="misc")
            woutT = misc[:, 0:C]
            ident = misc[:, C:C + 128]
            nc.gpsimd.dma_start(out=woutT, in_=woutT_ext[:])
            make_identity(nc, ident)
            bo2 = consts.tile([128, 2], F32, name="bo2", tag="bo2")
            bo = [bo2[:, cb:cb + 1] for cb in range(2)]
            for cb in range(2):
                nc.gpsimd.dma_start(out=bo[cb], in_=bout_ext[cb * 128:(cb + 1) * 128, :])
            nc.vector.memset(zacc[:], 0.0)
            # Exp table preload
            scratch = consts.tile([128, 1], F32, name="scratch", tag="scratch")
            nc.vector.memset(scratch[:], 0.0)
            nc.scalar.activation(scratch[:], scratch[:], AF.Exp)

            # ---- PE warmup (p-state ramp) ----
            wsrc = consts.tile([128, 128], BF16, name="wsrc", tag="wsrc")
            nc.vector.memset(wsrc[:], 1.0)
            with tc.tile_pool(name="psW", bufs=1, space="PSUM") as psW:
                wtile = psW.tile([128, 128], F32, tag="warm")
                for _ in range(WARM_MMS):
                    nc.tensor.matmul(wtile[:], wsrc[:], wsrc[:], start=True, stop=True)

            # ---- x loads: all on the HWDGE sync ring, 1024-col chunks
            #      interleaved cb0/cb1 in stage-1 consumption order ----
            for lo, hi in ((512, 1536), (1536, 2560), (2560, 3584), (3584, 4096)):
                nc.sync.dma_start(out=xsb[0][:, lo:hi], in_=x_ext[0:128, lo:hi])
                nc.sync.dma_start(out=xsb[1][:, lo:hi],
                                  in_=x_ext[128:256, lo:hi])

            # ---- phase A chunk order: h-sweeps over jb0-7 (software-
            #      pipelined into stage 1), then jb8-15 h-minor. ----
            order = []
            for h in range(2):
                for jb in range(8):
                    order.append((jb, h))
            if ORDER_D:
                for jb in range(8):
                    order.append((jb, 2))
                    order.append((jb, 3))
            else:
                for h in (2, 3):
                    for jb in range(8):
                        order.append((jb, h))
            for jb in range(8, 16):
                for h in range(4):
                    order.append((jb, h))

            psA_cm = tc.tile_pool(name="psA", bufs=2, space="PSUM")
            psA = psA_cm.__enter__()
            stg_cm = tc.tile_pool(name="stg", bufs=1)
            stg = stg_cm.__enter__()
            Qtmp = stg.tile([128, 32 * 128], FP8, name="Qtmp", tag="Qtmp")
            Ktmp = stg.tile([128, 32 * 64], FP8, name="Ktmp", tag="Ktmp")
            Vf = stg.tile([128, 32 * 64], BF16, name="Vf", tag="Vf")
            Vv = Vf.rearrange("p (b c) -> p b c", c=64)
            holder = {}
            bankX = [None] * 4   # i'-groups 4-7: live jb0..SPLIT_JB
            bankY = [None] * 4   # i'-groups 0-3: burst+live, stop jb15
            pe_q = []   # (ready_chunk_idx, emit_fn): deferred PE MMs

            def flush(cidx, budget=4):
                n = 0
                while pe_q and pe_q[0][0] <= cidx and n < budget:
                    pe_q.pop(0)[1]()
                    n += 1

            def stats(jb):
                nc.vector.reduce_sum(
                    out=zsum[:, jb:jb + 1], in_=zacc[:, jb * 8:(jb + 1) * 8],
                    axis=AX.X)
                nc.vector.reciprocal(zinv[:, jb:jb + 1], zsum[:, jb:jb + 1])
                nc.vector.tensor_scalar_mul(v_sl(jb), v_sl(jb),
                                            zinv[:, jb:jb + 1])

            def out_mm(bank, g, jb, start, stop):
                nc.tensor.matmul(
                    bank[:], v_sl(jb),
                    P[:, jb * HW + g * 512: jb * HW + (g + 1) * 512],
                    start=start, stop=stop,
                )

            def emit_chunk(cidx):
                jb, h = order[cidx]
                pa = psA.tile([128, 1024], F32, name="pa", tag="pa")
                for n2 in range(2):
                    if cidx < 16:
                        # unsplit fp8 (128-partition contraction, one 64-wide
                        # k block per MM, stacked via tile_position):
                        # independent of the gather DMAs
                        for t in range(2):
                            nc.tensor.matmul(
                                pa[64 * t:64 * t + 64,
                                   n2 * 512:(n2 + 1) * 512],
                                Ktmp[:, (2 * jb + t) * 64:
                                     (2 * jb + t + 1) * 64],
                                Qtmp[:, (8 * h + 4 * n2) * 128:
                                     (8 * h + 4 * n2 + 4) * 128],
                                start=True, stop=True,
                                tile_position=(0, 64 * t),
                            )
                    else:
                        o = (8 * h + 4 * n2) * 128
                        nc.tensor.matmul(
                            pa[:, n2 * 512:(n2 + 1) * 512],
                            QK8sg[:, :, 4096 + jb * 128:4096 + (jb + 1) * 128],
                            QK8sg[:, :, o:o + 512],
                            start=True, stop=True,
                            perf_mode=DR,
                        )
                psl = P[:, jb * HW + h * 1024: jb * HW + (h + 1) * 1024]
                zc0 = zacc[:, jb * 8 + 2 * h: jb * 8 + 2 * h + 1]
                zc1 = zacc[:, jb * 8 + 2 * h + 1: jb * 8 + 2 * h + 2]
                # Z per chunk via ACT accum_out (free row-sums). Split
                # chunks run half on DVE (Schraudolph fast-exp + reduce)
                # and half on ACT concurrently to balance the engines.
                if cidx in SCHR_CIDX:
                    w = SCHR_W
                    nc.vector.tensor_scalar(
                        out=psl[:, 0:w].bitcast(U16), in0=pa[:, 0:w],
                        scalar1=SCHR_A, scalar2=SCHR_B,
                        op0=AluOpType.mult, op1=AluOpType.add)
                    nc.vector.reduce_sum(out=zc0, in_=psl[:, 0:w],
                                         axis=AX.X)
                    if w < 1024:
                        nc.scalar.activation(out=psl[:, w:1024],
                                             in_=pa[:, w:1024], func=AF.Exp,
                                             scale=SCALE, accum_out=zc1)
                else:
                    nc.scalar.activation(out=psl, in_=pa[:], func=AF.Exp,
                                         scale=SCALE, accum_out=zc0)
                flush(cidx, budget=FLUSH_BUDGET)
                if h == 3:
                    stats(jb)
                    psBi = holder["psBi"]
                    if jb == 0:
                        for g in range(4):
                            bankX[g] = psBi.tile([128, 512], F32,
                                                 name=f"bk{g}", tag=f"bk{g}")
                    if jb <= SPLIT_JB:
                        for g in range(4):
                            pe_q.append((cidx + XRDY,
                                         (lambda g=g, jb=jb: out_mm(
                                             bankX[g], g + 4, jb,
                                             jb == 0, jb == SPLIT_JB))))
                    else:
                        for g in range(4):
                            pe_q.append((cidx + YRDY,
                                         (lambda g=g, jb=jb: out_mm(
                                             bankY[g], g, jb,
                                             False, jb == 15))))
                    if jb == SPLIT_JB:
                        # spill X banks; queue Y bursts (groups 0-3)
                        # jb0..SPLIT_JB from persistent P.
                        def spill_and_y():
                            for g in range(4):
                                if SPILL_ACT and g % 2 == 0:
                                    nc.scalar.copy(spill[g][:], bankX[g][:])
                                else:
                                    nc.vector.tensor_copy(spill[g][:],
                                                          bankX[g][:])
                            for g in range(4):
                                bankY[g] = psBi.tile([128, 512], F32,
                                                     name=f"bk{g}",
                                                     tag=f"bk{g}")
                        pe_q.append((cidx + 2, spill_and_y))
                        for jbq in range(SPLIT_JB + 1):
                            for g in range(4):
                                pe_q.append((cidx + 2 + jbq // 2,
                                             (lambda g=g, jbq=jbq: out_mm(
                                                 bankY[g], g, jbq,
                                                 jbq == 0, False))))

            # chunks emitted between stage-1 quads (deps: k needs its jb's
            # quads, q needs quads 2h,2h+1; all cidx<16 are gather-free)
            INTER = {1: [0, 1, 2, 3], 2: [4, 5], 3: [6, 7, 8, 9, 10, 11],
                     4: [12, 13], 5: [14, 15]}

            # ---- stage 1: quads of FT blocks -> Qtmp/Ktmp (fp8) + Vf
            #      (bf16), gather DMAs -> QK8s, XBAR -> vsb; early phase-A
            #      chunks interleave (psF on banks 4-7, psA on 0-3). ----
            with tc.tile_pool(name="psF", bufs=2, space="PSUM") as psF:
                for q in range(8):
                    pf = psF.tile([128, 1024], F32, name="pf", tag="pf")
                    pf3 = pf.rearrange("p (s c) -> p s c", s=4)
                    for s in range(4):
                        sc = 4 * q + s
                        r = sc % 3
                        nc.tensor.matmul(pf3[:, s, 0:256], ones1[:],
                                         brow[:, r * 256:(r + 1) * 256],
                                         start=True, stop=False)
                        for cb in range(2):
                            nc.tensor.matmul(
                                pf3[:, s, 0:256],
                                xsb[cb][:, sc * 128:(sc + 1) * 128],
                                wq_sl(cb, r),
                                start=False, stop=(cb == 1),
                            )
                    cp = (nc.scalar.copy if q < DRAIN_ACT_QUADS
                          else nc.vector.tensor_copy)
                    cp(Qtmp[:, q * 512:(q + 1) * 512], pf3[:, :, 0:128])
                    cp(Ktmp[:, q * 256:(q + 1) * 256], pf3[:, :, 128:192])
                    # v tokens for k-block sc live in v-block sc+1: store the
                    # v drain one block down (with wrap) so vsb aligns with kT.
                    if q == 0:
                        cp(Vv[:, 31:32, :], pf3[:, 0:1, 192:256])
                        cp(Vv[:, 0:3, :], pf3[:, 1:4, 192:256])
                    else:
                        cp(Vv[:, 4 * q - 1:4 * q + 3, :], pf3[:, :, 192:256])
                    # gather DMAs per 16-block round: contiguous q/k copies
                    # of each d-half (g=1 is the partition-shifted 64:128
                    # half) into the per-g layout [q 4096 | k 2048].
                    if q % 4 == 3:
                        bg = q // 4
                        for g in range(2):
                            nc.sync.dma_start(
                                out=QK8s[0:64, g * 6144 + bg * 2048:
                                         g * 6144 + (bg + 1) * 2048],
                                in_=Qtmp[64 * g:64 * g + 64,
                                         bg * 2048:(bg + 1) * 2048])
                            nc.scalar.dma_start(
                                out=QK8s[0:64, g * 6144 + 4096 + bg * 1024:
                                         g * 6144 + 4096 + (bg + 1) * 1024],
                                in_=Ktmp[64 * g:64 * g + 64,
                                         bg * 1024:(bg + 1) * 1024])
                    # XBAR transposes: vsb[:, jb, :] = Vf[:, jb, :].T, in
                    # two halves so jb0-7 stats don't wait on all drains
                    if q == 4:
                        nc.scalar.dma_start_transpose(
                            out=vsb[:, 0:1024].rearrange("p (b c) -> p b c",
                                                         c=128),
                            in_=Vf[:, 0:1024])
                    if q == 7:
                        nc.scalar.dma_start_transpose(
                            out=vsb[:, 1024:2048].rearrange(
                                "p (b c) -> p b c", c=128),
                            in_=Vf[:, 1024:2048])
                    for c in INTER.get(q, ()):
                        emit_chunk(c)

            stg_cm.__exit__(None, None, None)

            # ---- phase A remainder ----
            with tc.tile_pool(name="psBi", bufs=1, space="PSUM") as psBi:
                holder["psBi"] = psBi
                for cidx in range(16, 64):
                    emit_chunk(cidx)
                while pe_q:
                    pe_q.pop(0)[1]()

                # Y (groups 0-3) drain to outTa.
                for g in range(4):
                    if g % 2 == 0:
                        nc.scalar.copy(outTa[:, g * 512:(g + 1) * 512],
                                       bankY[g][:])
                    else:
                        nc.vector.tensor_copy(
                            outTa[:, g * 512:(g + 1) * 512], bankY[g][:])
            psA_cm.__exit__(None, None, None)
            out2a3 = out2a.rearrange("p (b t) -> p b t", t=128)
            with tc.tile_pool(name="psA2", bufs=1, space="PSUM") as psA2, \
                 tc.tile_pool(name="psC", bufs=2, space="PSUM") as psC, \
                 tc.tile_pool(name="psY", bufs=2, space="PSUM") as psY, \
                 tc.tile_pool(name="late", bufs=2) as late:

                # proj2 Gs complete in pairs; each pair's two y-slices per
                # cb ride one strided DMA to halve the descriptor train.
                PAIRS = [(0, 3), (6, 1), (4, 7), (2, 5)]
                pair_of = {g: (pi, hi) for pi, p in enumerate(PAIRS)
                           for hi, g in enumerate(p)}
                ygt = {}

                pyt = {}

                def proj2(G):
                    runs = _proj2_runs(G)
                    pi, hi = pair_of[G]
                    for cb in range(2):
                        if hi == 0:
                            pyt[(pi, cb)] = psY.tile(
                                [128, 1024], F32, name="py", tag="py")
                        lo = min(PAIRS[pi])
                        hc = PAIRS[pi][hi] != lo
                        py = pyt[(pi, cb)][:, 512:1024] if hc else \
                            pyt[(pi, cb)][:, 0:512]
                        off = 0
                        for ri, run in enumerate(runs):
                            w = 128 * len(run)
                            rhs = out2a3[:, run[0]:run[-1] + 1:3, :]
                            nc.tensor.matmul(
                                py[:, off:off + w],
                                woutT[:, cb * 128:(cb + 1) * 128], rhs,
                                start=(ri == 0), stop=(ri == len(runs) - 1),
                                skip_group_check=True,
                            )
                            off += w
                        if hi == 1:
                            # one pair-wide bias + one strided DMA per cb
                            key = (pi % 2, cb)
                            yg = late.tile([128, 1024], BF16,
                                           name=f"yg{key}", tag=f"yg{key}")
                            if cb == 0:
                                nc.scalar.activation(yg[:], pyt[(pi, cb)][:],
                                                     AF.Identity, bias=bo[cb])
                            else:
                                nc.vector.tensor_scalar_add(
                                    yg[:], pyt[(pi, cb)][:], bo[cb])
                            d = (max(PAIRS[pi]) - lo) * 512
                            outap = y_ext[cb * 128:(cb + 1) * 128,
                                          lo * 512:].rearrange(
                                "p (a c) -> p a c", c=512)[:, 0:d // 512 + 1:
                                                           d // 512, :]
                            eng = [nc.sync, nc.scalar, nc.gpsimd][
                                (pi * 2 + cb) % 3]
                            eng.dma_start(out=outap, in_=yg[:])

                def tp2(g0):
                    # transpose a pair of 512-col groups; one wide drain
                    tpc = psC.tile([128, 1024], BF16, name="tpc", tag="tpc")
                    for s in range(8):
                        nc.tensor.transpose(
                            tpc[:, s * 128:(s + 1) * 128],
                            outTa[:, g0 * 512 + s * 128:
                                  g0 * 512 + (s + 1) * 128],
                            ident)
                    if g0 % 4 == 0:
                        nc.scalar.copy(
                            out2a[:, g0 * 512:(g0 + 2) * 512], tpc[:])
                    else:
                        nc.vector.tensor_scalar_add(
                            out2a[:, g0 * 512:(g0 + 2) * 512], tpc[:], 0.0)

                # Y groups (0-3) completed at phase-A end; X tails chain
                # jb SPLIT_JB+1..15 on psA's freed banks, draining with the
                # spill added; TAIL_ORDER interleaves everything.
                bA2 = [None] * 4

                def xchain(g):
                    bA2[g] = psA2.tile([128, 512], F32, name=f"bA2{g % 2}",
                                       tag=f"bA2{g % 2}")
                    for jb in range(SPLIT_JB + 1, 16):
                        out_mm(bA2[g], g + 4, jb, jb == SPLIT_JB + 1, jb == 15)

                def xdrain(g):
                    nc.vector.tensor_tensor(
                        out=outTa[:, (g + 4) * 512:(g + 5) * 512],
                        in0=bA2[g][:], in1=spill[g][:], op=AluOpType.add)

                for step in TAIL_ORDER:
                    kind, arg = step
                    if kind == "x":
                        xchain(arg)
                    elif kind == "d":
                        xdrain(arg)
                    elif kind == "p":
                        proj2(arg)
                    elif kind == "t":
                        tp2(arg)
                # remaining: G7 g{5,5,6,7}, G2 g{6,6,7,0}, G5 g{7,7,0,1}
                for G in (7, 2, 5):
                    proj2(G)

        if debug_hook:
            debug_hook(nc, dict(QK8s=QK8s, vsb=vsb,
                                P=P, zsum=zsum, zinv=zinv, outTa=outTa,
                                out2a=out2a))

    nc.compile()
    return nc


def get_nc():
    if "nc" not in _CACHE:
        _CACHE["nc"] = build_nc()
    return _CACHE["nc"]


def make_in_maps(x, W_qkv, b_qkv, W_out, b_out):
    x = np.asarray(x, dtype=np.float32)
    W_qkv = np.asarray(W_qkv, dtype=np.float32)
    b_qkv = np.asarray(b_qkv, dtype=np.float32)
    W_out = np.asarray(W_out, dtype=np.float32)
    b_out = np.asarray(b_out, dtype=np.float32)

    operm = (np.arange(O) + O // 2) % O      # rotate qkv channels by 192
    eperm = (np.arange(E) + E // 2) % E      # rotate e-axis by 64

    halves = []
    for h in range(2):
        if h == 0:
            wq, bqv, wo, bov = W_qkv, b_qkv, W_out, b_out
        else:
            wq = W_qkv[operm]
            bqv = b_qkv[operm]
            wo = W_out[:, eperm]
            bov = np.zeros_like(b_out)
        orders = [
            [3 * t + r for t in range(128)]
            + [3 * t + (r + 2) % 3 for t in range(64)]
            + [3 * t + (r + 1) % 3 for t in range(64)]
            for r in range(3)
        ]
        wqv = np.concatenate([wq.T[:, o] for o in orders], axis=1)     # (C, 768)
        brv = np.concatenate([bqv[o][None, :] for o in orders], axis=1)  # (1, 768)
        halves.append({
            "wqkvT": np.ascontiguousarray(wqv).astype(ml_dtypes.bfloat16),
            "brow": np.ascontiguousarray(brv).astype(ml_dtypes.bfloat16),
            "woutT": np.ascontiguousarray(wo.T).astype(ml_dtypes.bfloat16),
            "bout": np.ascontiguousarray(bov.reshape(C, 1)),
        })

    xb = [np.ascontiguousarray(x[n].reshape(C, HW)).astype(ml_dtypes.bfloat16)
          for n in range(N)]
    in_maps = []
    for core in range(8):
        n, h = core // 2, core % 2
        m = {"x": xb[n]}
        m.update(halves[h])
        in_maps.append(m)
    return in_maps


def run(inputs, trace=False, **kw):
    nc = get_nc()
    in_maps = make_in_maps(**inputs)
    res = run_bass_kernel_spmd(nc, in_maps, core_ids=list(range(8)), trace=trace, **kw)
    ys = [np.asarray(res.results[i]["out"], dtype=np.float32) for i in range(8)]
    y = np.stack([ys[2 * n] + ys[2 * n + 1] for n in range(N)])
    return y.reshape(N, C, 64, 64), res


def kernel(**inputs):
    y, _ = run(inputs, trace=False)
    return y


# revision 60
# speedup vs baseline: 1.0954x; 1.0104x over previous
"""Trainium2 Bass kernel for nn_Attention (dense_transformer), v3.

Reference computation (per batch n of 4):
  qkv = W_qkv @ x + b          (384, 4096)   [x flattened to (256, 64*64)]
  raw C-order reinterpret of qkv flat buffer as (4096, 384) -> q|k|v (4096,128)
  scores = q @ k.T / 64        (4096, 4096)
  soft = softmax(scores, axis=-2)             [column softmax]
  out = soft @ v               (4096, 128)
  raw reinterpret of out as (128, 4096)
  y = W_out @ out2 + b_out     (256, 4096)

Sharding: 8 cores = 4 batches x 2 j-halves (t-halves of the permuted j
enumeration; the host-side 192-rotation of qkv channels and 64-rotation of
W_out's e-axis make the SPMD program identical on all cores). Host sums the
per-pair partial y.

v3 dataflow (vs v2): the q/k tensors are quantized to fp8e4 at the stage-1
drain and the score matmuls run in DoubleRow perf mode (2 fp8 rows per PE
pass, 2x throughput): contraction d=128 is split into two 64-partition
groups, with the d-hi half moved onto partitions 0:64 by SBUF->SBUF DMAs
(idle engines). exp runs on ACT with accum_out supplying the column-softmax
Z sums for most chunks; a subset of chunks uses a Schraudolph fast-exp on
DVE (bf16 bits = S*128*log2e/64 + 16249.7 via f32->u16 convert, written
through a bitcast view) plus a DVE row-reduce, to keep both engines busy.
v transposes ride the DMA XBAR instead of the PE. Bias enters PSUM via
ones-row matmuls so all stage-1 drains are plain copies. Out accumulation
rotates the 4 spare PSUM banks in two eras split at jb10: groups 4-7 live
jb0-10 then spill; groups 0-3 chain burst(jb0-10)+live(jb11-15); groups
4-7 finish jb11-15 in the tail on psA's freed banks, adding the spill at
drain. proj2 folds the psi_q permutation into stride-3 rhs gathers.
"""

import numpy as np
import ml_dtypes

import concourse.bass as bass
import concourse.bacc as bacc
import concourse.mybir as mybir
from concourse.bass_utils import run_bass_kernel_spmd
from concourse.tile import TileContext, add_dep_helper
from concourse.masks import make_identity
from concourse.alu_op_type import AluOpType

BF16 = mybir.dt.bfloat16
F32 = mybir.dt.float32
FP8 = mybir.dt.float8e4
U16 = mybir.dt.uint16
AF = mybir.ActivationFunctionType
AX = mybir.AxisListType
DR = mybir.MatmulPerfMode.DoubleRow

N, C, E, O, HW = 4, 256, 128, 384, 4096
JC = HW // 2          # j per core
NJB = JC // 128       # 16 j-blocks
SCALE = 1.0 / 64.0    # 1/sqrt(HW)
SPLIT_JB = 10         # era split for out accumulation
DIRECT_CIDX = 8       # chunks below this use unsplit fp8 scores
WARM_MMS = 16
SCHR_CIDX2 = None
FLUSH_BUDGET = 1
ORDER_D = False
DRAIN_ACT_QUADS = 2   # quads below this drain on ACT instead of DVE
SPILL_ACT = False
XRDY = 2
GATHER_Q = 2
YRDY = 4
TAIL_ORDER = [("t", 0), ("t", 2), ("p", 0), ("p", 3),
              ("x", 0), ("x", 1), ("d", 0), ("x", 2), ("d", 1), ("t", 4),
              ("x", 3), ("p", 6), ("d", 2), ("p", 1), ("d", 3), ("t", 6),
              ("p", 4)]

# Schraudolph fast-exp on DVE for a subset of chunks: bf16 bits of e^x are
# ~ x*(128*log2e) + 16249.7 (HW rounds on f32->u16 convert; rel rms ~1.8%).
SCHR_A = 128.0 * 1.4426950408889634 * SCALE
SCHR_B = 16249.7
# Schraudolph chunk set: spread over the h2 sweep and the h1 slots of the
# jb-minor phase, avoiding stats-critical h3 chunks and DVE-heavy clusters.
SCHR_CIDX = {16, 19, 22, 25, 28, 30, 33, 36, 39, 42, 45, 48, 51, 54, 57,
             60, 63}
SCHR_W = 1024

_CACHE = {}


def _psiq_inv(m):
    if m <= 10:
        return 3 * m
    if m <= 21:
        return 3 * (m - 11) + 1
    return 3 * (m - 22) + 2


def _proj2_runs(G):
    """Maximal stride-3 source-chunk runs feeding y columns [4G*128,(4G+4)*128)."""
    srcs = [_psiq_inv(4 * G + k) for k in range(4)]
    runs = []
    for s in srcs:
        if runs and s == runs[-1][-1] + 3:
            runs[-1].append(s)
        else:
            runs.append([s])
    return runs


def build_nc(debug_hook=None):
    nc = bacc.Bacc("TRN2", target_bir_lowering=False, debug=False, num_devices=8)

    x_ext = nc.dram_tensor("x", [C, HW], BF16, kind="ExternalInput").ap()
    wqkvT_ext = nc.dram_tensor("wqkvT", [C, 768], BF16, kind="ExternalInput").ap()
    brow_ext = nc.dram_tensor("brow", [1, 768], BF16, kind="ExternalInput").ap()
    woutT_ext = nc.dram_tensor("woutT", [E, C], BF16, kind="ExternalInput").ap()
    bout_ext = nc.dram_tensor("bout", [C, 1], F32, kind="ExternalInput").ap()
    y_ext = nc.dram_tensor("out", [C, HW], BF16, kind="ExternalOutput").ap()

    # persistent SBUF
    xsb = [nc.alloc_sbuf_tensor(f"x{cb}", [128, HW], BF16).ap() for cb in range(2)]
    QK8s = nc.alloc_sbuf_tensor("QK8s", [128, 2 * 6144], FP8).ap()
    vsb = nc.alloc_sbuf_tensor("vsb", [128, JC], BF16).ap()
    P = nc.alloc_sbuf_tensor("P", [128, NJB * HW], BF16).ap()
    outTa = nc.alloc_sbuf_tensor("outTa", [128, HW], BF16).ap()
    out2a = nc.alloc_sbuf_tensor("out2a", [128, HW], BF16).ap()
    spill = [nc.alloc_sbuf_tensor(f"spill{g}", [128, 512], F32).ap() for g in range(4)]
    zacc = nc.alloc_sbuf_tensor("zacc", [128, 128], F32).ap()
    zsum = nc.alloc_sbuf_tensor("zsum", [128, 16], F32).ap()
    zinv = nc.alloc_sbuf_tensor("zinv", [128, 16], F32).ap()


    # gathered layout: per g-half, q contiguous (4096) then k (2048)
    QK8sg = QK8s[0:64, :].rearrange("p (g c) -> p g c", g=2)

    def v_sl(jb):
        return vsb[:, jb * 128:(jb + 1) * 128]

    with TileContext(nc) as tc:
        with tc.tile_pool(name="consts", bufs=1) as consts:
            # ---- weights/constants ----
            nc.sync.dma_start(out=xsb[0][:, 0:512], in_=x_ext[0:128, 0:512])
            nc.sync.dma_start(out=xsb[1][:, 0:512], in_=x_ext[128:256, 0:512])
            brow = consts.tile([1, 768], BF16, name="brow", tag="brow")
            nc.sync.dma_start(out=brow, in_=brow_ext[:])
            wq_all = consts.tile([128, 2 * 768], BF16, name="wq_all", tag="wq_all")
            for cb in range(2):
                nc.sync.dma_start(out=wq_all[:, cb * 768:(cb + 1) * 768],
                                  in_=wqkvT_ext[cb * 128:(cb + 1) * 128, :])
            ones1 = consts.tile([1, 128], BF16, name="ones1", tag="ones1")
            nc.vector.memset(ones1[:], 1.0)

            def wq_sl(cb, r):
                return wq_all[:, cb * 768 + r * 256: cb * 768 + (r + 1) * 256]

            misc = consts.tile([128, C + 128], BF16, name="misc", tag="misc")
            woutT = misc[:, 0:C]
            ident = misc[:, C:C + 128]
            nc.gpsimd.dma_start(out=woutT, in_=woutT_ext[:])
            make_identity(nc, ident)
            bo2 = consts.tile([128, 2], F32, name="bo2", tag="bo2")
            bo = [bo2[:, cb:cb + 1] for cb in range(2)]
            for cb in range(2):
                nc.gpsimd.dma_start(out=bo[cb], in_=bout_ext[cb * 128:(cb + 1) * 128, :])
            nc.vector.memset(zacc[:], 0.0)
            # Exp table preload
            scratch = consts.tile([128, 1], F32, name="scratch", tag="scratch")
            nc.vector.memset(scratch[:], 0.0)
            nc.scalar.activation(scratch[:], scratch[:], AF.Exp)

            # ---- PE warmup (p-state ramp) ----
            wsrc = consts.tile([128, 128], BF16, name="wsrc", tag="wsrc")
            nc.vector.memset(wsrc[:], 1.0)
            with tc.tile_pool(name="psW", bufs=1, space="PSUM") as psW:
                wtile = psW.tile([128, 128], F32, tag="warm")
                for _ in range(WARM_MMS):
                    nc.tensor.matmul(wtile[:], wsrc[:], wsrc[:], start=True, stop=True)

            # ---- x loads: all on the HWDGE sync ring, 1024-col chunks
            #      interleaved cb0/cb1 in stage-1 consumption order ----
            for lo, hi in ((512, 1536), (1536, 2560), (2560, 3584), (3584, 4096)):
                nc.sync.dma_start(out=xsb[0][:, lo:hi], in_=x_ext[0:128, lo:hi])
                nc.sync.dma_start(out=xsb[1][:, lo:hi],
                                  in_=x_ext[128:256, lo:hi])

            # ---- phase A chunk order: h-sweeps over jb0-7 (software-
            #      pipelined into stage 1), then jb8-15 h-minor. ----
            order = []
            for h in range(2):
                for jb in range(8):
                    order.append((jb, h))
            if ORDER_D:
                for jb in range(8):
                    order.append((jb, 2))
                    order.append((jb, 3))
            else:
                for h in (2, 3):
                    for jb in range(8):
                        order.append((jb, h))
            for jb in range(8, 16):
                for h in range(4):
                    order.append((jb, h))

            psA_cm = tc.tile_pool(name="psA", bufs=2, space="PSUM")
            psA = psA_cm.__enter__()
            stg_cm = tc.tile_pool(name="stg", bufs=1)
            stg = stg_cm.__enter__()
            Qtmp = stg.tile([128, 32 * 128], FP8, name="Qtmp", tag="Qtmp")
            Ktmp = stg.tile([128, 32 * 64], FP8, name="Ktmp", tag="Ktmp")
            Vf = stg.tile([128, 32 * 64], BF16, name="Vf", tag="Vf")
            Vv = Vf.rearrange("p (b c) -> p b c", c=64)
            holder = {}
            bankX = [None] * 4   # i'-groups 4-7: live jb0..SPLIT_JB
            bankY = [None] * 4   # i'-groups 0-3: burst+live, stop jb15
            pe_q = []   # (ready_chunk_idx, emit_fn): deferred PE MMs

            def flush(cidx, budget=4):
                n = 0
                while pe_q and pe_q[0][0] <= cidx and n < budget:
                    pe_q.pop(0)[1]()
                    n += 1

            def stats(jb):
                nc.vector.reduce_sum(
                    out=zsum[:, jb:jb + 1], in_=zacc[:, jb * 8:(jb + 1) * 8],
                    axis=AX.X)
                nc.vector.reciprocal(zinv[:, jb:jb + 1], zsum[:, jb:jb + 1])
                nc.vector.tensor_scalar_mul(v_sl(jb), v_sl(jb),
                                            zinv[:, jb:jb + 1])

            def out_mm(bank, g, jb, start, stop):
                nc.tensor.matmul(
                    bank[:], v_sl(jb),
                    P[:, jb * HW + g * 512: jb * HW + (g + 1) * 512],
                    start=start, stop=stop,
                )

            def emit_chunk(cidx):
                jb, h = order[cidx]
                pa = psA.tile([128, 1024], F32, name="pa", tag="pa")
                for n2 in range(2):
                    if cidx < 16:
                        # unsplit fp8 (128-partition contraction, one 64-wide
                        # k block per MM, stacked via tile_position):
                        # independent of the gather DMAs
                        for t in range(2):
                            nc.tensor.matmul(
                                pa[64 * t:64 * t + 64,
                                   n2 * 512:(n2 + 1) * 512],
                                Ktmp[:, (2 * jb + t) * 64:
                                     (2 * jb + t + 1) * 64],
                                Qtmp[:, (8 * h + 4 * n2) * 128:
                                     (8 * h + 4 * n2 + 4) * 128],
                                start=True, stop=True,
                                tile_position=(0, 64 * t),
                            )
                    else:
                        o = (8 * h + 4 * n2) * 128
                        nc.tensor.matmul(
                            pa[:, n2 * 512:(n2 + 1) * 512],
                            QK8sg[:, :, 4096 + jb * 128:4096 + (jb + 1) * 128],
                            QK8sg[:, :, o:o + 512],
                            start=True, stop=True,
                            perf_mode=DR,
                        )
                psl = P[:, jb * HW + h * 1024: jb * HW + (h + 1) * 1024]
                zc0 = zacc[:, jb * 8 + 2 * h: jb * 8 + 2 * h + 1]
                zc1 = zacc[:, jb * 8 + 2 * h + 1: jb * 8 + 2 * h + 2]
                # Z per chunk via ACT accum_out (free row-sums). Split
                # chunks run half on DVE (Schraudolph fast-exp + reduce)
                # and half on ACT concurrently to balance the engines.
                if cidx in SCHR_CIDX:
                    w = SCHR_W
                    nc.vector.tensor_scalar(
                        out=psl[:, 0:w].bitcast(U16), in0=pa[:, 0:w],
                        scalar1=SCHR_A, scalar2=SCHR_B,
                        op0=AluOpType.mult, op1=AluOpType.add)
                    nc.vector.reduce_sum(out=zc0, in_=psl[:, 0:w],
                                         axis=AX.X)
                    if w < 1024:
                        nc.scalar.activation(out=psl[:, w:1024],
                                             in_=pa[:, w:1024], func=AF.Exp,
                                             scale=SCALE, accum_out=zc1)
                else:
                    nc.scalar.activation(out=psl, in_=pa[:], func=AF.Exp,
                                         scale=SCALE, accum_out=zc0)
                flush(cidx, budget=FLUSH_BUDGET)
                if h == 3:
                    stats(jb)
                    psBi = holder["psBi"]
                    if jb == 0:
                        for g in range(4):
                            bankX[g] = psBi.tile([128, 512], F32,
                                                 name=f"bk{g}", tag=f"bk{g}")
                    if jb <= SPLIT_JB:
                        for g in range(4):
                            pe_q.append((cidx + XRDY,
                                         (lambda g=g, jb=jb: out_mm(
                                             bankX[g], g + 4, jb,
                                             jb == 0, jb == SPLIT_JB))))
                    else:
                        for g in range(4):
                            pe_q.append((cidx + YRDY,
                                         (lambda g=g, jb=jb: out_mm(
                                             bankY[g], g, jb,
                                             False, jb == 15))))
                    if jb == SPLIT_JB:
                        # spill X banks; queue Y bursts (groups 0-3)
                        # jb0..SPLIT_JB from persistent P.
                        def spill_and_y():
                            for g in range(4):
                                if SPILL_ACT and g % 2 == 0:
                                    nc.scalar.copy(spill[g][:], bankX[g][:])
                                else:
                                    nc.vector.tensor_copy(spill[g][:],
                                                          bankX[g][:])
                            for g in range(4):
                                bankY[g] = psBi.tile([128, 512], F32,
                                                     name=f"bk{g}",
                                                     tag=f"bk{g}")
                        pe_q.append((cidx + 2, spill_and_y))
                        for jbq in range(SPLIT_JB + 1):
                            for g in range(4):
                                pe_q.append((cidx + 2 + jbq // 2,
                                             (lambda g=g, jbq=jbq: out_mm(
                                                 bankY[g], g, jbq,
                                                 jbq == 0, False))))

            # chunks emitted between stage-1 quads (deps: k needs its jb's
            # quads, q needs quads 2h,2h+1; all cidx<16 are gather-free)
            INTER = {1: [0, 1, 2, 3], 2: [4, 5], 3: [6, 7, 8, 9, 10, 11],
                     4: [12, 13], 5: [14, 15]}

            # ---- stage 1: quads of FT blocks -> Qtmp/Ktmp (fp8) + Vf
            #      (bf16), gather DMAs -> QK8s, XBAR -> vsb; early phase-A
            #      chunks interleave (psF on banks 4-7, psA on 0-3). ----
            with tc.tile_pool(name="psF", bufs=2, space="PSUM") as psF:
                for q in range(8):
                    pf = psF.tile([128, 1024], F32, name="pf", tag="pf")
                    pf3 = pf.rearrange("p (s c) -> p s c", s=4)
                    for s in range(4):
                        sc = 4 * q + s
                        r = sc % 3
                        nc.tensor.matmul(pf3[:, s, 0:256], ones1[:],
                                         brow[:, r * 256:(r + 1) * 256],
                                         start=True, stop=False)
                        for cb in range(2):
                            nc.tensor.matmul(
                                pf3[:, s, 0:256],
                                xsb[cb][:, sc * 128:(sc + 1) * 128],
                                wq_sl(cb, r),
                                start=False, stop=(cb == 1),
                            )
                    cp = (nc.scalar.copy if q < DRAIN_ACT_QUADS
                          else nc.vector.tensor_copy)
                    cp(Qtmp[:, q * 512:(q + 1) * 512], pf3[:, :, 0:128])
                    cp(Ktmp[:, q * 256:(q + 1) * 256], pf3[:, :, 128:192])
                    # v tokens for k-block sc live in v-block sc+1: store the
                    # v drain one block down (with wrap) so vsb aligns with kT.
                    if q == 0:
                        cp(Vv[:, 31:32, :], pf3[:, 0:1, 192:256])
                        cp(Vv[:, 0:3, :], pf3[:, 1:4, 192:256])
                    else:
                        cp(Vv[:, 4 * q - 1:4 * q + 3, :], pf3[:, :, 192:256])
                    # gather DMAs per 16-block round: contiguous q/k copies
                    # of each d-half (g=1 is the partition-shifted 64:128
                    # half) into the per-g layout [q 4096 | k 2048].
                    if q % GATHER_Q == GATHER_Q - 1:
                        bg = q // GATHER_Q
                        qw, kw = GATHER_Q * 512, GATHER_Q * 256
                        for g in range(2):
                            nc.sync.dma_start(
                                out=QK8s[0:64, g * 6144 + bg * qw:
                                         g * 6144 + (bg + 1) * qw],
                                in_=Qtmp[64 * g:64 * g + 64,
                                         bg * qw:(bg + 1) * qw])
                            nc.scalar.dma_start(
                                out=QK8s[0:64, g * 6144 + 4096 + bg * kw:
                                         g * 6144 + 4096 + (bg + 1) * kw],
                                in_=Ktmp[64 * g:64 * g + 64,
                                         bg * kw:(bg + 1) * kw])
                    # XBAR transposes: vsb[:, jb, :] = Vf[:, jb, :].T, in
                    # two halves so jb0-7 stats don't wait on all drains
                    if q == 4:
                        nc.scalar.dma_start_transpose(
                            out=vsb[:, 0:1024].rearrange("p (b c) -> p b c",
                                                         c=128),
                            in_=Vf[:, 0:1024])
                    if q == 7:
                        nc.scalar.dma_start_transpose(
                            out=vsb[:, 1024:2048].rearrange(
                                "p (b c) -> p b c", c=128),
                            in_=Vf[:, 1024:2048])
                    for c in INTER.get(q, ()):
                        emit_chunk(c)

            stg_cm.__exit__(None, None, None)

            # ---- phase A remainder ----
            with tc.tile_pool(name="psBi", bufs=1, space="PSUM") as psBi:
                holder["psBi"] = psBi
                for cidx in range(16, 64):
                    emit_chunk(cidx)
                while pe_q:
                    pe_q.pop(0)[1]()

                # Y (groups 0-3) drain to outTa.
                for g in range(4):
                    if g % 2 == 0:
                        nc.scalar.copy(outTa[:, g * 512:(g + 1) * 512],
                                       bankY[g][:])
                    else:
                        nc.vector.tensor_copy(
                            outTa[:, g * 512:(g + 1) * 512], bankY[g][:])
            psA_cm.__exit__(None, None, None)
            out2a3 = out2a.rearrange("p (b t) -> p b t", t=128)
            with tc.tile_pool(name="psA2", bufs=1, space="PSUM") as psA2, \
                 tc.tile_pool(name="psC", bufs=2, space="PSUM") as psC, \
                 tc.tile_pool(name="psY", bufs=2, space="PSUM") as psY, \
                 tc.tile_pool(name="late", bufs=2) as late:

                # proj2 Gs complete in pairs; each pair's two y-slices per
                # cb ride one strided DMA to halve the descriptor train.
                PAIRS = [(0, 3), (6, 1), (4, 7), (2, 5)]
                pair_of = {g: (pi, hi) for pi, p in enumerate(PAIRS)
                           for hi, g in enumerate(p)}
                ygt = {}

                pyt = {}

                def proj2(G):
                    runs = _proj2_runs(G)
                    pi, hi = pair_of[G]
                    for cb in range(2):
                        if hi == 0:
                            pyt[(pi, cb)] = psY.tile(
                                [128, 1024], F32, name="py", tag="py")
                        lo = min(PAIRS[pi])
                        hc = PAIRS[pi][hi] != lo
                        py = pyt[(pi, cb)][:, 512:1024] if hc else \
                            pyt[(pi, cb)][:, 0:512]
                        off = 0
                        for ri, run in enumerate(runs):
                            w = 128 * len(run)
                            rhs = out2a3[:, run[0]:run[-1] + 1:3, :]
                            nc.tensor.matmul(
                                py[:, off:off + w],
                                woutT[:, cb * 128:(cb + 1) * 128], rhs,
                                start=(ri == 0), stop=(ri == len(runs) - 1),
                                skip_group_check=True,
                            )
                            off += w
                        if hi == 1:
                            # one pair-wide bias + one strided DMA per cb
                            key = (pi % 2, cb)
                            yg = late.tile([128, 1024], BF16,
                                           name=f"yg{key}", tag=f"yg{key}")
                            if cb == 0:
                                nc.scalar.activation(yg[:], pyt[(pi, cb)][:],
                                                     AF.Identity, bias=bo[cb])
                            else:
                                nc.vector.tensor_scalar_add(
                                    yg[:], pyt[(pi, cb)][:], bo[cb])
                            d = (max(PAIRS[pi]) - lo) * 512
                            outap = y_ext[cb * 128:(cb + 1) * 128,
                                          lo * 512:].rearrange(
                                "p (a c) -> p a c", c=512)[:, 0:d // 512 + 1:
                                                           d // 512, :]
                            eng = [nc.sync, nc.scalar, nc.gpsimd][
                                (pi * 2 + cb) % 3]
                            eng.dma_start(out=outap, in_=yg[:])

                def tp2(g0):
                    # transpose a pair of 512-col groups; one wide drain
                    tpc = psC.tile([128, 1024], BF16, name="tpc", tag="tpc")
                    for s in range(8):
                        nc.tensor.transpose(
                            tpc[:, s * 128:(s + 1) * 128],
                            outTa[:, g0 * 512 + s * 128:
                                  g0 * 512 + (s + 1) * 128],
                            ident)
                    if g0 % 4 == 0:
                        nc.scalar.copy(
                            out2a[:, g0 * 512:(g0 + 2) * 512], tpc[:])
                    else:
                        nc.vector.tensor_scalar_add(
                            out2a[:, g0 * 512:(g0 + 2) * 512], tpc[:], 0.0)

                # Y groups (0-3) completed at phase-A end; X tails chain
                # jb SPLIT_JB+1..15 on psA's freed banks, draining with the
                # spill added; TAIL_ORDER interleaves everything.
                bA2 = [None] * 4

                def xchain(g):
                    bA2[g] = psA2.tile([128, 512], F32, name=f"bA2{g % 2}",
                                       tag=f"bA2{g % 2}")
                    for jb in range(SPLIT_JB + 1, 16):
                        out_mm(bA2[g], g + 4, jb, jb == SPLIT_JB + 1, jb == 15)

                def xdrain(g):
                    nc.vector.tensor_tensor(
                        out=outTa[:, (g + 4) * 512:(g + 5) * 512],
                        in0=bA2[g][:], in1=spill[g][:], op=AluOpType.add)

                for step in TAIL_ORDER:
                    kind, arg = step
                    if kind == "x":
                        xchain(arg)
                    elif kind == "d":
                        xdrain(arg)
                    elif kind == "p":
                        proj2(arg)
                    elif kind == "t":
                        tp2(arg)
                # remaining: G7 g{5,5,6,7}, G2 g{6,6,7,0}, G5 g{7,7,0,1}
                for G in (7, 2, 5):
                    proj2(G)

        if debug_hook:
            debug_hook(nc, dict(QK8s=QK8s, vsb=vsb,
                                P=P, zsum=zsum, zinv=zinv, outTa=outTa,
                                out2a=out2a))

    nc.compile()
    return nc


def get_nc():
    if "nc" not in _CACHE:
        _CACHE["nc"] = build_nc()
    return _CACHE["nc"]


def make_in_maps(x, W_qkv, b_qkv, W_out, b_out):
    x = np.asarray(x, dtype=np.float32)
    W_qkv = np.asarray(W_qkv, dtype=np.float32)
    b_qkv = np.asarray(b_qkv, dtype=np.float32)
    W_out = np.asarray(W_out, dtype=np.float32)
    b_out = np.asarray(b_out, dtype=np.float32)

    operm = (np.arange(O) + O // 2) % O      # rotate qkv channels by 192
    eperm = (np.arange(E) + E // 2) % E      # rotate e-axis by 64

    halves = []
    for h in range(2):
        if h == 0:
            wq, bqv, wo, bov = W_qkv, b_qkv, W_out, b_out
        else:
            wq = W_qkv[operm]
            bqv = b_qkv[operm]
            wo = W_out[:, eperm]
            bov = np.zeros_like(b_out)
        orders = [
            [3 * t + r for t in range(128)]
            + [3 * t + (r + 2) % 3 for t in range(64)]
            + [3 * t + (r + 1) % 3 for t in range(64)]
            for r in range(3)
        ]
        wqv = np.concatenate([wq.T[:, o] for o in orders], axis=1)     # (C, 768)
        brv = np.concatenate([bqv[o][None, :] for o in orders], axis=1)  # (1, 768)
        halves.append({
            "wqkvT": np.ascontiguousarray(wqv).astype(ml_dtypes.bfloat16),
            "brow": np.ascontiguousarray(brv).astype(ml_dtypes.bfloat16),
            "woutT": np.ascontiguousarray(wo.T).astype(ml_dtypes.bfloat16),
            "bout": np.ascontiguousarray(bov.reshape(C, 1)),
        })

    xb = [np.ascontiguousarray(x[n].reshape(C, HW)).astype(ml_dtypes.bfloat16)
          for n in range(N)]
    in_maps = []
    for core in range(8):
        n, h = core // 2, core % 2
        m = {"x": xb[n]}
        m.update(halves[h])
        in_maps.append(m)
    return in_maps


def run(inputs, trace=False, **kw):
    nc = get_nc()
    in_maps = make_in_maps(**inputs)
    res = run_bass_kernel_spmd(nc, in_maps, core_ids=list(range(8)), trace=trace, **kw)
    ys = [np.asarray(res.results[i]["out"], dtype=np.float32) for i in range(8)]
    y = np.stack([ys[2 * n] + ys[2 * n + 1] for n in range(N)])
    return y.reshape(N, C, 64, 64), res


def kernel(**inputs):
    y, _ = run(inputs, trace=False)
    return y


# revision 61
# speedup vs baseline: 1.0960x; 1.0006x over previous
"""Trainium2 Bass kernel for nn_Attention (dense_transformer), v3.

Reference computation (per batch n of 4):
  qkv = W_qkv @ x + b          (384, 4096)   [x flattened to (256, 64*64)]
  raw C-order reinterpret of qkv flat buffer as (4096, 384) -> q|k|v (4096,128)
  scores = q @ k.T / 64        (4096, 4096)
  soft = softmax(scores, axis=-2)             [column softmax]
  out = soft @ v               (4096, 128)
  raw reinterpret of out as (128, 4096)
  y = W_out @ out2 + b_out     (256, 4096)

Sharding: 8 cores = 4 batches x 2 j-halves (t-halves of the permuted j
enumeration; the host-side 192-rotation of qkv channels and 64-rotation of
W_out's e-axis make the SPMD program identical on all cores). Host sums the
per-pair partial y.

v3 dataflow (vs v2): the q/k tensors are quantized to fp8e4 at the stage-1
drain and the score matmuls run in DoubleRow perf mode (2 fp8 rows per PE
pass, 2x throughput): contraction d=128 is split into two 64-partition
groups, with the d-hi half moved onto partitions 0:64 by SBUF->SBUF DMAs
(idle engines). exp runs on ACT with accum_out supplying the column-softmax
Z sums for most chunks; a subset of chunks uses a Schraudolph fast-exp on
DVE (bf16 bits = S*128*log2e/64 + 16249.7 via f32->u16 convert, written
through a bitcast view) plus a DVE row-reduce, to keep both engines busy.
v transposes ride the DMA XBAR instead of the PE. Bias enters PSUM via
ones-row matmuls so all stage-1 drains are plain copies. Out accumulation
rotates the 4 spare PSUM banks in two eras split at jb10: groups 4-7 live
jb0-10 then spill; groups 0-3 chain burst(jb0-10)+live(jb11-15); groups
4-7 finish jb11-15 in the tail on psA's freed banks, adding the spill at
drain. proj2 folds the psi_q permutation into stride-3 rhs gathers.
"""

import numpy as np
import ml_dtypes

import concourse.bass as bass
import concourse.bacc as bacc
import concourse.mybir as mybir
from concourse.bass_utils import run_bass_kernel_spmd
from concourse.tile import TileContext, add_dep_helper
from concourse.masks import make_identity
from concourse.alu_op_type import AluOpType

BF16 = mybir.dt.bfloat16
F32 = mybir.dt.float32
FP8 = mybir.dt.float8e4
U16 = mybir.dt.uint16
AF = mybir.ActivationFunctionType
AX = mybir.AxisListType
DR = mybir.MatmulPerfMode.DoubleRow

N, C, E, O, HW = 4, 256, 128, 384, 4096
JC = HW // 2          # j per core
NJB = JC // 128       # 16 j-blocks
SCALE = 1.0 / 64.0    # 1/sqrt(HW)
SPLIT_JB = 10         # era split for out accumulation
DIRECT_CIDX = 8       # chunks below this use unsplit fp8 scores
WARM_MMS = 16
SCHR_CIDX2 = None
FLUSH_BUDGET = 1
ORDER_D = False
DRAIN_ACT_QUADS = 2   # quads below this drain on ACT instead of DVE
SPILL_ACT = False
XRDY = 2
GATHER_Q = 2
YRDY = 4
TAIL_ORDER = [("t", 0), ("t", 2), ("p", 0), ("p", 3),
              ("x", 0), ("x", 1), ("d", 0), ("x", 2), ("d", 1), ("t", 4),
              ("x", 3), ("p", 6), ("d", 2), ("p", 1), ("d", 3), ("t", 6),
              ("p", 4)]

# Schraudolph fast-exp on DVE for a subset of chunks: bf16 bits of e^x are
# ~ x*(128*log2e) + 16249.7 (HW rounds on f32->u16 convert; rel rms ~1.8%).
SCHR_A = 128.0 * 1.4426950408889634 * SCALE
SCHR_B = 16249.7
# Schraudolph chunk set: spread over the h2 sweep and the h1 slots of the
# jb-minor phase, avoiding stats-critical h3 chunks and DVE-heavy clusters.
SCHR_CIDX = {14, 16, 19, 22, 25, 28, 30, 33, 36, 39, 42, 45, 48, 51, 54,
             57, 60, 63}
SCHR_W = 1024

_CACHE = {}


def _psiq_inv(m):
    if m <= 10:
        return 3 * m
    if m <= 21:
        return 3 * (m - 11) + 1
    return 3 * (m - 22) + 2


def _proj2_runs(G):
    """Maximal stride-3 source-chunk runs feeding y columns [4G*128,(4G+4)*128)."""
    srcs = [_psiq_inv(4 * G + k) for k in range(4)]
    runs = []
    for s in srcs:
        if runs and s == runs[-1][-1] + 3:
            runs[-1].append(s)
        else:
            runs.append([s])
    return runs


def build_nc(debug_hook=None):
    nc = bacc.Bacc("TRN2", target_bir_lowering=False, debug=False, num_devices=8)

    x_ext = nc.dram_tensor("x", [C, HW], BF16, kind="ExternalInput").ap()
    wqkvT_ext = nc.dram_tensor("wqkvT", [C, 768], BF16, kind="ExternalInput").ap()
    brow_ext = nc.dram_tensor("brow", [1, 768], BF16, kind="ExternalInput").ap()
    woutT_ext = nc.dram_tensor("woutT", [E, C], BF16, kind="ExternalInput").ap()
    bout_ext = nc.dram_tensor("bout", [C, 1], F32, kind="ExternalInput").ap()
    y_ext = nc.dram_tensor("out", [C, HW], BF16, kind="ExternalOutput").ap()

    # persistent SBUF
    xsb = [nc.alloc_sbuf_tensor(f"x{cb}", [128, HW], BF16).ap() for cb in range(2)]
    QK8s = nc.alloc_sbuf_tensor("QK8s", [128, 2 * 6144], FP8).ap()
    vsb = nc.alloc_sbuf_tensor("vsb", [128, JC], BF16).ap()
    P = nc.alloc_sbuf_tensor("P", [128, NJB * HW], BF16).ap()
    outTa = nc.alloc_sbuf_tensor("outTa", [128, HW], BF16).ap()
    out2a = nc.alloc_sbuf_tensor("out2a", [128, HW], BF16).ap()
    spill = [nc.alloc_sbuf_tensor(f"spill{g}", [128, 512], F32).ap() for g in range(4)]
    zacc = nc.alloc_sbuf_tensor("zacc", [128, 128], F32).ap()
    zsum = nc.alloc_sbuf_tensor("zsum", [128, 16], F32).ap()
    zinv = nc.alloc_sbuf_tensor("zinv", [128, 16], F32).ap()


    # gathered layout: per g-half, q contiguous (4096) then k (2048)
    QK8sg = QK8s[0:64, :].rearrange("p (g c) -> p g c", g=2)

    def v_sl(jb):
        return vsb[:, jb * 128:(jb + 1) * 128]

    with TileContext(nc) as tc:
        with tc.tile_pool(name="consts", bufs=1) as consts:
            # ---- weights/constants ----
            nc.sync.dma_start(out=xsb[0][:, 0:512], in_=x_ext[0:128, 0:512])
            nc.sync.dma_start(out=xsb[1][:, 0:512], in_=x_ext[128:256, 0:512])
            brow = consts.tile([1, 768], BF16, name="brow", tag="brow")
            nc.sync.dma_start(out=brow, in_=brow_ext[:])
            wq_all = consts.tile([128, 2 * 768], BF16, name="wq_all", tag="wq_all")
            for cb in range(2):
                nc.sync.dma_start(out=wq_all[:, cb * 768:(cb + 1) * 768],
                                  in_=wqkvT_ext[cb * 128:(cb + 1) * 128, :])
            ones1 = consts.tile([1, 128], BF16, name="ones1", tag="ones1")
            nc.vector.memset(ones1[:], 1.0)

            def wq_sl(cb, r):
                return wq_all[:, cb * 768 + r * 256: cb * 768 + (r + 1) * 256]

            misc = consts.tile([128, C + 128], BF16, name="misc", tag="misc")
            woutT = misc[:, 0:C]
            ident = misc[:, C:C + 128]
            nc.gpsimd.dma_start(out=woutT, in_=woutT_ext[:])
            make_identity(nc, ident)
            bo2 = consts.tile([128, 2], F32, name="bo2", tag="bo2")
            bo = [bo2[:, cb:cb + 1] for cb in range(2)]
            for cb in range(2):
                nc.gpsimd.dma_start(out=bo[cb], in_=bout_ext[cb * 128:(cb + 1) * 128, :])
            nc.vector.memset(zacc[:], 0.0)
            # Exp table preload
            scratch = consts.tile([128, 1], F32, name="scratch", tag="scratch")
            nc.vector.memset(scratch[:], 0.0)
            nc.scalar.activation(scratch[:], scratch[:], AF.Exp)

            # ---- PE warmup (p-state ramp) ----
            wsrc = consts.tile([128, 128], BF16, name="wsrc", tag="wsrc")
            nc.vector.memset(wsrc[:], 1.0)
            with tc.tile_pool(name="psW", bufs=1, space="PSUM") as psW:
                wtile = psW.tile([128, 128], F32, tag="warm")
                for _ in range(WARM_MMS):
                    nc.tensor.matmul(wtile[:], wsrc[:], wsrc[:], start=True, stop=True)

            # ---- x loads: all on the HWDGE sync ring, 1024-col chunks
            #      interleaved cb0/cb1 in stage-1 consumption order ----
            for lo, hi in ((512, 1536), (1536, 2560), (2560, 3584), (3584, 4096)):
                nc.sync.dma_start(out=xsb[0][:, lo:hi], in_=x_ext[0:128, lo:hi])
                nc.sync.dma_start(out=xsb[1][:, lo:hi],
                                  in_=x_ext[128:256, lo:hi])

            # ---- phase A chunk order: h-sweeps over jb0-7 (software-
            #      pipelined into stage 1), then jb8-15 h-minor. ----
            order = []
            for h in range(2):
                for jb in range(8):
                    order.append((jb, h))
            if ORDER_D:
                for jb in range(8):
                    order.append((jb, 2))
                    order.append((jb, 3))
            else:
                for h in (2, 3):
                    for jb in range(8):
                        order.append((jb, h))
            for jb in range(8, 16):
                for h in range(4):
                    order.append((jb, h))

            psA_cm = tc.tile_pool(name="psA", bufs=2, space="PSUM")
            psA = psA_cm.__enter__()
            stg_cm = tc.tile_pool(name="stg", bufs=1)
            stg = stg_cm.__enter__()
            Qtmp = stg.tile([128, 32 * 128], FP8, name="Qtmp", tag="Qtmp")
            Ktmp = stg.tile([128, 32 * 64], FP8, name="Ktmp", tag="Ktmp")
            Vf = stg.tile([128, 32 * 64], BF16, name="Vf", tag="Vf")
            Vv = Vf.rearrange("p (b c) -> p b c", c=64)
            holder = {}
            bankX = [None] * 4   # i'-groups 4-7: live jb0..SPLIT_JB
            bankY = [None] * 4   # i'-groups 0-3: burst+live, stop jb15
            pe_q = []   # (ready_chunk_idx, emit_fn): deferred PE MMs

            def flush(cidx, budget=4):
                n = 0
                while pe_q and pe_q[0][0] <= cidx and n < budget:
                    pe_q.pop(0)[1]()
                    n += 1

            def stats(jb):
                nc.vector.reduce_sum(
                    out=zsum[:, jb:jb + 1], in_=zacc[:, jb * 8:(jb + 1) * 8],
                    axis=AX.X)
                nc.vector.reciprocal(zinv[:, jb:jb + 1], zsum[:, jb:jb + 1])
                nc.vector.tensor_scalar_mul(v_sl(jb), v_sl(jb),
                                            zinv[:, jb:jb + 1])

            def out_mm(bank, g, jb, start, stop):
                nc.tensor.matmul(
                    bank[:], v_sl(jb),
                    P[:, jb * HW + g * 512: jb * HW + (g + 1) * 512],
                    start=start, stop=stop,
                )

            def emit_chunk(cidx):
                jb, h = order[cidx]
                pa = psA.tile([128, 1024], F32, name="pa", tag="pa")
                for n2 in range(2):
                    if cidx < 16:
                        # unsplit fp8 (128-partition contraction, one 64-wide
                        # k block per MM, stacked via tile_position):
                        # independent of the gather DMAs
                        for t in range(2):
                            nc.tensor.matmul(
                                pa[64 * t:64 * t + 64,
                                   n2 * 512:(n2 + 1) * 512],
                                Ktmp[:, (2 * jb + t) * 64:
                                     (2 * jb + t + 1) * 64],
                                Qtmp[:, (8 * h + 4 * n2) * 128:
                                     (8 * h + 4 * n2 + 4) * 128],
                                start=True, stop=True,
                                tile_position=(0, 64 * t),
                            )
                    else:
                        o = (8 * h + 4 * n2) * 128
                        nc.tensor.matmul(
                            pa[:, n2 * 512:(n2 + 1) * 512],
                            QK8sg[:, :, 4096 + jb * 128:4096 + (jb + 1) * 128],
                            QK8sg[:, :, o:o + 512],
                            start=True, stop=True,
                            perf_mode=DR,
                        )
                psl = P[:, jb * HW + h * 1024: jb * HW + (h + 1) * 1024]
                zc0 = zacc[:, jb * 8 + 2 * h: jb * 8 + 2 * h + 1]
                zc1 = zacc[:, jb * 8 + 2 * h + 1: jb * 8 + 2 * h + 2]
                # Z per chunk via ACT accum_out (free row-sums). Split
                # chunks run half on DVE (Schraudolph fast-exp + reduce)
                # and half on ACT concurrently to balance the engines.
                if cidx in SCHR_CIDX:
                    w = SCHR_W
                    nc.vector.tensor_scalar(
                        out=psl[:, 0:w].bitcast(U16), in0=pa[:, 0:w],
                        scalar1=SCHR_A, scalar2=SCHR_B,
                        op0=AluOpType.mult, op1=AluOpType.add)
                    nc.vector.reduce_sum(out=zc0, in_=psl[:, 0:w],
                                         axis=AX.X)
                    if w < 1024:
                        nc.scalar.activation(out=psl[:, w:1024],
                                             in_=pa[:, w:1024], func=AF.Exp,
                                             scale=SCALE, accum_out=zc1)
                else:
                    nc.scalar.activation(out=psl, in_=pa[:], func=AF.Exp,
                                         scale=SCALE, accum_out=zc0)
                flush(cidx, budget=FLUSH_BUDGET)
                if h == 3:
                    stats(jb)
                    psBi = holder["psBi"]
                    if jb == 0:
                        for g in range(4):
                            bankX[g] = psBi.tile([128, 512], F32,
                                                 name=f"bk{g}", tag=f"bk{g}")
                    if jb <= SPLIT_JB:
                        for g in range(4):
                            pe_q.append((cidx + XRDY,
                                         (lambda g=g, jb=jb: out_mm(
                                             bankX[g], g + 4, jb,
                                             jb == 0, jb == SPLIT_JB))))
                    else:
                        for g in range(4):
                            pe_q.append((cidx + YRDY,
                                         (lambda g=g, jb=jb: out_mm(
                                             bankY[g], g, jb,
                                             False, jb == 15))))
                    if jb == SPLIT_JB:
                        # spill X banks; queue Y bursts (groups 0-3)
                        # jb0..SPLIT_JB from persistent P.
                        def spill_and_y():
                            for g in range(4):
                                if SPILL_ACT and g % 2 == 0:
                                    nc.scalar.copy(spill[g][:], bankX[g][:])
                                else:
                                    nc.vector.tensor_copy(spill[g][:],
                                                          bankX[g][:])
                            for g in range(4):
                                bankY[g] = psBi.tile([128, 512], F32,
                                                     name=f"bk{g}",
                                                     tag=f"bk{g}")
                        pe_q.append((cidx + 2, spill_and_y))
                        for jbq in range(SPLIT_JB + 1):
                            for g in range(4):
                                pe_q.append((cidx + 2 + jbq // 2,
                                             (lambda g=g, jbq=jbq: out_mm(
                                                 bankY[g], g, jbq,
                                                 jbq == 0, False))))

            # chunks emitted between stage-1 quads (deps: k needs its jb's
            # quads, q needs quads 2h,2h+1; all cidx<16 are gather-free)
            INTER = {1: [0, 1, 2, 3], 2: [4, 5], 3: [6, 7, 8, 9, 10, 11],
                     4: [12, 13], 5: [14, 15]}

            # ---- stage 1: quads of FT blocks -> Qtmp/Ktmp (fp8) + Vf
            #      (bf16), gather DMAs -> QK8s, XBAR -> vsb; early phase-A
            #      chunks interleave (psF on banks 4-7, psA on 0-3). ----
            with tc.tile_pool(name="psF", bufs=2, space="PSUM") as psF:
                for q in range(8):
                    pf = psF.tile([128, 1024], F32, name="pf", tag="pf")
                    pf3 = pf.rearrange("p (s c) -> p s c", s=4)
                    for s in range(4):
                        sc = 4 * q + s
                        r = sc % 3
                        nc.tensor.matmul(pf3[:, s, 0:256], ones1[:],
                                         brow[:, r * 256:(r + 1) * 256],
                                         start=True, stop=False)
                        for cb in range(2):
                            nc.tensor.matmul(
                                pf3[:, s, 0:256],
                                xsb[cb][:, sc * 128:(sc + 1) * 128],
                                wq_sl(cb, r),
                                start=False, stop=(cb == 1),
                            )
                    cp = (nc.scalar.copy if q < DRAIN_ACT_QUADS
                          else nc.vector.tensor_copy)
                    cp(Qtmp[:, q * 512:(q + 1) * 512], pf3[:, :, 0:128])
                    cp(Ktmp[:, q * 256:(q + 1) * 256], pf3[:, :, 128:192])
                    # v tokens for k-block sc live in v-block sc+1: store the
                    # v drain one block down (with wrap) so vsb aligns with kT.
                    if q == 0:
                        cp(Vv[:, 31:32, :], pf3[:, 0:1, 192:256])
                        cp(Vv[:, 0:3, :], pf3[:, 1:4, 192:256])
                    else:
                        cp(Vv[:, 4 * q - 1:4 * q + 3, :], pf3[:, :, 192:256])
                    # gather DMAs per 16-block round: contiguous q/k copies
                    # of each d-half (g=1 is the partition-shifted 64:128
                    # half) into the per-g layout [q 4096 | k 2048].
                    if q % GATHER_Q == GATHER_Q - 1:
                        bg = q // GATHER_Q
                        qw, kw = GATHER_Q * 512, GATHER_Q * 256
                        for g in range(2):
                            nc.sync.dma_start(
                                out=QK8s[0:64, g * 6144 + bg * qw:
                                         g * 6144 + (bg + 1) * qw],
                                in_=Qtmp[64 * g:64 * g + 64,
                                         bg * qw:(bg + 1) * qw])
                            nc.scalar.dma_start(
                                out=QK8s[0:64, g * 6144 + 4096 + bg * kw:
                                         g * 6144 + 4096 + (bg + 1) * kw],
                                in_=Ktmp[64 * g:64 * g + 64,
                                         bg * kw:(bg + 1) * kw])
                    # XBAR transposes: vsb[:, jb, :] = Vf[:, jb, :].T, in
                    # two halves so jb0-7 stats don't wait on all drains
                    if q == 4:
                        nc.scalar.dma_start_transpose(
                            out=vsb[:, 0:1024].rearrange("p (b c) -> p b c",
                                                         c=128),
                            in_=Vf[:, 0:1024])
                    if q == 7:
                        nc.scalar.dma_start_transpose(
                            out=vsb[:, 1024:2048].rearrange(
                                "p (b c) -> p b c", c=128),
                            in_=Vf[:, 1024:2048])
                    for c in INTER.get(q, ()):
                        emit_chunk(c)

            stg_cm.__exit__(None, None, None)

            # ---- phase A remainder ----
            with tc.tile_pool(name="psBi", bufs=1, space="PSUM") as psBi:
                holder["psBi"] = psBi
                for cidx in range(16, 64):
                    emit_chunk(cidx)
                while pe_q:
                    pe_q.pop(0)[1]()

                # Y (groups 0-3) drain to outTa.
                for g in range(4):
                    if g % 2 == 0:
                        nc.scalar.copy(outTa[:, g * 512:(g + 1) * 512],
                                       bankY[g][:])
                    else:
                        nc.vector.tensor_copy(
                            outTa[:, g * 512:(g + 1) * 512], bankY[g][:])
            psA_cm.__exit__(None, None, None)
            out2a3 = out2a.rearrange("p (b t) -> p b t", t=128)
            with tc.tile_pool(name="psA2", bufs=1, space="PSUM") as psA2, \
                 tc.tile_pool(name="psC", bufs=2, space="PSUM") as psC, \
                 tc.tile_pool(name="psY", bufs=2, space="PSUM") as psY, \
                 tc.tile_pool(name="late", bufs=2) as late:

                # proj2 Gs complete in pairs; each pair's two y-slices per
                # cb ride one strided DMA to halve the descriptor train.
                PAIRS = [(0, 3), (6, 1), (4, 7), (2, 5)]
                pair_of = {g: (pi, hi) for pi, p in enumerate(PAIRS)
                           for hi, g in enumerate(p)}
                ygt = {}

                pyt = {}

                def proj2(G):
                    runs = _proj2_runs(G)
                    pi, hi = pair_of[G]
                    for cb in range(2):
                        if hi == 0:
                            pyt[(pi, cb)] = psY.tile(
                                [128, 1024], F32, name="py", tag="py")
                        lo = min(PAIRS[pi])
                        hc = PAIRS[pi][hi] != lo
                        py = pyt[(pi, cb)][:, 512:1024] if hc else \
                            pyt[(pi, cb)][:, 0:512]
                        off = 0
                        for ri, run in enumerate(runs):
                            w = 128 * len(run)
                            rhs = out2a3[:, run[0]:run[-1] + 1:3, :]
                            nc.tensor.matmul(
                                py[:, off:off + w],
                                woutT[:, cb * 128:(cb + 1) * 128], rhs,
                                start=(ri == 0), stop=(ri == len(runs) - 1),
                                skip_group_check=True,
                            )
                            off += w
                        if hi == 1:
                            # one pair-wide bias + one strided DMA per cb
                            key = (pi % 2, cb)
                            yg = late.tile([128, 1024], BF16,
                                           name=f"yg{key}", tag=f"yg{key}")
                            if cb == 0:
                                nc.scalar.activation(yg[:], pyt[(pi, cb)][:],
                                                     AF.Identity, bias=bo[cb])
                            else:
                                nc.vector.tensor_scalar_add(
                                    yg[:], pyt[(pi, cb)][:], bo[cb])
                            d = (max(PAIRS[pi]) - lo) * 512
                            outap = y_ext[cb * 128:(cb + 1) * 128,
                                          lo * 512:].rearrange(
                                "p (a c) -> p a c", c=512)[:, 0:d // 512 + 1:
                                                           d // 512, :]
                            eng = [nc.sync, nc.scalar, nc.gpsimd][
                                (pi * 2 + cb) % 3]
                            eng.dma_start(out=outap, in_=yg[:])

                def tp2(g0):
                    # transpose a pair of 512-col groups; one wide drain
                    tpc = psC.tile([128, 1024], BF16, name="tpc", tag="tpc")
                    for s in range(8):
                        nc.tensor.transpose(
                            tpc[:, s * 128:(s + 1) * 128],
                            outTa[:, g0 * 512 + s * 128:
                                  g0 * 512 + (s + 1) * 128],
                            ident)
                    if g0 % 4 == 0:
                        nc.scalar.copy(
                            out2a[:, g0 * 512:(g0 + 2) * 512], tpc[:])
                    else:
                        nc.vector.tensor_scalar_add(
                            out2a[:, g0 * 512:(g0 + 2) * 512], tpc[:], 0.0)

                # Y groups (0-3) completed at phase-A end; X tails chain
                # jb SPLIT_JB+1..15 on psA's freed banks, draining with the
                # spill added; TAIL_ORDER interleaves everything.
                bA2 = [None] * 4

                def xchain(g):
                    bA2[g] = psA2.tile([128, 512], F32, name=f"bA2{g % 2}",
                                       tag=f"bA2{g % 2}")
                    for jb in range(SPLIT_JB + 1, 16):
                        out_mm(bA2[g], g + 4, jb, jb == SPLIT_JB + 1, jb == 15)

                def xdrain(g):
                    nc.vector.tensor_tensor(
                        out=outTa[:, (g + 4) * 512:(g + 5) * 512],
                        in0=bA2[g][:], in1=spill[g][:], op=AluOpType.add)

                for step in TAIL_ORDER:
                    kind, arg = step
                    if kind == "x":
                        xchain(arg)
                    elif kind == "d":
                        xdrain(arg)
                    elif kind == "p":
                        proj2(arg)
                    elif kind == "t":
                        tp2(arg)
                # remaining: G7 g{5,5,6,7}, G2 g{6,6,7,0}, G5 g{7,7,0,1}
                for G in (7, 2, 5):
                    proj2(G)

        if debug_hook:
            debug_hook(nc, dict(QK8s=QK8s, vsb=vsb,
                                P=P, zsum=zsum, zinv=zinv, outTa=outTa,
                                out2a=out2a))

    nc.compile()
    return nc


def get_nc():
    if "nc" not in _CACHE:
        _CACHE["nc"] = build_nc()
    return _CACHE["nc"]


def make_in_maps(x, W_qkv, b_qkv, W_out, b_out):
    x = np.asarray(x, dtype=np.float32)
    W_qkv = np.asarray(W_qkv, dtype=np.float32)
    b_qkv = np.asarray(b_qkv, dtype=np.float32)
    W_out = np.asarray(W_out, dtype=np.float32)
    b_out = np.asarray(b_out, dtype=np.float32)

    operm = (np.arange(O) + O // 2) % O      # rotate qkv channels by 192
    eperm = (np.arange(E) + E // 2) % E      # rotate e-axis by 64

    halves = []
    for h in range(2):
        if h == 0:
            wq, bqv, wo, bov = W_qkv, b_qkv, W_out, b_out
        else:
            wq = W_qkv[operm]
            bqv = b_qkv[operm]
            wo = W_out[:, eperm]
            bov = np.zeros_like(b_out)
        orders = [
            [3 * t + r for t in range(128)]
            + [3 * t + (r + 2) % 3 for t in range(64)]
            + [3 * t + (r + 1) % 3 for t in range(64)]
            for r in range(3)
        ]
        wqv = np.concatenate([wq.T[:, o] for o in orders], axis=1)     # (C, 768)
        brv = np.concatenate([bqv[o][None, :] for o in orders], axis=1)  # (1, 768)
        halves.append({
            "wqkvT": np.ascontiguousarray(wqv).astype(ml_dtypes.bfloat16),
            "brow": np.ascontiguousarray(brv).astype(ml_dtypes.bfloat16),
            "woutT": np.ascontiguousarray(wo.T).astype(ml_dtypes.bfloat16),
            "bout": np.ascontiguousarray(bov.reshape(C, 1)),
        })

    xb = [np.ascontiguousarray(x[n].reshape(C, HW)).astype(ml_dtypes.bfloat16)
          for n in range(N)]
    in_maps = []
    for core in range(8):
        n, h = core // 2, core % 2
        m = {"x": xb[n]}
        m.update(halves[h])
        in_maps.append(m)
    return in_maps


def run(inputs, trace=False, **kw):
    nc = get_nc()
    in_maps = make_in_maps(**inputs)
    res = run_bass_kernel_spmd(nc, in_maps, core_ids=list(range(8)), trace=trace, **kw)
    ys = [np.asarray(res.results[i]["out"], dtype=np.float32) for i in range(8)]
    y = np.stack([ys[2 * n] + ys[2 * n + 1] for n in range(N)])
    return y.reshape(N, C, 64, 64), res


def kernel(**inputs):
    y, _ = run(inputs, trace=False)
    return y
